# revision 1
# baseline (speedup 1.0000x reference)
"""Trainium2 Bass kernel for Transformer-XL style MHSA (nn_MHSAModule).

Problem (hardcoded):
  B=4, T=1024, D=512, H=8, DK=64, L=2*T-1=2047, eps=1e-3
  out = x + (MHSA(LayerNorm(x), pos) @ Wo + bo)

Sharding: 8 cores = 4 batches x 2 head-groups (4 heads each).
Core c handles batch c//2, heads 4*(c%2) .. 4*(c%2)+3. Each core returns a
partial output [T, D] (its heads' contribution); the host sums the two
partials per batch and adds the residual x + bo.

Device-side layout notes:
  - All activations are kept "transposed" (feature-major): xT/yT [D, T],
    posT [D, L]; projections produce qT/kT [DK, T] per head directly.
  - LayerNorm is computed in transposed space: column sums via ones-matmul
    on the PE, the per-token scale/shift rows are bounced through DRAM to
    replicate them across partitions.
  - gamma/beta are folded into the projection weights/biases on the host.
  - The Transformer-XL rel_shift is implemented by bouncing the positional
    score band [128, 1152] per q-block through DRAM (fp16) and reading it
    back with a skewed access pattern (stride L-1 trick), then adding it to
    the content scores in PSUM via an fp16 identity matmul.
  - Softmax is token-major: ACT computes exp(0.125*s) with a fused
    free-dim accumulation for the denominator; the reciprocal is applied to
    E as a per-partition tensor_scalar before the PE transposes E for the
    attention @ V matmul (contraction over keys requires keys on
    partitions).
"""
import numpy as np
from contextlib import ExitStack

import concourse.bass as bass
import concourse.bacc as bacc
import concourse.tile as tile
from concourse import mybir
from concourse import masks
from concourse.bass_utils import run_bass_kernel_spmd

F32 = mybir.dt.float32
F32R = mybir.dt.float32r
F16 = mybir.dt.float16
AF = mybir.ActivationFunctionType
OP = mybir.AluOpType

B, T, D, H, DK = 4, 1024, 512, 8, 64
L = 2 * T - 1
EPS = 1e-3
NH = 4          # heads per core
NP = 2          # head pairs per core
CH = D // 128   # 4 contraction chunks
QB = T // 128   # 8 q blocks
BAND = 1152     # positional band width per q block (>= T/8*... = 1151)
PL = L + 2      # padded pT free size (2 zero pad cols)


def _build_program() -> bass.Bass:
    nc = bacc.Bacc("TRN2", target_bir_lowering=False, debug=False)

    # ---- DRAM I/O ----
    xT = nc.dram_tensor("xT", [D, T], F32, kind="ExternalInput")
    posT = nc.dram_tensor("posT", [D, L], F32R, kind="ExternalInput")
    wq = nc.dram_tensor("wq", [D, NH * DK], F32R, kind="ExternalInput")
    wk = nc.dram_tensor("wk", [D, NH * DK], F32R, kind="ExternalInput")
    wv = nc.dram_tensor("wv", [D, NH * DK], F32R, kind="ExternalInput")
    wp = nc.dram_tensor("wp", [D, NH * DK], F32R, kind="ExternalInput")
    wo = nc.dram_tensor("wo", [DK, NH * D], F32R, kind="ExternalInput")
    qc_bias = nc.dram_tensor("qc_bias", [128, NP], F32, kind="ExternalInput")
    qp_bias = nc.dram_tensor("qp_bias", [128, NP], F32, kind="ExternalInput")
    k_bias = nc.dram_tensor("k_bias", [128, NP], F32, kind="ExternalInput")
    v_bias = nc.dram_tensor("v_bias", [NH * DK], F32, kind="ExternalInput")
    out_d = nc.dram_tensor("out_partial", [T, D], F32, kind="ExternalOutput")

    # internal scratch
    bounce = nc.dram_tensor("bounce", [2, QB, 128, BAND], F16)
    lnrows = nc.dram_tensor("lnrows", [2, T], F32)

    with tile.TileContext(nc) as tc, ExitStack() as ctx:
        sb = ctx.enter_context(tc.tile_pool(name="sb", bufs=1))
        sb2 = ctx.enter_context(tc.tile_pool(name="sb2", bufs=2))
        ps_sc = ctx.enter_context(tc.tile_pool(name="ps_sc", bufs=1, space="PSUM"))
        ps_b = ctx.enter_context(tc.tile_pool(name="ps_b", bufs=1, space="PSUM"))
        ps_m = ctx.enter_context(tc.tile_pool(name="ps_m", bufs=2, space="PSUM"))

        # ---- persistent SBUF ----
        xT_sb = sb.tile([128, CH * T], F32, tag="bigshared")
        yT_sb = sb.tile([128, CH * T], F32R)
        posT_sb = sb.tile([128, CH * L + 2], F32R)
        pT_sb = sb.tile([128, NP * PL], F32R)
        qcT_sb = sb.tile([128, NP * T], F32R)
        qpT_sb = sb.tile([128, NP * T], F32R)
        kT_sb = sb.tile([128, NP * T], F32R)
        v_sb = sb.tile([128, QB * NH * DK], F16)
        oT_sb = sb.tile([64, NH * T], F32R)
        wq_sb = sb.tile([128, CH * 256], F32R)
        wk_sb = sb.tile([128, CH * 256], F32R)
        wv_sb = sb.tile([128, CH * 256], F32R)
        wp_sb = sb.tile([128, CH * 256], F32R)
        wo_sb = sb.tile([64, NH * D], F32R)
        qcb_sb = sb.tile([128, NP], F32)
        qpb_sb = sb.tile([128, NP], F32)
        kb_sb = sb.tile([128, NP], F32)
        vb_sb = sb.tile([128, 256], F32)
        arep = sb.tile([128, T], F32)
        brep = sb.tile([128, T], F32)
        ident16 = sb.tile([128, 128], F16)
        ones_col = sb.tile([128, 1], F32)
        eps_col = sb.tile([1, 1], F32)

        masks.make_identity(nc, ident16[:])
        nc.vector.memset(ones_col[:], 1.0)
        nc.vector.memset(eps_col[:], EPS)

        # ---- loads ----
        for c in range(CH):
            nc.sync.dma_start(xT_sb[:, c * T:(c + 1) * T],
                              xT[c * 128:(c + 1) * 128, :])
            nc.sync.dma_start(posT_sb[:, c * L:(c + 1) * L],
                              posT[c * 128:(c + 1) * 128, :])
            for w_sb, w_d in ((wq_sb, wq), (wk_sb, wk), (wv_sb, wv),
                              (wp_sb, wp)):
                nc.sync.dma_start(w_sb[:, c * 256:(c + 1) * 256],
                                  w_d[c * 128:(c + 1) * 128, :])
        nc.sync.dma_start(wo_sb[:], wo[:])
        nc.sync.dma_start(qcb_sb[:], qc_bias[:])
        nc.sync.dma_start(qpb_sb[:], qp_bias[:])
        nc.sync.dma_start(kb_sb[:], k_bias[:])
        nc.sync.dma_start(
            vb_sb[:], bass.AP(v_bias[:].tensor, 0, [[0, 128], [1, 256]]))

        # ---- LayerNorm stats (transposed space) ----
        mu = sb.tile([1, 512], F32)
        ex2 = sb.tile([1, 512], F32)
        var = sb.tile([1, 512], F32)
        std = sb.tile([1, 512], F32)
        a_row = sb.tile([1, 512], F32)
        b_row = sb.tile([1, 512], F32)
        for tt in range(2):
            sums_ps = ps_m.tile([1, 512], F32, tag="misc")
            for c in range(CH):
                xt = xT_sb[:, c * T + tt * 512: c * T + tt * 512 + 512]
                nc.tensor.matmul(sums_ps[:], ones_col[:],
                                 xt,
                                 start=(c == 0), stop=(c == CH - 1))
            nc.vector.tensor_scalar_mul(mu[:], sums_ps[:], 1.0 / D)
            sumsq_ps = ps_m.tile([1, 512], F32, tag="misc")
            for c in range(CH):
                xsq = sb2.tile([128, 512], F32, tag="xsq")
                xt = xT_sb[:, c * T + tt * 512: c * T + tt * 512 + 512]
                nc.scalar.activation(xsq[:], xt, AF.Square)
                nc.tensor.matmul(sumsq_ps[:], ones_col[:],
                                 xsq[:],
                                 start=(c == 0), stop=(c == CH - 1))
            nc.vector.tensor_scalar_mul(ex2[:], sumsq_ps[:], 1.0 / D)
            nc.vector.tensor_tensor(var[:], mu[:], mu[:], op=OP.mult)
            nc.vector.tensor_tensor(var[:], ex2[:], var[:], op=OP.subtract)
            nc.scalar.activation(std[:], var[:], AF.Sqrt, bias=eps_col[:])
            nc.vector.reciprocal(a_row[:], std[:])
            nc.vector.tensor_tensor(b_row[:], mu[:], a_row[:], op=OP.mult)
            nc.vector.tensor_scalar_mul(b_row[:], b_row[:], -1.0)
            nc.sync.dma_start(lnrows[0, tt * 512:(tt + 1) * 512], a_row[:])
            nc.sync.dma_start(lnrows[1, tt * 512:(tt + 1) * 512], b_row[:])
        nc.sync.dma_start(arep[:],
                          bass.AP(lnrows[:].tensor, 0, [[0, 128], [1, T]]))
        nc.sync.dma_start(brep[:],
                          bass.AP(lnrows[:].tensor, T, [[0, 128], [1, T]]))

        # ---- LayerNorm apply: yT = xT * a + b ----
        for c in range(CH):
            for tt in range(2):
                xs = xT_sb[:, c * T + tt * 512: c * T + tt * 512 + 512]
                ys = yT_sb[:, c * T + tt * 512: c * T + tt * 512 + 512]
                ar = arep[:, tt * 512:(tt + 1) * 512]
                br = brep[:, tt * 512:(tt + 1) * 512]
                nc.vector.tensor_tensor(ys, xs, ar, op=OP.mult)
                nc.vector.tensor_tensor(ys, ys, br, op=OP.add)

        # ---- q/k projections (per head pair) ----
        for p in range(NP):
            for nt in range(2):
                for which, w_sb, dst, bias in (
                    ("q", wq_sb, None, None),
                    ("k", wk_sb, kT_sb, kb_sb),
                ):
                    prj = ps_m.tile([128, 512], F32, tag="misc")
                    for c in range(CH):
                        nc.tensor.matmul(
                            prj[:],
                            w_sb[:, c * 256 + p * 128: c * 256 + p * 128 + 128
                                 ],
                            yT_sb[:, c * T + nt * 512: c * T + nt * 512 + 512
                                  ],
                            start=(c == 0), stop=(c == CH - 1))
                    o = p * T + nt * 512
                    if which == "q":
                        nc.scalar.activation(
                            qcT_sb[:, o:o + 512], prj[:], AF.Identity,
                            bias=qcb_sb[:, p:p + 1])
                        nc.scalar.activation(
                            qpT_sb[:, o:o + 512], prj[:], AF.Identity,
                            bias=qpb_sb[:, p:p + 1])
                    else:
                        nc.scalar.activation(
                            dst[:, o:o + 512], prj[:], AF.Identity,
                            bias=bias[:, p:p + 1])

        # ---- v projection (token-major) ----
        for t8 in range(QB):
            vps = ps_m.tile([128, 256], F32, tag="misc")
            for c in range(CH):
                nc.tensor.matmul(
                    vps[:],
                    yT_sb[:, c * T + t8 * 128: c * T + t8 * 128 + 128
                          ],
                    wv_sb[:, c * 256:(c + 1) * 256],
                    start=(c == 0), stop=(c == CH - 1))
            nc.vector.tensor_tensor(
                v_sb[:, t8 * 256:(t8 + 1) * 256], vps[:], vb_sb[:],
                op=OP.add)

        # ---- p projection ----
        # last tile reads one column past L (junk, lands in the pad column
        # of pT which is re-zeroed below); posT_sb has 2 junk columns
        zrow = sb.tile([128, 2], F32)
        nc.vector.memset(zrow[:], 0.0)
        nc.vector.tensor_copy(posT_sb[:, CH * L:], zrow[:])
        for p in range(NP):
            for nt in range(4):
                pps = ps_m.tile([128, 512], F32, tag="misc")
                for c in range(CH):
                    nc.tensor.matmul(
                        pps[:],
                        wp_sb[:, c * 256 + p * 128: c * 256 + p * 128 + 128
                              ],
                        posT_sb[:, c * L + nt * 512: c * L + nt * 512 + 512
                                ],
                        start=(c == 0), stop=(c == CH - 1))
                nc.scalar.copy(
                    pT_sb[:, p * PL + nt * 512: p * PL + nt * 512 + 512],
                    pps[:])
        for p in range(NP):
            nc.vector.tensor_copy(pT_sb[:, p * PL + L: (p + 1) * PL], zrow[:])

        # ---- attention per head ----
        for h in range(NH):
            p = h // 2
            off = (h % 2) * 64
            ping = h % 2
            qp_h = lambda lo, w: qpT_sb[off:off + 64, p * T + lo: p * T + lo + w]
            qc_h = lambda lo, w: qcT_sb[off:off + 64, p * T + lo: p * T + lo + w]
            k_h = lambda lo, w: kT_sb[off:off + 64, p * T + lo: p * T + lo + w]
            p_h = lambda lo, w: pT_sb[off:off + 64, p * PL + lo: p * PL + lo + w]

            # positional band scores + bounce out
            for qb in range(QB):
                s0 = 897 - qb * 128
                bps = ps_b.tile([128, BAND], F32, tag="band")
                for bt, w in enumerate((512, 512, 128)):
                    nc.tensor.matmul(
                        bps[:, bt * 512: bt * 512 + w],
                        qp_h(qb * 128, 128),
                        p_h(s0 + bt * 512, w),
                        start=True, stop=True)
                b16 = sb2.tile([128, BAND], F16, tag="band16")
                nc.vector.tensor_copy(b16[:], bps[:])
                nc.sync.dma_start(bounce[ping, qb], b16[:])

            # skewed (rel_shift) read back: one DMA for the whole head
            shifted = sb.tile([128, QB * T], F16, tag="bigshared")
            src = bass.AP(bounce[:].tensor,
                          ping * (QB * 128 * BAND) + 127,
                          [[BAND - 1, 128], [128 * BAND, QB], [1, T]])
            nc.sync.dma_start(shifted[:], src)

            for qbp in range(QB // 2):
                E_sb = sb2.tile([128, 2 * T], F16, tag="E")
                den = sb2.tile([128, 2], F32, tag="den")
                rec = sb2.tile([128, 2], F32, tag="rec")
                for qi in range(2):
                    qb = qbp * 2 + qi
                    sps = ps_sc.tile([128, T], F32, tag="scores")
                    for nt in range(2):
                        nc.tensor.matmul(
                            sps[:, nt * 512: nt * 512 + 512],
                            qc_h(qb * 128, 128),
                            k_h(nt * 512, 512),
                            start=True, stop=False)
                        if qb == 0 and nt == 1:
                            # scores[0, 1023] += (q+pos_bias)[1] . p[0]
                            nc.tensor.matmul(
                                sps[0:1, 1023:1024],
                                qp_h(1, 1).bitcast(F32),
                                p_h(0, 1).bitcast(F32),
                                start=False, stop=False)
                        nc.tensor.matmul(
                            sps[:, nt * 512: nt * 512 + 512],
                            ident16[:],
                            shifted[:, qb * T + nt * 512: qb * T + nt * 512 + 512],
                            start=False, stop=True)
                    nc.scalar.activation(
                        E_sb[:, qi * T:(qi + 1) * T], sps[:], AF.Exp,
                        scale=0.125, accum_out=den[:, qi:qi + 1])
                    nc.vector.reciprocal(rec[:, qi:qi + 1], den[:, qi:qi + 1])
                    nc.vector.tensor_scalar_mul(
                        E_sb[:, qi * T:(qi + 1) * T],
                        E_sb[:, qi * T:(qi + 1) * T],
                        rec[:, qi:qi + 1])
                # transpose E (fp16) -> ET [keys, 256], one psum bank at a time
                ET_sb = sb2.tile([128, QB * 256], F16, tag="ET")
                for half in range(2):
                    etps = ps_b.tile([128, 4 * 256], F16, tag="et")
                    for qi in range(2):
                        for kc in range(4):
                            kca = half * 4 + kc
                            nc.tensor.transpose(
                                etps[:, kc * 256 + qi * 128: kc * 256 + qi * 128 + 128],
                                E_sb[:, qi * T + kca * 128: qi * T + kca * 128 + 128],
                                ident16[:])
                    if half == 0:
                        nc.vector.tensor_copy(
                            ET_sb[:, :1024], etps[:])
                    else:
                        nc.scalar.copy(ET_sb[:, 1024:], etps[:])
                # attention @ V -> oT [64, 256]
                otps = ps_m.tile([64, 256], F32, tag="misc")
                for kc in range(QB):
                    nc.tensor.matmul(
                        otps[:],
                        v_sb[:, kc * 256 + h * 64: kc * 256 + h * 64 + 64],
                        ET_sb[:, kc * 256:(kc + 1) * 256],
                        start=(kc == 0), stop=(kc == QB - 1))
                nc.vector.tensor_copy(
                    oT_sb[:, h * T + qbp * 256: h * T + qbp * 256 + 256],
                    otps[:])

        # ---- output projection ----
        for t8 in range(QB):
            ops_ = ps_m.tile([128, 512], F32, tag="misc")
            for h in range(NH):
                nc.tensor.matmul(
                    ops_[:],
                    oT_sb[:, h * T + t8 * 128: h * T + t8 * 128 + 128
                          ],
                    wo_sb[:, h * D:(h + 1) * D],
                    start=(h == 0), stop=(h == NH - 1))
            osb = sb2.tile([128, 512], F32, tag="osb")
            nc.vector.tensor_copy(osb[:], ops_[:])
            nc.sync.dma_start(out_d[t8 * 128:(t8 + 1) * 128, :], osb[:])

    nc.compile()
    return nc


_PROGRAM_CACHE: dict = {}


def _get_program() -> bass.Bass:
    if "nc" not in _PROGRAM_CACHE:
        _PROGRAM_CACHE["nc"] = _build_program()
    return _PROGRAM_CACHE["nc"]


def _prepare_in_maps(x, pos, content_bias, pos_bias, gamma, beta,
                     Wq, bq, Wk, bk, Wv, bv, Wp, Wo, bo):
    x = np.asarray(x, np.float32)
    pos = np.asarray(pos, np.float32)
    gamma = np.asarray(gamma, np.float32)
    beta = np.asarray(beta, np.float32)

    # gamma folding: y = yln*gamma + beta  =>  y@W = yln@(gamma*W) + beta@W
    def fold(W):
        W = np.asarray(W, np.float32)
        return W * gamma[:, None, None], np.einsum("d,dhk->hk", beta, W)

    Wq_f, bq_f = fold(Wq)
    Wk_f, bk_f = fold(Wk)
    Wv_f, bv_f = fold(Wv)
    Wp = np.asarray(Wp, np.float32)
    Wo = np.asarray(Wo, np.float32)

    in_maps = []
    for core in range(8):
        b = core // 2
        g = core % 2
        hs = slice(4 * g, 4 * g + 4)
        qcb = (np.asarray(bq) + np.asarray(content_bias) + bq_f)[hs]
        qpb = (np.asarray(bq) + np.asarray(pos_bias) + bq_f)[hs]
        kb = (np.asarray(bk) + bk_f)[hs]
        vb = (np.asarray(bv) + bv_f)[hs]
        in_maps.append({
            "xT": np.ascontiguousarray(x[b].T),
            "posT": np.ascontiguousarray(pos[b].T),
            "wq": np.ascontiguousarray(Wq_f[:, hs, :].reshape(D, NH * DK)),
            "wk": np.ascontiguousarray(Wk_f[:, hs, :].reshape(D, NH * DK)),
            "wv": np.ascontiguousarray(Wv_f[:, hs, :].reshape(D, NH * DK)),
            "wp": np.ascontiguousarray(Wp[:, hs, :].reshape(D, NH * DK)),
            "wo": np.ascontiguousarray(
                np.asarray(Wo)[hs].transpose(1, 0, 2).reshape(DK, NH * D)),
            "qc_bias": np.ascontiguousarray(qcb.reshape(2, 128).T),
            "qp_bias": np.ascontiguousarray(qpb.reshape(2, 128).T),
            "k_bias": np.ascontiguousarray(kb.reshape(2, 128).T),
            "v_bias": np.ascontiguousarray(vb.reshape(NH * DK)),
        })

    return in_maps


def _combine(x, bo, results):
    parts = [r["out_partial"] for r in results]
    out = np.asarray(x, np.float32) + np.asarray(bo, np.float32)[None, None, :]
    for b in range(B):
        out[b] += parts[2 * b] + parts[2 * b + 1]
    return out.astype(np.float32)


def kernel(x, pos, content_bias, pos_bias, gamma, beta,
           Wq, bq, Wk, bk, Wv, bv, Wp, Wo, bo) -> np.ndarray:
    in_maps = _prepare_in_maps(x, pos, content_bias, pos_bias, gamma, beta,
                               Wq, bq, Wk, bk, Wv, bv, Wp, Wo, bo)
    nc = _get_program()
    res = run_bass_kernel_spmd(nc, in_maps, core_ids=list(range(8)))
    return _combine(x, bo, res.results)



# revision 2
# speedup vs baseline: 139.0505x; 139.0505x over previous
"""Trainium2 Bass kernel v2 for Transformer-XL style MHSA (nn_MHSAModule).

Problem (hardcoded):
  B=4, T=1024, D=512, H=8, DK=64, L=2*T-1=2047, eps=1e-3
  out = x + (MHSA(LayerNorm(x), pos) @ Wo + bo)

Sharding: 8 cores = 4 batches x 2 head-groups (4 heads each). Each core
returns a partial output [T, D] f16 (its heads' contribution); the host
sums the two partials per batch and adds the residual x + bo.

v2 design notes (vs v1):
  - f16 activations/weights on the matmul path; f8e4m3 only for matmul
    B-operands no vector engine reads back: kT, pT, and the rel-shift
    band bounce (halves its DMA volume).
  - exp runs on Act straight from PSUM with accum_out giving the softmax
    denominator for free; E is normalized by one per-partition f16
    tensor_scalar instead of v1's psum-copy + scale chain.
  - PSUM extraction is the scarce resource (~4x an SBUF read): band
    extraction is split across DVE/Act/Pool, scores are extracted by the
    exp itself, ET extracted as f16 (2x cheaper than f32).
  - LayerNorm stats via f16 ones-matmuls; a/b rows reach all partitions
    via gpsimd.partition_broadcast instead of a DRAM round-trip.
  - attention@V stacks head pairs on 128 PSUM partitions; Wo is
    pair-stacked so the output projection contracts 128 rows per step.
  - engines execute in-order, so emission order IS the schedule: x/wq
    load first, the band of head h+1 and the attn@V of the previous pair
    are interleaved into head h's per-q-block score loop.
"""
import numpy as np
from contextlib import ExitStack

import concourse.bass as bass
import concourse.bacc as bacc
import concourse.tile as tile
from concourse import mybir
from concourse import masks
from concourse.bass_utils import run_bass_kernel_spmd

F32 = mybir.dt.float32
F16 = mybir.dt.float16
F8 = mybir.dt.float8e4
AF = mybir.ActivationFunctionType
OP = mybir.AluOpType

B, T, D, H, DK = 4, 1024, 512, 8, 64
L = 2 * T - 1
EPS = 1e-3
NH = 4          # heads per core
NP = 2          # head pairs per core
CH = D // 128   # 4 contraction chunks
QB = T // 128   # 8 q blocks
BAND = 1152    # positional band width per q block
PL = L + 2      # padded pT free size (2 zero pad cols)


def _build_program() -> bass.Bass:
    nc = bacc.Bacc("TRN2", target_bir_lowering=False, debug=False)

    # ---- DRAM I/O ----
    xT = nc.dram_tensor("xT", [D, T], F16, kind="ExternalInput")
    posT = nc.dram_tensor("posT", [D, L], F16, kind="ExternalInput")
    wq = nc.dram_tensor("wq", [D, NH * DK], F16, kind="ExternalInput")
    wk = nc.dram_tensor("wk", [D, NH * DK], F16, kind="ExternalInput")
    wv = nc.dram_tensor("wv", [D, NH * DK], F16, kind="ExternalInput")
    wp = nc.dram_tensor("wp", [D, NH * DK], F16, kind="ExternalInput")
    wo = nc.dram_tensor("wo", [128, NP * D], F16, kind="ExternalInput")
    qc_bias = nc.dram_tensor("qc_bias", [128, NP], F32, kind="ExternalInput")
    qp_bias = nc.dram_tensor("qp_bias", [128, NP], F32, kind="ExternalInput")
    k_bias = nc.dram_tensor("k_bias", [128, NP], F32, kind="ExternalInput")
    v_bias = nc.dram_tensor("v_bias", [NH * DK], F32, kind="ExternalInput")
    out_d = nc.dram_tensor("out_partial", [T, D], F16, kind="ExternalOutput")

    # internal scratch: rel-shift bounce, f8, double buffered
    bounce = nc.dram_tensor("bounce", [2, QB, 128, BAND], F8)

    with tile.TileContext(nc) as tc, ExitStack() as ctx:
        sb = ctx.enter_context(tc.tile_pool(name="sb", bufs=1))
        sb2 = ctx.enter_context(tc.tile_pool(name="sb2", bufs=2))
        sb3 = ctx.enter_context(tc.tile_pool(name="sb3", bufs=3))
        sbE = ctx.enter_context(tc.tile_pool(name="sbE", bufs=4))
        # PSUM: sc 2x[128,512]f32 (2 banks) + band 1x[128,1152]f32
        # (3 banks) + et 2x[128,1024]f16 (2 banks) + o 1x[128,512]f32
        # (1 bank) = 8 banks.
        ps_sc = ctx.enter_context(tc.tile_pool(name="ps_sc", bufs=2,
                                               space="PSUM"))
        # band psum: three independently-released tiles so the next band
        # matmul only waits on the one engine that extracts each slice
        ps_b = ctx.enter_context(tc.tile_pool(name="ps_b", bufs=1,
                                              space="PSUM"))
        ps_et = ctx.enter_context(tc.tile_pool(name="ps_et", bufs=1,
                                               space="PSUM"))
        ps_o = ctx.enter_context(tc.tile_pool(name="ps_o", bufs=1,
                                              space="PSUM"))

        # ---- persistent SBUF ----
        xT_sb = sb.tile([128, CH * T], F16)
        yT_sb = sb.tile([128, CH * T], F16)
        posT_sb = sb.tile([128, CH * L + 2], F16)
        pT_sb = sb.tile([128, NP * PL], F8)
        qcT_sb = sb.tile([128, NP * T], F8)
        qpT_sb = sb.tile([128, NP * T], F8)
        kT_sb = sb.tile([128, NP * T], F8)
        v_sb = sb.tile([128, QB * NH * DK], F16)
        oT_sb = sb.tile([128, NP * T], F16)
        wq_sb = sb.tile([128, CH * 256], F16)
        wk_sb = sb.tile([128, CH * 256], F16)
        wv_sb = sb.tile([128, CH * 256], F16)
        wp_sb = sb.tile([128, CH * 256], F16)
        wo_sb = sb.tile([128, NP * D], F16)
        qcb_sb = sb.tile([128, NP], F32)
        qpb_sb = sb.tile([128, NP], F32)
        kb_sb = sb.tile([128, NP], F32)
        vb_sb = sb.tile([128, 256], F32)
        arep = sb.tile([128, T], F32)
        brep = sb.tile([128, T], F32)
        ident16 = sb.tile([128, 128], F16)
        ident8 = sb.tile([128, 128], F8)
        ones_col = sb.tile([128, 1], F16)
        eps_col = sb.tile([1, 1], F32)

        nc.vector.memset(ones_col[:], 1.0)
        nc.vector.memset(eps_col[:], EPS)

        # ---- loads: x first (stats), then pos+wp (p proj), then q/k ----
        for c in range(CH):
            nc.sync.dma_start(xT_sb[:, c * T:(c + 1) * T],
                              xT[c * 128:(c + 1) * 128, :])
        for c in range(CH):
            nc.sync.dma_start(posT_sb[:, c * L:(c + 1) * L],
                              posT[c * 128:(c + 1) * 128, :])
            nc.sync.dma_start(wp_sb[:, c * 256:(c + 1) * 256],
                              wp[c * 128:(c + 1) * 128, :])
        for c in range(CH):
            nc.sync.dma_start(wq_sb[:, c * 256:(c + 1) * 256],
                              wq[c * 128:(c + 1) * 128, :])
            nc.sync.dma_start(wk_sb[:, c * 256:(c + 1) * 256],
                              wk[c * 128:(c + 1) * 128, :])
        nc.sync.dma_start(qcb_sb[:], qc_bias[:])
        nc.sync.dma_start(qpb_sb[:], qp_bias[:])
        nc.sync.dma_start(kb_sb[:], k_bias[:])
        for c in range(CH):
            nc.sync.dma_start(wv_sb[:, c * 256:(c + 1) * 256],
                              wv[c * 128:(c + 1) * 128, :])
        nc.sync.dma_start(wo_sb[:], wo[:])
        nc.sync.dma_start(
            vb_sb[:], bass.AP(v_bias[:].tensor, 0, [[0, 128], [1, 256]]))

        # ---- LayerNorm stats; short [1,512]-row chain (row ops are
        # lane-serial, so every op counts): a = rsqrt(var+eps),
        # b = -mu*a, computed as
        #   t1 = s1*s1; v' = D*s2 - t1; a = Rsqrt(v'/D^2 + eps);
        #   b = (s1 * -1/D) * a
        a_row = sb.tile([1, T], F32)
        b_row = sb.tile([1, T], F32)
        for tt in range(2):
            s1 = ps_b.tile([1, 512], F32, tag="band0")
            for c in range(CH):
                xt = xT_sb[:, c * T + tt * 512: c * T + tt * 512 + 512]
                nc.tensor.matmul(s1[:], ones_col[:], xt,
                                 start=(c == 0), stop=(c == CH - 1))
            s2 = ps_b.tile([1, 512], F32, tag="band1")
            for c in range(CH):
                xsq = sb3.tile([128, 512], F16, tag="xsq", name="xsq")
                xt = xT_sb[:, c * T + tt * 512: c * T + tt * 512 + 512]
                nc.vector.tensor_tensor(xsq[:], xt, xt, op=OP.mult)
                nc.tensor.matmul(s2[:], ones_col[:], xsq[:],
                                 start=(c == 0), stop=(c == CH - 1))
            t1 = sb2.tile([1, 512], F32, tag="t1", name="t1")
            nc.scalar.activation(t1[:], s1[:], AF.Square)
            vv = sb2.tile([1, 512], F32, tag="vv", name="vv")
            nc.vector.scalar_tensor_tensor(vv[:], s2[:], float(D), t1[:],
                                           op0=OP.mult, op1=OP.subtract)
            stdh = sb2.tile([1, 512], F32, tag="stdh", name="stdh")
            nc.scalar.activation(stdh[:], vv[:], AF.Sqrt,
                                 scale=1.0 / (D * D), bias=eps_col[:])
            a_half = a_row[:, tt * 512:(tt + 1) * 512]
            nc.vector.reciprocal(a_half, stdh[:])
            nc.vector.scalar_tensor_tensor(
                b_row[:, tt * 512:(tt + 1) * 512], s1[:], -1.0 / D, a_half,
                op0=OP.mult, op1=OP.mult)
        nc.gpsimd.partition_broadcast(arep[:], a_row[:])
        nc.gpsimd.partition_broadcast(brep[:], b_row[:])
        masks.make_identity(nc, ident16[:])
        masks.make_identity(nc, ident8[:])

        # ---- LayerNorm apply: yT = xT * a + b (f16), 512-col pieces so
        # the first projections start after the nt=0 halves ----
        for nt in range(2):
            for c in range(CH):
                eng = nc.vector if c % 2 == 0 else nc.gpsimd
                xs = xT_sb[:, c * T + nt * 512: c * T + nt * 512 + 512]
                ys = yT_sb[:, c * T + nt * 512: c * T + nt * 512 + 512]
                ar = arep[:, nt * 512:(nt + 1) * 512]
                br = brep[:, nt * 512:(nt + 1) * 512]
                eng.tensor_tensor(ys, xs, ar, op=OP.mult)
                eng.tensor_tensor(ys, ys, br, op=OP.add)

        # pad pT columns
        z8 = sb.tile([128, 4], F8)
        nc.vector.memset(z8[:], 0.0)
        zrow = sb.tile([128, 2], F16)
        nc.vector.memset(zrow[:], 0.0)
        nc.vector.tensor_copy(posT_sb[:, CH * L:], zrow[:])

        # ---- projection emitters ----
        # extraction engines are spread (qc->Act, qp->DVE, k/p->Pool,
        # v->DVE) so consecutive users of the psum ping-pong release in
        # parallel queues.
        def q_proj(p, nt):
            prj = ps_sc.tile([128, 512], F32, tag="sc")
            for ci, c in enumerate((0, 2, 1, 3)):
                nc.tensor.matmul(
                    prj[:],
                    wq_sb[:, c * 256 + p * 128: c * 256 + p * 128 + 128],
                    yT_sb[:, c * T + nt * 512: c * T + nt * 512 + 512],
                    start=(c == 0), stop=(c == CH - 1))
            o = p * T + nt * 512
            nc.scalar.activation(qcT_sb[:, o:o + 512], prj[:],
                                 AF.Identity, bias=qcb_sb[:, p:p + 1])
            nc.vector.tensor_scalar_add(qpT_sb[:, o:o + 512], prj[:],
                                        qpb_sb[:, p:p + 1])

        def k_proj(p, nt):
            prjk = ps_sc.tile([128, 512], F32, tag="sc")
            for ci, c in enumerate((0, 2, 1, 3)):
                nc.tensor.matmul(
                    prjk[:],
                    wk_sb[:, c * 256 + p * 128: c * 256 + p * 128 + 128],
                    yT_sb[:, c * T + nt * 512: c * T + nt * 512 + 512],
                    start=(ci == 0), stop=(ci == CH - 1))
            nc.vector.tensor_scalar_add(kT_sb[:, p * T + nt * 512:
                                              p * T + nt * 512 + 512],
                                        prjk[:], kb_sb[:, p:p + 1])

        def p_proj(p, nt):
            pps = ps_sc.tile([128, 512], F32, tag="sc")
            for ci, c in enumerate((0, 2, 1, 3)):
                nc.tensor.matmul(
                    pps[:],
                    wp_sb[:, c * 256 + p * 128: c * 256 + p * 128 + 128],
                    posT_sb[:, c * L + nt * 512: c * L + nt * 512 + 512],
                    start=(ci == 0), stop=(ci == CH - 1))
            nc.scalar.copy(
                pT_sb[:, p * PL + nt * 512: p * PL + nt * 512 + 512],
                pps[:])
            if nt == 3:
                nc.gpsimd.tensor_copy(pT_sb[:, p * PL + L: (p + 1) * PL],
                                      z8[:, :PL - L])

        def v_proj(t8):
            vps = ps_sc.tile([128, 256], F32, tag="sc")
            for c in range(CH):
                nc.tensor.matmul(
                    vps[:],
                    yT_sb[:, c * T + t8 * 128: c * T + t8 * 128 + 128],
                    wv_sb[:, c * 256:(c + 1) * 256],
                    start=(c == 0), stop=(c == CH - 1))
            nc.vector.tensor_tensor(
                v_sb[:, t8 * 256:(t8 + 1) * 256], vps[:], vb_sb[:],
                op=OP.add)

        # ---- attention emitters ----
        def hsl(tile_, h, lo, w, stride=T):
            p, off = h // 2, (h % 2) * 64
            return tile_[off:off + 64, p * stride + lo: p * stride + lo + w]

        def band_qb(h, qb, shifted):
            """Band scores for (h, qb), bounce out, and the skewed read of
            this qb's block straight back into `shifted`. Three psum tiles,
            one extraction engine each, so the next band matmul waits on
            exactly one engine per tile."""
            ping = h % 2
            s0 = 897 - qb * 128
            qp_blk = hsl(qpT_sb, h, qb * 128, 128)
            bps0 = ps_b.tile([128, 768], F32, tag="band0")
            nc.tensor.matmul(bps0[:, :512], qp_blk,
                             hsl(pT_sb, h, s0, 512, stride=PL),
                             start=True, stop=True)
            nc.tensor.matmul(bps0[:, 512:], qp_blk,
                             hsl(pT_sb, h, s0 + 512, 256, stride=PL),
                             start=True, stop=True)
            bps1 = ps_b.tile([128, 384], F32, tag="band1")
            nc.tensor.matmul(bps1[:], qp_blk,
                             hsl(pT_sb, h, s0 + 768, 384, stride=PL),
                             start=True, stop=True)
            b8 = sb2.tile([128, BAND], F8, tag="band8", name="b8")
            if h == 0:
                # Act is exp-free before head 0's scores; give it the
                # wide slice so the DVE prefix backlog stays short
                nc.scalar.copy(b8[:, :768], bps0[:])
                nc.vector.tensor_copy(b8[:, 768:], bps1[:])
            else:
                nc.vector.tensor_copy(b8[:, :768], bps0[:])
                nc.scalar.copy(b8[:, 768:], bps1[:])
            nc.sync.dma_start(bounce[ping, qb], b8[:])
            src = bass.AP(bounce[:].tensor,
                          (ping * QB + qb) * (128 * BAND) + 127,
                          [[BAND - 1, 128], [1, T]])
            nc.sync.dma_start(shifted[:, qb * T:(qb + 1) * T], src)

        def shifted_tile():
            shifted = sb2.tile([128, QB * T], F8, tag="shifted",
                               name="shifted")
            return shifted

        def scores_qb(h, qb, shifted):
            """Scores+exp+normalize for (h, qb); returns the E tile.
            Transposes are emitted one qb later (transpose_qb) so the PE
            queue never waits on the Act/DVE chain of the same qb."""
            E_sb = sbE.tile([128, T], F16, tag="E", name="E")
            den = sb2.tile([128, 2], F32, tag="den", name="den")
            for half in range(2):
                sps = ps_sc.tile([128, 512], F32, tag="sc")
                nc.tensor.matmul(
                    sps[:],
                    hsl(qcT_sb, h, qb * 128, 128),
                    hsl(kT_sb, h, half * 512, 512),
                    start=True, stop=False)
                if qb == 0 and half == 1:
                    # scores[0, 1023] += (q+pos_bias)[1] . p[0]
                    # (the reference rel_shift reshape wraps this element)
                    nc.tensor.matmul(
                        sps[0:1, 511:512],
                        hsl(qpT_sb, h, 1, 1),
                        hsl(pT_sb, h, 0, 1, stride=PL),
                        start=False, stop=False)
                nc.tensor.matmul(
                    sps[:],
                    ident8[:],
                    shifted[:, qb * T + half * 512:
                            qb * T + half * 512 + 512],
                    start=False, stop=True)
                nc.scalar.activation(
                    E_sb[:, half * 512:(half + 1) * 512], sps[:], AF.Exp,
                    scale=0.125, accum_out=den[:, half:half + 1])
            rec = sb2.tile([128, 1], F32, tag="rec", name="rec")
            nc.gpsimd.tensor_tensor(rec[:], den[:, 0:1], den[:, 1:2],
                                    op=OP.add)
            nc.vector.reciprocal(rec[:], rec[:])
            nc.vector.tensor_scalar_mul(E_sb[:], E_sb[:], rec[:])
            return E_sb

        def transpose_qb(qb, E_sb, ET_sb):
            # transpose E (f16): 8 PE transposes -> one f16 psum bank
            etps = ps_et.tile([128, T], F16, tag="et")
            for kc in range(QB):
                nc.tensor.transpose(
                    etps[:, kc * 128: kc * 128 + 128],
                    E_sb[:, kc * 128: kc * 128 + 128],
                    ident16[:])
            # scatter-extract: ET_sb[:, kc*T + qb*128 ...] = etps block kc
            dst = bass.AP(ET_sb[:].tensor, qb * 128,
                          [[QB * T, 128], [T, QB], [1, 128]])
            nc.vector.tensor_copy(dst, etps[:])

        def attnv_chunk(p, ib, hh, kh, ET_pair, otps):
            hloc = 2 * p + hh
            for kc in range(4 * kh, 4 * kh + 4):
                nc.tensor.matmul(
                    otps[hh * 64:hh * 64 + 64, ib * 512:ib * 512 + 512],
                    v_sb[:, kc * 256 + hloc * 64: kc * 256 + hloc * 64 + 64],
                    ET_pair[hh][:, kc * T + ib * 512: kc * T + ib * 512 + 512],
                    start=(kc == 0), stop=(kc == QB - 1))

        def attnv_extract(p, otps):
            eng = nc.vector if p % 2 == 0 else nc.scalar
            if p % 2 == 0:
                nc.vector.tensor_copy(oT_sb[:, p * T:(p + 1) * T], otps[:])
            else:
                nc.scalar.copy(oT_sb[:, p * T:(p + 1) * T], otps[:])

        def out_proj(t8):
            ops_ = ps_sc.tile([128, 512], F32, tag="sc")
            for p in range(NP):
                nc.tensor.matmul(
                    ops_[:],
                    oT_sb[:, p * T + t8 * 128: p * T + t8 * 128 + 128],
                    wo_sb[:, p * D:(p + 1) * D],
                    start=(p == 0), stop=(p == NP - 1))
            osb = sb2.tile([128, 512], F16, tag="osb", name="osb")
            nc.vector.tensor_copy(osb[:, :256], ops_[:, :256])
            nc.scalar.copy(osb[:, 256:], ops_[:, 256:])
            nc.sync.dma_start(out_d[t8 * 128:(t8 + 1) * 128, :], osb[:])

        # ---- emission schedule ----
        # p proj first (needs no LayerNorm -> fills the stats-chain
        # latency); then alternate psum users with different extraction
        # engines
        for nt in range(4):
            p_proj(0, nt)
        q_proj(0, 0)
        k_proj(0, 0)
        q_proj(0, 1)
        k_proj(0, 1)
        shifted = {0: shifted_tile()}
        for qb in range(QB):
            band_qb(0, qb, shifted[0])
        q_proj(1, 0)
        p_proj(1, 0)
        q_proj(1, 1)
        p_proj(1, 1)
        k_proj(1, 0)
        p_proj(1, 2)
        k_proj(1, 1)
        p_proj(1, 3)
        for t8 in range(QB):
            v_proj(t8)

        ET_tiles = {}
        attnv_work = []   # deferred attn@V chunks for the previous pair
        for h in range(NH):
            # bufs=3: head h+2 must not wait on the deferred attn@V reads
            # of head h's ET (they interleave into head h+2's score loop)
            ET_sb = sb3.tile([128, QB * T], F16, tag="ET", name="ET")
            ET_tiles[h] = ET_sb
            if h + 1 < NH:
                shifted[h + 1] = shifted_tile()
            E_hist = []
            for qb in range(QB):
                if h + 1 < NH:
                    band_qb(h + 1, qb, shifted[h + 1])
                E_hist.append(scores_qb(h, qb, shifted[h]))
                if qb >= 2:
                    transpose_qb(qb - 2, E_hist[qb - 2], ET_sb)
                # slot one deferred attn@V chunk of the previous pair
                if attnv_work:
                    attnv_work.pop(0)()
            transpose_qb(QB - 2, E_hist[QB - 2], ET_sb)
            transpose_qb(QB - 1, E_hist[QB - 1], ET_sb)
            if h % 2 == 1:
                p = h // 2
                ET_pair = (ET_tiles[2 * p], ET_tiles[2 * p + 1])
                chunks = []
                state = {}

                def make_chunk(p_, ib_, hh_, kh_, ET_pair_):
                    def go():
                        if ("ot", p_) not in state:
                            state[("ot", p_)] = ps_o.tile(
                                [128, 2 * 512], F32, tag="o", name="otps")
                        otps = state[("ot", p_)]
                        attnv_chunk(p_, ib_, hh_, kh_, ET_pair_, otps)
                        if ib_ == 1 and hh_ == 1 and kh_ == 1:
                            attnv_extract(p_, otps)
                    return go

                for ib in range(2):
                    for hh in range(2):
                        for kh in range(2):
                            chunks.append(make_chunk(p, ib, hh, kh, ET_pair))
                if h == NH - 1:
                    for cfn in chunks:
                        cfn()
                    for t8 in range(QB):
                        out_proj(t8)
                else:
                    attnv_work.extend(chunks)

    nc.compile()
    return nc


_PROGRAM_CACHE: dict = {}


def _get_program() -> bass.Bass:
    if "nc" not in _PROGRAM_CACHE:
        _PROGRAM_CACHE["nc"] = _build_program()
    return _PROGRAM_CACHE["nc"]


def _prepare_in_maps(x, pos, content_bias, pos_bias, gamma, beta,
                     Wq, bq, Wk, bk, Wv, bv, Wp, Wo, bo):
    x = np.asarray(x, np.float32)
    pos = np.asarray(pos, np.float32)
    gamma = np.asarray(gamma, np.float32)
    beta = np.asarray(beta, np.float32)

    # gamma folding: y = yln*gamma + beta  =>  y@W = yln@(gamma*W) + beta@W
    def fold(W):
        W = np.asarray(W, np.float32)
        return W * gamma[:, None, None], np.einsum("d,dhk->hk", beta, W)

    Wq_f, bq_f = fold(Wq)
    Wk_f, bk_f = fold(Wk)
    Wv_f, bv_f = fold(Wv)
    Wp = np.asarray(Wp, np.float32)
    Wo = np.asarray(Wo, np.float32)

    in_maps = []
    for core in range(8):
        b = core // 2
        g = core % 2
        hs = slice(4 * g, 4 * g + 4)
        qcb = (np.asarray(bq) + np.asarray(content_bias) + bq_f)[hs]
        qpb = (np.asarray(bq) + np.asarray(pos_bias) + bq_f)[hs]
        kb = (np.asarray(bk) + bk_f)[hs]
        vb = (np.asarray(bv) + bv_f)[hs]
        # Wo pair-stacked: [128, NP*D]; pair p rows = Wo[2p] ++ Wo[2p+1]
        Wo_h = np.asarray(Wo)[hs]          # [4, DK, D]
        wo2 = np.concatenate(
            [np.concatenate([Wo_h[2 * p], Wo_h[2 * p + 1]], axis=0)
             for p in range(NP)], axis=1)  # [128, NP*D]
        in_maps.append({
            "xT": np.ascontiguousarray(x[b].T).astype(np.float16),
            "posT": np.ascontiguousarray(pos[b].T).astype(np.float16),
            "wq": np.ascontiguousarray(
                Wq_f[:, hs, :].reshape(D, NH * DK)).astype(np.float16),
            "wk": np.ascontiguousarray(
                Wk_f[:, hs, :].reshape(D, NH * DK)).astype(np.float16),
            "wv": np.ascontiguousarray(
                Wv_f[:, hs, :].reshape(D, NH * DK)).astype(np.float16),
            "wp": np.ascontiguousarray(
                Wp[:, hs, :].reshape(D, NH * DK)).astype(np.float16),
            "wo": np.ascontiguousarray(wo2).astype(np.float16),
            "qc_bias": np.ascontiguousarray(qcb.reshape(2, 128).T),
            "qp_bias": np.ascontiguousarray(qpb.reshape(2, 128).T),
            "k_bias": np.ascontiguousarray(kb.reshape(2, 128).T),
            "v_bias": np.ascontiguousarray(vb.reshape(NH * DK)),
        })

    return in_maps


def _combine(x, bo, results):
    parts = [r["out_partial"] for r in results]
    out = np.asarray(x, np.float32) + np.asarray(bo, np.float32)[None, None, :]
    for b in range(B):
        out[b] += parts[2 * b].astype(np.float32)
        out[b] += parts[2 * b + 1].astype(np.float32)
    return out.astype(np.float32)


def kernel(x, pos, content_bias, pos_bias, gamma, beta,
           Wq, bq, Wk, bk, Wv, bv, Wp, Wo, bo) -> np.ndarray:
    in_maps = _prepare_in_maps(x, pos, content_bias, pos_bias, gamma, beta,
                               Wq, bq, Wk, bk, Wv, bv, Wp, Wo, bo)
    nc = _get_program()
    res = run_bass_kernel_spmd(nc, in_maps, core_ids=list(range(8)))
    return _combine(x, bo, res.results)


# revision 3
# speedup vs baseline: 139.6177x; 1.0041x over previous
"""Trainium2 Bass kernel v2 for Transformer-XL style MHSA (nn_MHSAModule).

Problem (hardcoded):
  B=4, T=1024, D=512, H=8, DK=64, L=2*T-1=2047, eps=1e-3
  out = x + (MHSA(LayerNorm(x), pos) @ Wo + bo)

Sharding: 8 cores = 4 batches x 2 head-groups (4 heads each). Each core
returns a partial output [T, D] f16 (its heads' contribution); the host
sums the two partials per batch and adds the residual x + bo.

v2 design notes (vs v1):
  - f16 activations/weights on the matmul path; f8e4m3 only for matmul
    B-operands no vector engine reads back: kT, pT, and the rel-shift
    band bounce (halves its DMA volume).
  - exp runs on Act straight from PSUM with accum_out giving the softmax
    denominator for free; E is normalized by one per-partition f16
    tensor_scalar instead of v1's psum-copy + scale chain.
  - PSUM extraction is the scarce resource (~4x an SBUF read): band
    extraction is split across DVE/Act/Pool, scores are extracted by the
    exp itself, ET extracted as f16 (2x cheaper than f32).
  - LayerNorm stats via f16 ones-matmuls; a/b rows reach all partitions
    via gpsimd.partition_broadcast instead of a DRAM round-trip.
  - attention@V stacks head pairs on 128 PSUM partitions; Wo is
    pair-stacked so the output projection contracts 128 rows per step.
  - engines execute in-order, so emission order IS the schedule: x/wq
    load first, the band of head h+1 and the attn@V of the previous pair
    are interleaved into head h's per-q-block score loop.
"""
import numpy as np
from contextlib import ExitStack

import concourse.bass as bass
import concourse.bacc as bacc
import concourse.tile as tile
from concourse import mybir
from concourse import masks
from concourse.bass_utils import run_bass_kernel_spmd

F32 = mybir.dt.float32
F16 = mybir.dt.float16
F8 = mybir.dt.float8e4
AF = mybir.ActivationFunctionType
OP = mybir.AluOpType

B, T, D, H, DK = 4, 1024, 512, 8, 64
L = 2 * T - 1
EPS = 1e-3
NH = 4          # heads per core
NP = 2          # head pairs per core
CH = D // 128   # 4 contraction chunks
QB = T // 128   # 8 q blocks
BAND = 1152    # positional band width per q block
PL = L + 2      # padded pT free size (2 zero pad cols)


def _build_program() -> bass.Bass:
    nc = bacc.Bacc("TRN2", target_bir_lowering=False, debug=False)

    # ---- DRAM I/O ----
    xT = nc.dram_tensor("xT", [D, T], F16, kind="ExternalInput")
    posT = nc.dram_tensor("posT", [D, L], F16, kind="ExternalInput")
    wq = nc.dram_tensor("wq", [D, NH * DK], F16, kind="ExternalInput")
    wk = nc.dram_tensor("wk", [D, NH * DK], F16, kind="ExternalInput")
    wv = nc.dram_tensor("wv", [D, NH * DK], F16, kind="ExternalInput")
    wp = nc.dram_tensor("wp", [D, NH * DK], F16, kind="ExternalInput")
    wo = nc.dram_tensor("wo", [128, NP * D], F16, kind="ExternalInput")
    qc_bias = nc.dram_tensor("qc_bias", [128, NP], F32, kind="ExternalInput")
    qp_bias = nc.dram_tensor("qp_bias", [128, NP], F32, kind="ExternalInput")
    k_bias = nc.dram_tensor("k_bias", [128, NP], F32, kind="ExternalInput")
    v_bias = nc.dram_tensor("v_bias", [NH * DK], F32, kind="ExternalInput")
    out_d = nc.dram_tensor("out_partial", [T, D], F16, kind="ExternalOutput")

    # internal scratch: rel-shift bounce, f8, double buffered
    bounce = nc.dram_tensor("bounce", [2, QB, 128, BAND], F8)

    with tile.TileContext(nc) as tc, ExitStack() as ctx:
        sb = ctx.enter_context(tc.tile_pool(name="sb", bufs=1))
        sb2 = ctx.enter_context(tc.tile_pool(name="sb2", bufs=2))
        sb3 = ctx.enter_context(tc.tile_pool(name="sb3", bufs=3))
        sbE = ctx.enter_context(tc.tile_pool(name="sbE", bufs=4))
        # PSUM: sc 2x[128,512]f32 (2 banks) + band 1x[128,1152]f32
        # (3 banks) + et 2x[128,1024]f16 (2 banks) + o 1x[128,512]f32
        # (1 bank) = 8 banks.
        ps_sc = ctx.enter_context(tc.tile_pool(name="ps_sc", bufs=2,
                                               space="PSUM"))
        # band psum: three independently-released tiles so the next band
        # matmul only waits on the one engine that extracts each slice
        ps_b = ctx.enter_context(tc.tile_pool(name="ps_b", bufs=1,
                                              space="PSUM"))
        ps_et = ctx.enter_context(tc.tile_pool(name="ps_et", bufs=1,
                                               space="PSUM"))
        ps_o = ctx.enter_context(tc.tile_pool(name="ps_o", bufs=1,
                                              space="PSUM"))

        # ---- persistent SBUF ----
        xT_sb = sb.tile([128, CH * T], F16)
        yT_sb = sb.tile([128, CH * T], F16)
        posT_sb = sb.tile([128, CH * L + 2], F16)
        pT_sb = sb.tile([128, NP * PL], F8)
        qcT_sb = sb.tile([128, NP * T], F8)
        qpT_sb = sb.tile([128, NP * T], F8)
        kT_sb = sb.tile([128, NP * T], F8)
        v_sb = sb.tile([128, QB * NH * DK], F16)
        oT_sb = sb.tile([128, NP * T], F16)
        wq_sb = sb.tile([128, CH * 256], F16)
        wk_sb = sb.tile([128, CH * 256], F16)
        wv_sb = sb.tile([128, CH * 256], F16)
        wp_sb = sb.tile([128, CH * 256], F16)
        wo_sb = sb.tile([128, NP * D], F16)
        qcb_sb = sb.tile([128, NP], F32)
        qpb_sb = sb.tile([128, NP], F32)
        kb_sb = sb.tile([128, NP], F32)
        vb_sb = sb.tile([128, 256], F32)
        arep = sb.tile([128, T], F32)
        brep = sb.tile([128, T], F32)
        ident16 = sb.tile([128, 128], F16)
        ident8 = sb.tile([128, 128], F8)
        ones_col = sb.tile([128, 1], F16)
        eps_col = sb.tile([1, 1], F32)

        nc.vector.memset(ones_col[:], 1.0)
        nc.vector.memset(eps_col[:], EPS)

        # ---- loads: x first (stats), then pos+wp (p proj), then q/k ----
        for c in range(CH):
            nc.sync.dma_start(xT_sb[:, c * T:(c + 1) * T],
                              xT[c * 128:(c + 1) * 128, :])
        for c in range(CH):
            nc.sync.dma_start(posT_sb[:, c * L:(c + 1) * L],
                              posT[c * 128:(c + 1) * 128, :])
            nc.sync.dma_start(wp_sb[:, c * 256:(c + 1) * 256],
                              wp[c * 128:(c + 1) * 128, :])
        for c in range(CH):
            nc.sync.dma_start(wq_sb[:, c * 256:(c + 1) * 256],
                              wq[c * 128:(c + 1) * 128, :])
            nc.sync.dma_start(wk_sb[:, c * 256:(c + 1) * 256],
                              wk[c * 128:(c + 1) * 128, :])
        nc.sync.dma_start(qcb_sb[:], qc_bias[:])
        nc.sync.dma_start(qpb_sb[:], qp_bias[:])
        nc.sync.dma_start(kb_sb[:], k_bias[:])
        for c in range(CH):
            nc.sync.dma_start(wv_sb[:, c * 256:(c + 1) * 256],
                              wv[c * 128:(c + 1) * 128, :])
        nc.sync.dma_start(wo_sb[:], wo[:])
        nc.sync.dma_start(
            vb_sb[:], bass.AP(v_bias[:].tensor, 0, [[0, 128], [1, 256]]))

        # ---- LayerNorm stats; short [1,512]-row chain (row ops are
        # lane-serial, so every op counts): a = rsqrt(var+eps),
        # b = -mu*a, computed as
        #   t1 = s1*s1; v' = D*s2 - t1; a = Rsqrt(v'/D^2 + eps);
        #   b = (s1 * -1/D) * a
        a_row = sb.tile([1, T], F32)
        b_row = sb.tile([1, T], F32)
        for tt in range(2):
            s1 = ps_b.tile([1, 512], F32, tag="band0")
            for c in range(CH):
                xt = xT_sb[:, c * T + tt * 512: c * T + tt * 512 + 512]
                nc.tensor.matmul(s1[:], ones_col[:], xt,
                                 start=(c == 0), stop=(c == CH - 1))
            s2 = ps_b.tile([1, 512], F32, tag="band1")
            for c in range(CH):
                xsq = sb3.tile([128, 512], F16, tag="xsq", name="xsq")
                xt = xT_sb[:, c * T + tt * 512: c * T + tt * 512 + 512]
                nc.vector.tensor_tensor(xsq[:], xt, xt, op=OP.mult)
                nc.tensor.matmul(s2[:], ones_col[:], xsq[:],
                                 start=(c == 0), stop=(c == CH - 1))
            t1 = sb2.tile([1, 512], F32, tag="t1", name="t1")
            nc.scalar.activation(t1[:], s1[:], AF.Square)
            vv = sb2.tile([1, 512], F32, tag="vv", name="vv")
            nc.vector.scalar_tensor_tensor(vv[:], s2[:], float(D), t1[:],
                                           op0=OP.mult, op1=OP.subtract)
            stdh = sb2.tile([1, 512], F32, tag="stdh", name="stdh")
            nc.scalar.activation(stdh[:], vv[:], AF.Sqrt,
                                 scale=1.0 / (D * D), bias=eps_col[:])
            a_half = a_row[:, tt * 512:(tt + 1) * 512]
            nc.vector.reciprocal(a_half, stdh[:])
            nc.vector.scalar_tensor_tensor(
                b_row[:, tt * 512:(tt + 1) * 512], s1[:], -1.0 / D, a_half,
                op0=OP.mult, op1=OP.mult)
        nc.gpsimd.partition_broadcast(arep[:], a_row[:])
        nc.gpsimd.partition_broadcast(brep[:], b_row[:])
        masks.make_identity(nc, ident16[:])
        masks.make_identity(nc, ident8[:])

        # ---- LayerNorm apply: yT = xT * a + b (f16), 512-col pieces so
        # the first projections start after the nt=0 halves ----
        for nt in range(2):
            for c in range(CH):
                eng = nc.vector if c % 2 == 0 else nc.gpsimd
                xs = xT_sb[:, c * T + nt * 512: c * T + nt * 512 + 512]
                ys = yT_sb[:, c * T + nt * 512: c * T + nt * 512 + 512]
                ar = arep[:, nt * 512:(nt + 1) * 512]
                br = brep[:, nt * 512:(nt + 1) * 512]
                eng.tensor_tensor(ys, xs, ar, op=OP.mult)
                eng.tensor_tensor(ys, ys, br, op=OP.add)

        # pad pT columns
        z8 = sb.tile([128, 4], F8)
        nc.vector.memset(z8[:], 0.0)
        zrow = sb.tile([128, 2], F16)
        nc.vector.memset(zrow[:], 0.0)
        nc.vector.tensor_copy(posT_sb[:, CH * L:], zrow[:])

        # ---- projection emitters ----
        # extraction engines are spread (qc->Act, qp->DVE, k/p->Pool,
        # v->DVE) so consecutive users of the psum ping-pong release in
        # parallel queues.
        def q_proj(p, nt, pool=None, ptag="sc"):
            prj = (pool or ps_sc).tile([128, 512], F32, tag=ptag,
                                       name="prj")
            for ci, c in enumerate((0, 2, 1, 3)):
                nc.tensor.matmul(
                    prj[:],
                    wq_sb[:, c * 256 + p * 128: c * 256 + p * 128 + 128],
                    yT_sb[:, c * T + nt * 512: c * T + nt * 512 + 512],
                    start=(c == 0), stop=(c == CH - 1))
            o = p * T + nt * 512
            nc.scalar.activation(qcT_sb[:, o:o + 512], prj[:],
                                 AF.Identity, bias=qcb_sb[:, p:p + 1])
            nc.vector.tensor_scalar_add(qpT_sb[:, o:o + 512], prj[:],
                                        qpb_sb[:, p:p + 1])

        def k_proj(p, nt, pool=None, ptag="sc"):
            prjk = (pool or ps_sc).tile([128, 512], F32, tag=ptag,
                                        name="prjk")
            for ci, c in enumerate((0, 2, 1, 3)):
                nc.tensor.matmul(
                    prjk[:],
                    wk_sb[:, c * 256 + p * 128: c * 256 + p * 128 + 128],
                    yT_sb[:, c * T + nt * 512: c * T + nt * 512 + 512],
                    start=(ci == 0), stop=(ci == CH - 1))
            nc.vector.tensor_scalar_add(kT_sb[:, p * T + nt * 512:
                                              p * T + nt * 512 + 512],
                                        prjk[:], kb_sb[:, p:p + 1])

        def p_proj(p, nt, pool=None, ptag="sc"):
            pps = (pool or ps_sc).tile([128, 512], F32, tag=ptag,
                                       name="pps")
            for ci, c in enumerate((0, 2, 1, 3)):
                nc.tensor.matmul(
                    pps[:],
                    wp_sb[:, c * 256 + p * 128: c * 256 + p * 128 + 128],
                    posT_sb[:, c * L + nt * 512: c * L + nt * 512 + 512],
                    start=(ci == 0), stop=(ci == CH - 1))
            nc.scalar.copy(
                pT_sb[:, p * PL + nt * 512: p * PL + nt * 512 + 512],
                pps[:])
            if nt == 3:
                nc.gpsimd.tensor_copy(pT_sb[:, p * PL + L: (p + 1) * PL],
                                      z8[:, :PL - L])

        def v_proj(t8, pool=None, ptag="sc"):
            vps = (pool or ps_sc).tile([128, 256], F32, tag=ptag,
                                       name="vps")
            for c in range(CH):
                nc.tensor.matmul(
                    vps[:],
                    yT_sb[:, c * T + t8 * 128: c * T + t8 * 128 + 128],
                    wv_sb[:, c * 256:(c + 1) * 256],
                    start=(c == 0), stop=(c == CH - 1))
            nc.vector.tensor_tensor(
                v_sb[:, t8 * 256:(t8 + 1) * 256], vps[:], vb_sb[:],
                op=OP.add)

        # ---- attention emitters ----
        def hsl(tile_, h, lo, w, stride=T):
            p, off = h // 2, (h % 2) * 64
            return tile_[off:off + 64, p * stride + lo: p * stride + lo + w]

        def band_qb(h, qb, shifted):
            """Band scores for (h, qb), bounce out, and the skewed read of
            this qb's block straight back into `shifted`. Three psum tiles,
            one extraction engine each, so the next band matmul waits on
            exactly one engine per tile."""
            ping = h % 2
            s0 = 897 - qb * 128
            qp_blk = hsl(qpT_sb, h, qb * 128, 128)
            bps0 = ps_b.tile([128, 768], F32, tag="band0")
            nc.tensor.matmul(bps0[:, :512], qp_blk,
                             hsl(pT_sb, h, s0, 512, stride=PL),
                             start=True, stop=True)
            nc.tensor.matmul(bps0[:, 512:], qp_blk,
                             hsl(pT_sb, h, s0 + 512, 256, stride=PL),
                             start=True, stop=True)
            bps1 = ps_b.tile([128, 384], F32, tag="band1")
            nc.tensor.matmul(bps1[:], qp_blk,
                             hsl(pT_sb, h, s0 + 768, 384, stride=PL),
                             start=True, stop=True)
            b8 = sb2.tile([128, BAND], F8, tag="band8", name="b8")
            if h == 0:
                # Act is exp-free before head 0's scores; give it the
                # wide slice so the DVE prefix backlog stays short
                nc.scalar.copy(b8[:, :768], bps0[:])
                nc.vector.tensor_copy(b8[:, 768:], bps1[:])
            else:
                nc.vector.tensor_copy(b8[:, :768], bps0[:])
                nc.scalar.copy(b8[:, 768:], bps1[:])
            nc.sync.dma_start(bounce[ping, qb], b8[:])
            src = bass.AP(bounce[:].tensor,
                          (ping * QB + qb) * (128 * BAND) + 127,
                          [[BAND - 1, 128], [1, T]])
            nc.sync.dma_start(shifted[:, qb * T:(qb + 1) * T], src)

        def shifted_tile():
            shifted = sb2.tile([128, QB * T], F8, tag="shifted",
                               name="shifted")
            return shifted

        def scores_qb(h, qb, shifted):
            """Scores+exp+normalize for (h, qb); returns the E tile.
            Transposes are emitted one qb later (transpose_qb) so the PE
            queue never waits on the Act/DVE chain of the same qb."""
            E_sb = sbE.tile([128, T], F16, tag="E", name="E")
            den = sb2.tile([128, 2], F32, tag="den", name="den")
            for half in range(2):
                sps = ps_sc.tile([128, 512], F32, tag="sc")
                nc.tensor.matmul(
                    sps[:],
                    hsl(qcT_sb, h, qb * 128, 128),
                    hsl(kT_sb, h, half * 512, 512),
                    start=True, stop=False)
                if qb == 0 and half == 1:
                    # scores[0, 1023] += (q+pos_bias)[1] . p[0]
                    # (the reference rel_shift reshape wraps this element)
                    nc.tensor.matmul(
                        sps[0:1, 511:512],
                        hsl(qpT_sb, h, 1, 1),
                        hsl(pT_sb, h, 0, 1, stride=PL),
                        start=False, stop=False)
                nc.tensor.matmul(
                    sps[:],
                    ident8[:],
                    shifted[:, qb * T + half * 512:
                            qb * T + half * 512 + 512],
                    start=False, stop=True)
                nc.scalar.activation(
                    E_sb[:, half * 512:(half + 1) * 512], sps[:], AF.Exp,
                    scale=0.125, accum_out=den[:, half:half + 1])
            rec = sb2.tile([128, 1], F32, tag="rec", name="rec")
            nc.gpsimd.tensor_tensor(rec[:], den[:, 0:1], den[:, 1:2],
                                    op=OP.add)
            nc.vector.reciprocal(rec[:], rec[:])
            nc.vector.tensor_scalar_mul(E_sb[:], E_sb[:], rec[:])
            return E_sb

        def transpose_qb(qb, E_sb, ET_sb):
            # transpose E (f16): 8 PE transposes -> one f16 psum bank
            etps = ps_et.tile([128, T], F16, tag="et")
            for kc in range(QB):
                nc.tensor.transpose(
                    etps[:, kc * 128: kc * 128 + 128],
                    E_sb[:, kc * 128: kc * 128 + 128],
                    ident16[:])
            # scatter-extract: ET_sb[:, kc*T + qb*128 ...] = etps block kc
            dst = bass.AP(ET_sb[:].tensor, qb * 128,
                          [[QB * T, 128], [T, QB], [1, 128]])
            nc.vector.tensor_copy(dst, etps[:])

        def attnv_chunk(p, ib, hh, kh, ET_pair, otps):
            hloc = 2 * p + hh
            for kc in range(4 * kh, 4 * kh + 4):
                nc.tensor.matmul(
                    otps[hh * 64:hh * 64 + 64, ib * 512:ib * 512 + 512],
                    v_sb[:, kc * 256 + hloc * 64: kc * 256 + hloc * 64 + 64],
                    ET_pair[hh][:, kc * T + ib * 512: kc * T + ib * 512 + 512],
                    start=(kc == 0), stop=(kc == QB - 1))

        def attnv_extract(p, otps):
            eng = nc.vector if p % 2 == 0 else nc.scalar
            if p % 2 == 0:
                nc.vector.tensor_copy(oT_sb[:, p * T:(p + 1) * T], otps[:])
            else:
                nc.scalar.copy(oT_sb[:, p * T:(p + 1) * T], otps[:])

        def out_proj(t8):
            ops_ = ps_sc.tile([128, 512], F32, tag="sc")
            for p in range(NP):
                nc.tensor.matmul(
                    ops_[:],
                    oT_sb[:, p * T + t8 * 128: p * T + t8 * 128 + 128],
                    wo_sb[:, p * D:(p + 1) * D],
                    start=(p == 0), stop=(p == NP - 1))
            osb = sb2.tile([128, 512], F16, tag="osb", name="osb")
            nc.vector.tensor_copy(osb[:, :256], ops_[:, :256])
            nc.scalar.copy(osb[:, 256:], ops_[:, 256:])
            nc.sync.dma_start(out_d[t8 * 128:(t8 + 1) * 128, :], osb[:])

        # ---- emission schedule ----
        # p proj first (needs no LayerNorm -> fills the stats-chain
        # latency). Only pair-0 projections + head-0 bands run before the
        # attention loop; pair-1 projections and v-proj are deferred as
        # per-q-block filler inside the head-0/1 loops so the in-order PE
        # queue reaches head-0 scores ~12us earlier.
        for nt in range(4):
            p_proj(0, nt)
        q_proj(0, 0)
        k_proj(0, 0)
        q_proj(0, 1)
        k_proj(0, 1)
        shifted = {0: shifted_tile()}
        for qb in range(QB):
            band_qb(0, qb, shifted[0])

        filler = {
            0: [lambda: q_proj(1, 0, ps_o, "o"),
                lambda: q_proj(1, 1, ps_o, "o"),
                lambda: k_proj(1, 0, ps_o, "o"),
                lambda: k_proj(1, 1, ps_o, "o"),
                lambda: p_proj(1, 0, ps_o, "o"),
                lambda: p_proj(1, 1, ps_o, "o"),
                lambda: p_proj(1, 2, ps_o, "o"),
                lambda: p_proj(1, 3, ps_o, "o")],
            1: [(lambda t8=t8: v_proj(t8, ps_o, "o")) for t8 in range(QB)],
        }

        ET_tiles = {}
        attnv_work = []   # deferred attn@V chunks for the previous pair
        for h in range(NH):
            # bufs=3: head h+2 must not wait on the deferred attn@V reads
            # of head h's ET (they interleave into head h+2's score loop)
            ET_sb = sb3.tile([128, QB * T], F16, tag="ET", name="ET")
            ET_tiles[h] = ET_sb
            if h + 1 < NH:
                shifted[h + 1] = shifted_tile()
            E_hist = []
            for qb in range(QB):
                if h + 1 < NH:
                    band_qb(h + 1, qb, shifted[h + 1])
                E_hist.append(scores_qb(h, qb, shifted[h]))
                if qb >= 2:
                    transpose_qb(qb - 2, E_hist[qb - 2], ET_sb)
                for fn_ in filler.get(h, [])[qb:qb + 1]:
                    fn_()
                # slot one deferred attn@V chunk of the previous pair
                if attnv_work:
                    attnv_work.pop(0)()
            transpose_qb(QB - 2, E_hist[QB - 2], ET_sb)
            transpose_qb(QB - 1, E_hist[QB - 1], ET_sb)
            if h % 2 == 1:
                p = h // 2
                ET_pair = (ET_tiles[2 * p], ET_tiles[2 * p + 1])
                chunks = []
                state = {}

                def make_chunk(p_, ib_, hh_, kh_, ET_pair_):
                    def go():
                        if ("ot", p_) not in state:
                            state[("ot", p_)] = ps_o.tile(
                                [128, 2 * 512], F32, tag="o", name="otps")
                        otps = state[("ot", p_)]
                        attnv_chunk(p_, ib_, hh_, kh_, ET_pair_, otps)
                        if ib_ == 1 and hh_ == 1 and kh_ == 1:
                            attnv_extract(p_, otps)
                    return go

                for ib in range(2):
                    for hh in range(2):
                        for kh in range(2):
                            chunks.append(make_chunk(p, ib, hh, kh, ET_pair))
                if h == NH - 1:
                    for cfn in chunks:
                        cfn()
                    for t8 in range(QB):
                        out_proj(t8)
                else:
                    attnv_work.extend(chunks)

    nc.compile()
    return nc


_PROGRAM_CACHE: dict = {}


def _get_program() -> bass.Bass:
    if "nc" not in _PROGRAM_CACHE:
        _PROGRAM_CACHE["nc"] = _build_program()
    return _PROGRAM_CACHE["nc"]


def _prepare_in_maps(x, pos, content_bias, pos_bias, gamma, beta,
                     Wq, bq, Wk, bk, Wv, bv, Wp, Wo, bo):
    x = np.asarray(x, np.float32)
    pos = np.asarray(pos, np.float32)
    gamma = np.asarray(gamma, np.float32)
    beta = np.asarray(beta, np.float32)

    # gamma folding: y = yln*gamma + beta  =>  y@W = yln@(gamma*W) + beta@W
    def fold(W):
        W = np.asarray(W, np.float32)
        return W * gamma[:, None, None], np.einsum("d,dhk->hk", beta, W)

    Wq_f, bq_f = fold(Wq)
    Wk_f, bk_f = fold(Wk)
    Wv_f, bv_f = fold(Wv)
    Wp = np.asarray(Wp, np.float32)
    Wo = np.asarray(Wo, np.float32)

    in_maps = []
    for core in range(8):
        b = core // 2
        g = core % 2
        hs = slice(4 * g, 4 * g + 4)
        qcb = (np.asarray(bq) + np.asarray(content_bias) + bq_f)[hs]
        qpb = (np.asarray(bq) + np.asarray(pos_bias) + bq_f)[hs]
        kb = (np.asarray(bk) + bk_f)[hs]
        vb = (np.asarray(bv) + bv_f)[hs]
        # Wo pair-stacked: [128, NP*D]; pair p rows = Wo[2p] ++ Wo[2p+1]
        Wo_h = np.asarray(Wo)[hs]          # [4, DK, D]
        wo2 = np.concatenate(
            [np.concatenate([Wo_h[2 * p], Wo_h[2 * p + 1]], axis=0)
             for p in range(NP)], axis=1)  # [128, NP*D]
        in_maps.append({
            "xT": np.ascontiguousarray(x[b].T).astype(np.float16),
            "posT": np.ascontiguousarray(pos[b].T).astype(np.float16),
            "wq": np.ascontiguousarray(
                Wq_f[:, hs, :].reshape(D, NH * DK)).astype(np.float16),
            "wk": np.ascontiguousarray(
                Wk_f[:, hs, :].reshape(D, NH * DK)).astype(np.float16),
            "wv": np.ascontiguousarray(
                Wv_f[:, hs, :].reshape(D, NH * DK)).astype(np.float16),
            "wp": np.ascontiguousarray(
                Wp[:, hs, :].reshape(D, NH * DK)).astype(np.float16),
            "wo": np.ascontiguousarray(wo2).astype(np.float16),
            "qc_bias": np.ascontiguousarray(qcb.reshape(2, 128).T),
            "qp_bias": np.ascontiguousarray(qpb.reshape(2, 128).T),
            "k_bias": np.ascontiguousarray(kb.reshape(2, 128).T),
            "v_bias": np.ascontiguousarray(vb.reshape(NH * DK)),
        })

    return in_maps


def _combine(x, bo, results):
    parts = [r["out_partial"] for r in results]
    out = np.asarray(x, np.float32) + np.asarray(bo, np.float32)[None, None, :]
    for b in range(B):
        out[b] += parts[2 * b].astype(np.float32)
        out[b] += parts[2 * b + 1].astype(np.float32)
    return out.astype(np.float32)


def kernel(x, pos, content_bias, pos_bias, gamma, beta,
           Wq, bq, Wk, bk, Wv, bv, Wp, Wo, bo) -> np.ndarray:
    in_maps = _prepare_in_maps(x, pos, content_bias, pos_bias, gamma, beta,
                               Wq, bq, Wk, bk, Wv, bv, Wp, Wo, bo)
    nc = _get_program()
    res = run_bass_kernel_spmd(nc, in_maps, core_ids=list(range(8)))
    return _combine(x, bo, res.results)


# revision 4
# speedup vs baseline: 140.2099x; 1.0042x over previous
"""Trainium2 Bass kernel v2 for Transformer-XL style MHSA (nn_MHSAModule).

Problem (hardcoded):
  B=4, T=1024, D=512, H=8, DK=64, L=2*T-1=2047, eps=1e-3
  out = x + (MHSA(LayerNorm(x), pos) @ Wo + bo)

Sharding: 8 cores = 4 batches x 2 head-groups (4 heads each). Each core
returns a partial output [T, D] f16 (its heads' contribution); the host
sums the two partials per batch and adds the residual x + bo.

v2 design notes (vs v1):
  - f16 activations/weights on the matmul path; f8e4m3 only for matmul
    B-operands no vector engine reads back: kT, pT, and the rel-shift
    band bounce (halves its DMA volume).
  - exp runs on Act straight from PSUM with accum_out giving the softmax
    denominator for free; E is normalized by one per-partition f16
    tensor_scalar instead of v1's psum-copy + scale chain.
  - PSUM extraction is the scarce resource (~4x an SBUF read): band
    extraction is split across DVE/Act/Pool, scores are extracted by the
    exp itself, ET extracted as f16 (2x cheaper than f32).
  - LayerNorm stats via f16 ones-matmuls; a/b rows reach all partitions
    via gpsimd.partition_broadcast instead of a DRAM round-trip.
  - attention@V stacks head pairs on 128 PSUM partitions; Wo is
    pair-stacked so the output projection contracts 128 rows per step.
  - engines execute in-order, so emission order IS the schedule: x/wq
    load first, the band of head h+1 and the attn@V of the previous pair
    are interleaved into head h's per-q-block score loop.
"""
import numpy as np
from contextlib import ExitStack

import concourse.bass as bass
import concourse.bacc as bacc
import concourse.tile as tile
from concourse import mybir
from concourse import masks
from concourse.bass_utils import run_bass_kernel_spmd

F32 = mybir.dt.float32
F16 = mybir.dt.float16
F8 = mybir.dt.float8e4
AF = mybir.ActivationFunctionType
OP = mybir.AluOpType

B, T, D, H, DK = 4, 1024, 512, 8, 64
L = 2 * T - 1
EPS = 1e-3
NH = 4          # heads per core
NP = 2          # head pairs per core
CH = D // 128   # 4 contraction chunks
QB = T // 128   # 8 q blocks
BAND = 1152    # positional band width per q block
PL = L + 2      # padded pT free size (2 zero pad cols)


def _build_program() -> bass.Bass:
    nc = bacc.Bacc("TRN2", target_bir_lowering=False, debug=False)

    # ---- DRAM I/O ----
    xT = nc.dram_tensor("xT", [D, T], F16, kind="ExternalInput")
    posT = nc.dram_tensor("posT", [D, L], F16, kind="ExternalInput")
    wq = nc.dram_tensor("wq", [D, NH * DK], F16, kind="ExternalInput")
    wk = nc.dram_tensor("wk", [D, NH * DK], F16, kind="ExternalInput")
    wv = nc.dram_tensor("wv", [D, NH * DK], F16, kind="ExternalInput")
    wp = nc.dram_tensor("wp", [D, NH * DK], F16, kind="ExternalInput")
    wo = nc.dram_tensor("wo", [128, NP * D], F16, kind="ExternalInput")
    qc_bias = nc.dram_tensor("qc_bias", [128, NP], F32, kind="ExternalInput")
    qp_bias = nc.dram_tensor("qp_bias", [128, NP], F32, kind="ExternalInput")
    k_bias = nc.dram_tensor("k_bias", [128, NP], F32, kind="ExternalInput")
    v_bias = nc.dram_tensor("v_bias", [NH * DK], F32, kind="ExternalInput")
    out_d = nc.dram_tensor("out_partial", [T, D], F16, kind="ExternalOutput")

    # internal scratch: rel-shift bounce, f8, double buffered
    bounce = nc.dram_tensor("bounce", [2, QB, 128, BAND], F8)

    with tile.TileContext(nc) as tc, ExitStack() as ctx:
        sb = ctx.enter_context(tc.tile_pool(name="sb", bufs=1))
        sb2 = ctx.enter_context(tc.tile_pool(name="sb2", bufs=2))
        sb3 = ctx.enter_context(tc.tile_pool(name="sb3", bufs=3))
        sbE = ctx.enter_context(tc.tile_pool(name="sbE", bufs=4))
        # PSUM: sc 2x[128,512]f32 (2 banks) + band 1x[128,1152]f32
        # (3 banks) + et 2x[128,1024]f16 (2 banks) + o 1x[128,512]f32
        # (1 bank) = 8 banks.
        ps_sc = ctx.enter_context(tc.tile_pool(name="ps_sc", bufs=2,
                                               space="PSUM"))
        # band psum: three independently-released tiles so the next band
        # matmul only waits on the one engine that extracts each slice
        ps_b = ctx.enter_context(tc.tile_pool(name="ps_b", bufs=1,
                                              space="PSUM"))
        ps_et = ctx.enter_context(tc.tile_pool(name="ps_et", bufs=1,
                                               space="PSUM"))
        ps_o = ctx.enter_context(tc.tile_pool(name="ps_o", bufs=1,
                                              space="PSUM"))

        # ---- persistent SBUF ----
        xT_sb = sb.tile([128, CH * T], F16)
        yT_sb = sb.tile([128, CH * T], F16)
        posT_sb = sb.tile([128, CH * L + 2], F16)
        pT_sb = sb.tile([128, NP * PL], F8)
        qcT_sb = sb.tile([128, NP * T], F8)
        qpT_sb = sb.tile([128, NP * T], F8)
        kT_sb = sb.tile([128, NP * T], F8)
        v_sb = sb.tile([128, QB * NH * DK], F16)
        oT_sb = sb.tile([128, NP * T], F16)
        wq_sb = sb.tile([128, CH * 256], F16)
        wk_sb = sb.tile([128, CH * 256], F16)
        wv_sb = sb.tile([128, CH * 256], F16)
        wp_sb = sb.tile([128, CH * 256], F16)
        wo_sb = sb.tile([128, NP * D], F16)
        qcb_sb = sb.tile([128, NP], F32)
        qpb_sb = sb.tile([128, NP], F32)
        kb_sb = sb.tile([128, NP], F32)
        vb_sb = sb.tile([128, 256], F32)
        arep = sb.tile([128, T], F32)
        brep = sb.tile([128, T], F32)
        ident16 = sb.tile([128, 128], F16)
        ident8 = sb.tile([128, 128], F8)
        ones_col = sb.tile([128, 1], F16)
        eps_col = sb.tile([1, 1], F32)

        nc.vector.memset(ones_col[:], 1.0)
        nc.vector.memset(eps_col[:], EPS)

        # ---- loads: x first (stats), then pos+wp (p proj), then q/k ----
        for c in range(CH):
            nc.sync.dma_start(xT_sb[:, c * T:(c + 1) * T],
                              xT[c * 128:(c + 1) * 128, :])
        for c in range(CH):
            nc.sync.dma_start(posT_sb[:, c * L:(c + 1) * L],
                              posT[c * 128:(c + 1) * 128, :])
            nc.sync.dma_start(wp_sb[:, c * 256:(c + 1) * 256],
                              wp[c * 128:(c + 1) * 128, :])
        for c in range(CH):
            nc.sync.dma_start(wq_sb[:, c * 256:(c + 1) * 256],
                              wq[c * 128:(c + 1) * 128, :])
            nc.sync.dma_start(wk_sb[:, c * 256:(c + 1) * 256],
                              wk[c * 128:(c + 1) * 128, :])
        nc.sync.dma_start(qcb_sb[:], qc_bias[:])
        nc.sync.dma_start(qpb_sb[:], qp_bias[:])
        nc.sync.dma_start(kb_sb[:], k_bias[:])
        for c in range(CH):
            nc.sync.dma_start(wv_sb[:, c * 256:(c + 1) * 256],
                              wv[c * 128:(c + 1) * 128, :])
        nc.sync.dma_start(wo_sb[:], wo[:])
        nc.sync.dma_start(
            vb_sb[:], bass.AP(v_bias[:].tensor, 0, [[0, 128], [1, 256]]))

        # ---- LayerNorm stats; short [1,512]-row chain (row ops are
        # lane-serial, so every op counts): a = rsqrt(var+eps),
        # b = -mu*a, computed as
        #   t1 = s1*s1; v' = D*s2 - t1; a = Rsqrt(v'/D^2 + eps);
        #   b = (s1 * -1/D) * a
        a_row = sb.tile([1, T], F32)
        b_row = sb.tile([1, T], F32)
        for tt in range(2):
            s1 = ps_b.tile([1, 512], F32, tag="band0")
            for c in range(CH):
                xt = xT_sb[:, c * T + tt * 512: c * T + tt * 512 + 512]
                nc.tensor.matmul(s1[:], ones_col[:], xt,
                                 start=(c == 0), stop=(c == CH - 1))
            s2 = ps_b.tile([1, 512], F32, tag="band1")
            for c in range(CH):
                xsq = sb3.tile([128, 512], F16, tag="xsq", name="xsq")
                xt = xT_sb[:, c * T + tt * 512: c * T + tt * 512 + 512]
                nc.vector.tensor_tensor(xsq[:], xt, xt, op=OP.mult)
                nc.tensor.matmul(s2[:], ones_col[:], xsq[:],
                                 start=(c == 0), stop=(c == CH - 1))
            t1 = sb2.tile([1, 512], F32, tag="t1", name="t1")
            nc.scalar.activation(t1[:], s1[:], AF.Square)
            vv = sb2.tile([1, 512], F32, tag="vv", name="vv")
            nc.vector.scalar_tensor_tensor(vv[:], s2[:], float(D), t1[:],
                                           op0=OP.mult, op1=OP.subtract)
            stdh = sb2.tile([1, 512], F32, tag="stdh", name="stdh")
            nc.scalar.activation(stdh[:], vv[:], AF.Sqrt,
                                 scale=1.0 / (D * D), bias=eps_col[:])
            a_half = a_row[:, tt * 512:(tt + 1) * 512]
            nc.vector.reciprocal(a_half, stdh[:])
            nc.vector.scalar_tensor_tensor(
                b_row[:, tt * 512:(tt + 1) * 512], s1[:], -1.0 / D, a_half,
                op0=OP.mult, op1=OP.mult)
        nc.gpsimd.partition_broadcast(arep[:], a_row[:])
        nc.gpsimd.partition_broadcast(brep[:], b_row[:])
        masks.make_identity(nc, ident16[:])
        masks.make_identity(nc, ident8[:])

        # ---- LayerNorm apply: yT = xT * a + b (f16), 512-col pieces so
        # the first projections start after the nt=0 halves ----
        for nt in range(2):
            for c in range(CH):
                eng = nc.vector if c % 2 == 0 else nc.gpsimd
                xs = xT_sb[:, c * T + nt * 512: c * T + nt * 512 + 512]
                ys = yT_sb[:, c * T + nt * 512: c * T + nt * 512 + 512]
                ar = arep[:, nt * 512:(nt + 1) * 512]
                br = brep[:, nt * 512:(nt + 1) * 512]
                eng.tensor_tensor(ys, xs, ar, op=OP.mult)
                eng.tensor_tensor(ys, ys, br, op=OP.add)

        # pad pT columns
        z8 = sb.tile([128, 4], F8)
        nc.vector.memset(z8[:], 0.0)
        zrow = sb.tile([128, 2], F16)
        nc.vector.memset(zrow[:], 0.0)
        nc.vector.tensor_copy(posT_sb[:, CH * L:], zrow[:])

        # ---- projection emitters ----
        # extraction engines are spread (qc->Act, qp->DVE, k/p->Pool,
        # v->DVE) so consecutive users of the psum ping-pong release in
        # parallel queues.
        def q_proj(p, nt, pool=None, ptag="sc"):
            prj = (pool or ps_sc).tile([128, 512], F32, tag=ptag,
                                       name="prj")
            for ci, c in enumerate((0, 2, 1, 3)):
                nc.tensor.matmul(
                    prj[:],
                    wq_sb[:, c * 256 + p * 128: c * 256 + p * 128 + 128],
                    yT_sb[:, c * T + nt * 512: c * T + nt * 512 + 512],
                    start=(c == 0), stop=(c == CH - 1))
            o = p * T + nt * 512
            nc.scalar.activation(qcT_sb[:, o:o + 512], prj[:],
                                 AF.Identity, bias=qcb_sb[:, p:p + 1])
            nc.vector.tensor_scalar_add(qpT_sb[:, o:o + 512], prj[:],
                                        qpb_sb[:, p:p + 1])

        def k_proj(p, nt, pool=None, ptag="sc"):
            prjk = (pool or ps_sc).tile([128, 512], F32, tag=ptag,
                                        name="prjk")
            for ci, c in enumerate((0, 2, 1, 3)):
                nc.tensor.matmul(
                    prjk[:],
                    wk_sb[:, c * 256 + p * 128: c * 256 + p * 128 + 128],
                    yT_sb[:, c * T + nt * 512: c * T + nt * 512 + 512],
                    start=(ci == 0), stop=(ci == CH - 1))
            nc.vector.tensor_scalar_add(kT_sb[:, p * T + nt * 512:
                                              p * T + nt * 512 + 512],
                                        prjk[:], kb_sb[:, p:p + 1])

        def p_proj(p, nt, pool=None, ptag="sc"):
            pps = (pool or ps_sc).tile([128, 512], F32, tag=ptag,
                                       name="pps")
            for ci, c in enumerate((0, 2, 1, 3)):
                nc.tensor.matmul(
                    pps[:],
                    wp_sb[:, c * 256 + p * 128: c * 256 + p * 128 + 128],
                    posT_sb[:, c * L + nt * 512: c * L + nt * 512 + 512],
                    start=(ci == 0), stop=(ci == CH - 1))
            nc.scalar.copy(
                pT_sb[:, p * PL + nt * 512: p * PL + nt * 512 + 512],
                pps[:])
            if nt == 3:
                nc.gpsimd.tensor_copy(pT_sb[:, p * PL + L: (p + 1) * PL],
                                      z8[:, :PL - L])

        def v_proj(t8, pool=None, ptag="sc"):
            vps = (pool or ps_sc).tile([128, 256], F32, tag=ptag,
                                       name="vps")
            for c in range(CH):
                nc.tensor.matmul(
                    vps[:],
                    yT_sb[:, c * T + t8 * 128: c * T + t8 * 128 + 128],
                    wv_sb[:, c * 256:(c + 1) * 256],
                    start=(c == 0), stop=(c == CH - 1))
            nc.vector.tensor_tensor(
                v_sb[:, t8 * 256:(t8 + 1) * 256], vps[:], vb_sb[:],
                op=OP.add)

        # ---- attention emitters ----
        def hsl(tile_, h, lo, w, stride=T):
            p, off = h // 2, (h % 2) * 64
            return tile_[off:off + 64, p * stride + lo: p * stride + lo + w]

        def band_qb(h, qb, shifted):
            """Band scores for (h, qb), bounce out, and the skewed read of
            this qb's block straight back into `shifted`. Three psum tiles,
            one extraction engine each, so the next band matmul waits on
            exactly one engine per tile."""
            ping = h % 2
            s0 = 897 - qb * 128
            qp_blk = hsl(qpT_sb, h, qb * 128, 128)
            bps0 = ps_b.tile([128, 768], F32, tag="band0")
            nc.tensor.matmul(bps0[:, :512], qp_blk,
                             hsl(pT_sb, h, s0, 512, stride=PL),
                             start=True, stop=True)
            nc.tensor.matmul(bps0[:, 512:], qp_blk,
                             hsl(pT_sb, h, s0 + 512, 256, stride=PL),
                             start=True, stop=True)
            bps1 = ps_b.tile([128, 384], F32, tag="band1")
            nc.tensor.matmul(bps1[:], qp_blk,
                             hsl(pT_sb, h, s0 + 768, 384, stride=PL),
                             start=True, stop=True)
            b8 = sb2.tile([128, BAND], F8, tag="band8", name="b8")
            if h == 0:
                # Act is exp-free before head 0's scores; give it the
                # wide slice so the DVE prefix backlog stays short
                nc.scalar.copy(b8[:, :768], bps0[:])
                nc.vector.tensor_copy(b8[:, 768:], bps1[:])
            else:
                nc.vector.tensor_copy(b8[:, :768], bps0[:])
                nc.scalar.copy(b8[:, 768:], bps1[:])
            nc.sync.dma_start(bounce[ping, qb], b8[:])
            if qb % 2 == 1:
                src = bass.AP(bounce[:].tensor,
                              (ping * QB + qb - 1) * (128 * BAND) + 127,
                              [[BAND - 1, 128], [128 * BAND, 2], [1, T]])
                nc.sync.dma_start(
                    shifted[:, (qb - 1) * T:(qb + 1) * T], src)

        def shifted_tile():
            shifted = sb2.tile([128, QB * T], F8, tag="shifted",
                               name="shifted")
            return shifted

        def scores_qb(h, qb, shifted):
            """Scores+exp+normalize for (h, qb); returns the E tile.
            Transposes are emitted one qb later (transpose_qb) so the PE
            queue never waits on the Act/DVE chain of the same qb."""
            E_sb = sbE.tile([128, T], F16, tag="E", name="E")
            den = sb2.tile([128, 2], F32, tag="den", name="den")
            for half in range(2):
                sps = ps_sc.tile([128, 512], F32, tag="sc")
                nc.tensor.matmul(
                    sps[:],
                    hsl(qcT_sb, h, qb * 128, 128),
                    hsl(kT_sb, h, half * 512, 512),
                    start=True, stop=False)
                if qb == 0 and half == 1:
                    # scores[0, 1023] += (q+pos_bias)[1] . p[0]
                    # (the reference rel_shift reshape wraps this element)
                    nc.tensor.matmul(
                        sps[0:1, 511:512],
                        hsl(qpT_sb, h, 1, 1),
                        hsl(pT_sb, h, 0, 1, stride=PL),
                        start=False, stop=False)
                nc.tensor.matmul(
                    sps[:],
                    ident8[:],
                    shifted[:, qb * T + half * 512:
                            qb * T + half * 512 + 512],
                    start=False, stop=True)
                nc.scalar.activation(
                    E_sb[:, half * 512:(half + 1) * 512], sps[:], AF.Exp,
                    scale=0.125, accum_out=den[:, half:half + 1])
            rec = sb2.tile([128, 1], F32, tag="rec", name="rec")
            nc.gpsimd.tensor_tensor(rec[:], den[:, 0:1], den[:, 1:2],
                                    op=OP.add)
            nc.vector.reciprocal(rec[:], rec[:])
            nc.vector.tensor_scalar_mul(E_sb[:], E_sb[:], rec[:])
            return E_sb

        def transpose_qb(qb, E_sb, ET_sb):
            # transpose E (f16): 8 PE transposes -> one f16 psum bank
            etps = ps_et.tile([128, T], F16, tag="et")
            for kc in range(QB):
                nc.tensor.transpose(
                    etps[:, kc * 128: kc * 128 + 128],
                    E_sb[:, kc * 128: kc * 128 + 128],
                    ident16[:])
            # scatter-extract: ET_sb[:, kc*T + qb*128 ...] = etps block kc
            dst = bass.AP(ET_sb[:].tensor, qb * 128,
                          [[QB * T, 128], [T, QB], [1, 128]])
            nc.vector.tensor_copy(dst, etps[:])

        def attnv_chunk(p, ib, hh, kh, ET_pair, otps):
            hloc = 2 * p + hh
            for kc in range(4 * kh, 4 * kh + 4):
                nc.tensor.matmul(
                    otps[hh * 64:hh * 64 + 64, ib * 512:ib * 512 + 512],
                    v_sb[:, kc * 256 + hloc * 64: kc * 256 + hloc * 64 + 64],
                    ET_pair[hh][:, kc * T + ib * 512: kc * T + ib * 512 + 512],
                    start=(kc == 0), stop=(kc == QB - 1))

        def attnv_extract(p, otps):
            eng = nc.vector if p % 2 == 0 else nc.scalar
            if p % 2 == 0:
                nc.vector.tensor_copy(oT_sb[:, p * T:(p + 1) * T], otps[:])
            else:
                nc.scalar.copy(oT_sb[:, p * T:(p + 1) * T], otps[:])

        def out_proj(t8):
            ops_ = ps_sc.tile([128, 512], F32, tag="sc")
            for p in range(NP):
                nc.tensor.matmul(
                    ops_[:],
                    oT_sb[:, p * T + t8 * 128: p * T + t8 * 128 + 128],
                    wo_sb[:, p * D:(p + 1) * D],
                    start=(p == 0), stop=(p == NP - 1))
            osb = sb2.tile([128, 512], F16, tag="osb", name="osb")
            nc.vector.tensor_copy(osb[:, :256], ops_[:, :256])
            nc.scalar.copy(osb[:, 256:], ops_[:, 256:])
            nc.sync.dma_start(out_d[t8 * 128:(t8 + 1) * 128, :], osb[:])

        # ---- emission schedule ----
        # p proj first (needs no LayerNorm -> fills the stats-chain
        # latency). Only pair-0 projections + head-0 bands run before the
        # attention loop; pair-1 projections and v-proj are deferred as
        # per-q-block filler inside the head-0/1 loops so the in-order PE
        # queue reaches head-0 scores ~12us earlier.
        for nt in range(4):
            p_proj(0, nt)
        q_proj(0, 0)
        k_proj(0, 0)
        q_proj(0, 1)
        k_proj(0, 1)
        shifted = {0: shifted_tile()}
        for qb in range(QB):
            band_qb(0, qb, shifted[0])

        filler = {
            0: [lambda: q_proj(1, 0, ps_o, "o"),
                lambda: q_proj(1, 1, ps_o, "o"),
                lambda: k_proj(1, 0, ps_o, "o"),
                lambda: k_proj(1, 1, ps_o, "o"),
                lambda: p_proj(1, 0, ps_o, "o"),
                lambda: p_proj(1, 1, ps_o, "o"),
                lambda: p_proj(1, 2, ps_o, "o"),
                lambda: p_proj(1, 3, ps_o, "o")],
            1: [(lambda t8=t8: v_proj(t8, ps_o, "o")) for t8 in range(QB)],
        }

        ET_tiles = {}
        attnv_work = []   # deferred attn@V chunks for the previous pair
        for h in range(NH):
            # bufs=3: head h+2 must not wait on the deferred attn@V reads
            # of head h's ET (they interleave into head h+2's score loop)
            ET_sb = sb3.tile([128, QB * T], F16, tag="ET", name="ET")
            ET_tiles[h] = ET_sb
            if h + 1 < NH:
                shifted[h + 1] = shifted_tile()
            E_hist = []
            for qb in range(QB):
                if h + 1 < NH:
                    band_qb(h + 1, qb, shifted[h + 1])
                E_hist.append(scores_qb(h, qb, shifted[h]))
                if qb >= 2:
                    transpose_qb(qb - 2, E_hist[qb - 2], ET_sb)
                for fn_ in filler.get(h, [])[qb:qb + 1]:
                    fn_()
                # slot one deferred attn@V chunk of the previous pair
                if attnv_work:
                    attnv_work.pop(0)()
            transpose_qb(QB - 2, E_hist[QB - 2], ET_sb)
            transpose_qb(QB - 1, E_hist[QB - 1], ET_sb)
            if h % 2 == 1:
                p = h // 2
                ET_pair = (ET_tiles[2 * p], ET_tiles[2 * p + 1])
                chunks = []
                state = {}

                def make_chunk(p_, ib_, hh_, kh_, ET_pair_):
                    def go():
                        if ("ot", p_) not in state:
                            state[("ot", p_)] = ps_o.tile(
                                [128, 2 * 512], F32, tag="o", name="otps")
                        otps = state[("ot", p_)]
                        attnv_chunk(p_, ib_, hh_, kh_, ET_pair_, otps)
                        if ib_ == 1 and hh_ == 1 and kh_ == 1:
                            attnv_extract(p_, otps)
                    return go

                for ib in range(2):
                    for hh in range(2):
                        for kh in range(2):
                            chunks.append(make_chunk(p, ib, hh, kh, ET_pair))
                if h == NH - 1:
                    for cfn in chunks:
                        cfn()
                    for t8 in range(QB):
                        out_proj(t8)
                else:
                    attnv_work.extend(chunks)

    nc.compile()
    return nc


_PROGRAM_CACHE: dict = {}


def _get_program() -> bass.Bass:
    if "nc" not in _PROGRAM_CACHE:
        _PROGRAM_CACHE["nc"] = _build_program()
    return _PROGRAM_CACHE["nc"]


def _prepare_in_maps(x, pos, content_bias, pos_bias, gamma, beta,
                     Wq, bq, Wk, bk, Wv, bv, Wp, Wo, bo):
    x = np.asarray(x, np.float32)
    pos = np.asarray(pos, np.float32)
    gamma = np.asarray(gamma, np.float32)
    beta = np.asarray(beta, np.float32)

    # gamma folding: y = yln*gamma + beta  =>  y@W = yln@(gamma*W) + beta@W
    def fold(W):
        W = np.asarray(W, np.float32)
        return W * gamma[:, None, None], np.einsum("d,dhk->hk", beta, W)

    Wq_f, bq_f = fold(Wq)
    Wk_f, bk_f = fold(Wk)
    Wv_f, bv_f = fold(Wv)
    Wp = np.asarray(Wp, np.float32)
    Wo = np.asarray(Wo, np.float32)

    in_maps = []
    for core in range(8):
        b = core // 2
        g = core % 2
        hs = slice(4 * g, 4 * g + 4)
        qcb = (np.asarray(bq) + np.asarray(content_bias) + bq_f)[hs]
        qpb = (np.asarray(bq) + np.asarray(pos_bias) + bq_f)[hs]
        kb = (np.asarray(bk) + bk_f)[hs]
        vb = (np.asarray(bv) + bv_f)[hs]
        # Wo pair-stacked: [128, NP*D]; pair p rows = Wo[2p] ++ Wo[2p+1]
        Wo_h = np.asarray(Wo)[hs]          # [4, DK, D]
        wo2 = np.concatenate(
            [np.concatenate([Wo_h[2 * p], Wo_h[2 * p + 1]], axis=0)
             for p in range(NP)], axis=1)  # [128, NP*D]
        in_maps.append({
            "xT": np.ascontiguousarray(x[b].T).astype(np.float16),
            "posT": np.ascontiguousarray(pos[b].T).astype(np.float16),
            "wq": np.ascontiguousarray(
                Wq_f[:, hs, :].reshape(D, NH * DK)).astype(np.float16),
            "wk": np.ascontiguousarray(
                Wk_f[:, hs, :].reshape(D, NH * DK)).astype(np.float16),
            "wv": np.ascontiguousarray(
                Wv_f[:, hs, :].reshape(D, NH * DK)).astype(np.float16),
            "wp": np.ascontiguousarray(
                Wp[:, hs, :].reshape(D, NH * DK)).astype(np.float16),
            "wo": np.ascontiguousarray(wo2).astype(np.float16),
            "qc_bias": np.ascontiguousarray(qcb.reshape(2, 128).T),
            "qp_bias": np.ascontiguousarray(qpb.reshape(2, 128).T),
            "k_bias": np.ascontiguousarray(kb.reshape(2, 128).T),
            "v_bias": np.ascontiguousarray(vb.reshape(NH * DK)),
        })

    return in_maps


def _combine(x, bo, results):
    parts = [r["out_partial"] for r in results]
    out = np.asarray(x, np.float32) + np.asarray(bo, np.float32)[None, None, :]
    for b in range(B):
        out[b] += parts[2 * b].astype(np.float32)
        out[b] += parts[2 * b + 1].astype(np.float32)
    return out.astype(np.float32)


def kernel(x, pos, content_bias, pos_bias, gamma, beta,
           Wq, bq, Wk, bk, Wv, bv, Wp, Wo, bo) -> np.ndarray:
    in_maps = _prepare_in_maps(x, pos, content_bias, pos_bias, gamma, beta,
                               Wq, bq, Wk, bk, Wv, bv, Wp, Wo, bo)
    nc = _get_program()
    res = run_bass_kernel_spmd(nc, in_maps, core_ids=list(range(8)))
    return _combine(x, bo, res.results)


# revision 5
# speedup vs baseline: 150.2162x; 1.0714x over previous
"""Trainium2 Bass kernel v2 for Transformer-XL style MHSA (nn_MHSAModule).

Problem (hardcoded):
  B=4, T=1024, D=512, H=8, DK=64, L=2*T-1=2047, eps=1e-3
  out = x + (MHSA(LayerNorm(x), pos) @ Wo + bo)

Sharding: 8 cores = 4 batches x 2 head-groups (4 heads each). Each core
returns a partial output [T, D] f16 (its heads' contribution); the host
sums the two partials per batch and adds the residual x + bo.

v2 design notes (vs v1):
  - f16 activations/weights on the matmul path; f8e4m3 only for matmul
    B-operands no vector engine reads back: kT, pT, and the rel-shift
    band bounce (halves its DMA volume).
  - exp runs on Act straight from PSUM with accum_out giving the softmax
    denominator for free; E is normalized by one per-partition f16
    tensor_scalar instead of v1's psum-copy + scale chain.
  - PSUM extraction is the scarce resource (~4x an SBUF read): band
    extraction is split across DVE/Act/Pool, scores are extracted by the
    exp itself, ET extracted as f16 (2x cheaper than f32).
  - LayerNorm stats via f16 ones-matmuls; a/b rows reach all partitions
    via gpsimd.partition_broadcast instead of a DRAM round-trip.
  - attention@V stacks head pairs on 128 PSUM partitions; Wo is
    pair-stacked so the output projection contracts 128 rows per step.
  - engines execute in-order, so emission order IS the schedule: x/wq
    load first, the band of head h+1 and the attn@V of the previous pair
    are interleaved into head h's per-q-block score loop.
"""
import numpy as np
from contextlib import ExitStack

import concourse.bass as bass
import concourse.bacc as bacc
import concourse.tile as tile
from concourse import mybir
from concourse import masks
from concourse.bass_utils import run_bass_kernel_spmd

F32 = mybir.dt.float32
F16 = mybir.dt.float16
F8 = mybir.dt.float8e4
AF = mybir.ActivationFunctionType
OP = mybir.AluOpType

B, T, D, H, DK = 4, 1024, 512, 8, 64
L = 2 * T - 1
EPS = 1e-3
NH = 4          # heads per core
NP = 2          # head pairs per core
CH = D // 128   # 4 contraction chunks
QB = T // 128   # 8 q blocks
BAND = 1152    # positional band width per q block
PL = L + 2      # padded pT free size (2 zero pad cols)


def _build_program() -> bass.Bass:
    nc = bacc.Bacc("TRN2", target_bir_lowering=False, debug=False)

    # ---- DRAM I/O ----
    xT = nc.dram_tensor("xT", [D, T], F16, kind="ExternalInput")
    posT = nc.dram_tensor("posT", [D, L], F16, kind="ExternalInput")
    wq = nc.dram_tensor("wq", [D, NH * DK], F16, kind="ExternalInput")
    wk = nc.dram_tensor("wk", [D, NH * DK], F16, kind="ExternalInput")
    wv = nc.dram_tensor("wv", [D, NH * DK], F16, kind="ExternalInput")
    wp = nc.dram_tensor("wp", [D, NH * DK], F16, kind="ExternalInput")
    wo = nc.dram_tensor("wo", [128, NP * D], F16, kind="ExternalInput")
    qc_bias = nc.dram_tensor("qc_bias", [128, NP], F32, kind="ExternalInput")
    qp_bias = nc.dram_tensor("qp_bias", [128, NP], F32, kind="ExternalInput")
    k_bias = nc.dram_tensor("k_bias", [128, NP], F32, kind="ExternalInput")
    v_bias = nc.dram_tensor("v_bias", [NH * DK], F32, kind="ExternalInput")
    out_d = nc.dram_tensor("out_partial", [T, D], F16, kind="ExternalOutput")

    # internal scratch: rel-shift bounce, f8, double buffered
    bounce = nc.dram_tensor("bounce", [2, QB, 128, BAND], F8)

    with tile.TileContext(nc) as tc, ExitStack() as ctx:
        sb = ctx.enter_context(tc.tile_pool(name="sb", bufs=1))
        sb2 = ctx.enter_context(tc.tile_pool(name="sb2", bufs=2))
        sb3 = ctx.enter_context(tc.tile_pool(name="sb3", bufs=3))
        sbE = ctx.enter_context(tc.tile_pool(name="sbE", bufs=6))
        # PSUM: sc 2x[128,512]f32 (2 banks) + band 1x[128,1152]f32
        # (3 banks) + et 2x[128,1024]f16 (2 banks) + o 1x[128,512]f32
        # (1 bank) = 8 banks.
        ps_sc = ctx.enter_context(tc.tile_pool(name="ps_sc", bufs=2,
                                               space="PSUM"))
        # band psum: three independently-released tiles so the next band
        # matmul only waits on the one engine that extracts each slice
        ps_b = ctx.enter_context(tc.tile_pool(name="ps_b", bufs=1,
                                              space="PSUM"))
        ps_et = ctx.enter_context(tc.tile_pool(name="ps_et", bufs=2,
                                               space="PSUM"))
        ps_o = ctx.enter_context(tc.tile_pool(name="ps_o", bufs=1,
                                              space="PSUM"))

        # ---- persistent SBUF ----
        xT_sb = sb.tile([128, CH * T], F16)
        yT_sb = sb.tile([128, CH * T], F16)
        posT_sb = sb.tile([128, CH * L + 2], F16)
        pT_sb = sb.tile([128, NP * PL], F8)
        qcT_sb = sb.tile([128, NP * T], F8)
        qpT_sb = sb.tile([128, NP * T], F8)
        kT_sb = sb.tile([128, NP * T], F8)
        v_sb = sb.tile([128, QB * NH * DK], F16)
        oT_sb = sb.tile([128, NP * T], F16)
        wq_sb = sb.tile([128, CH * 256], F16)
        wk_sb = sb.tile([128, CH * 256], F16)
        wv_sb = sb.tile([128, CH * 256], F16)
        wp_sb = sb.tile([128, CH * 256], F16)
        wo_sb = sb.tile([128, NP * D], F16)
        qcb_sb = sb.tile([128, NP], F32)
        qpb_sb = sb.tile([128, NP], F32)
        kb_sb = sb.tile([128, NP], F32)
        vb_sb = sb.tile([128, 256], F32)
        arep = sb.tile([128, T], F32)
        brep = sb.tile([128, T], F32)
        ident16 = sb.tile([128, 128], F16)
        ident8 = sb.tile([128, 128], F8)
        ones_col = sb.tile([128, 1], F16)
        eps_col = sb.tile([1, 1], F32)

        nc.vector.memset(ones_col[:], 1.0)
        nc.vector.memset(eps_col[:], EPS)

        # ---- loads: x first (stats), then pos+wp (p proj), then q/k ----
        for c in range(CH):
            nc.sync.dma_start(xT_sb[:, c * T:(c + 1) * T],
                              xT[c * 128:(c + 1) * 128, :])
        for c in range(CH):
            nc.sync.dma_start(posT_sb[:, c * L:(c + 1) * L],
                              posT[c * 128:(c + 1) * 128, :])
            nc.sync.dma_start(wp_sb[:, c * 256:(c + 1) * 256],
                              wp[c * 128:(c + 1) * 128, :])
        for c in range(CH):
            nc.sync.dma_start(wq_sb[:, c * 256:(c + 1) * 256],
                              wq[c * 128:(c + 1) * 128, :])
            nc.sync.dma_start(wk_sb[:, c * 256:(c + 1) * 256],
                              wk[c * 128:(c + 1) * 128, :])
        nc.sync.dma_start(qcb_sb[:], qc_bias[:])
        nc.sync.dma_start(qpb_sb[:], qp_bias[:])
        nc.sync.dma_start(kb_sb[:], k_bias[:])
        for c in range(CH):
            nc.sync.dma_start(wv_sb[:, c * 256:(c + 1) * 256],
                              wv[c * 128:(c + 1) * 128, :])
        nc.sync.dma_start(wo_sb[:], wo[:])
        nc.sync.dma_start(
            vb_sb[:], bass.AP(v_bias[:].tensor, 0, [[0, 128], [1, 256]]))

        # ---- LayerNorm stats; short [1,512]-row chain (row ops are
        # lane-serial, so every op counts): a = rsqrt(var+eps),
        # b = -mu*a, computed as
        #   t1 = s1*s1; v' = D*s2 - t1; a = Rsqrt(v'/D^2 + eps);
        #   b = (s1 * -1/D) * a
        a_row = sb.tile([1, T], F32)
        b_row = sb.tile([1, T], F32)
        for tt in range(2):
            s1 = ps_b.tile([1, 512], F32, tag="band0")
            for c in range(CH):
                xt = xT_sb[:, c * T + tt * 512: c * T + tt * 512 + 512]
                nc.tensor.matmul(s1[:], ones_col[:], xt,
                                 start=(c == 0), stop=(c == CH - 1))
            s2 = ps_b.tile([1, 512], F32, tag="band1")
            for c in range(CH):
                xsq = sb3.tile([128, 512], F16, tag="xsq", name="xsq")
                xt = xT_sb[:, c * T + tt * 512: c * T + tt * 512 + 512]
                nc.vector.tensor_tensor(xsq[:], xt, xt, op=OP.mult)
                nc.tensor.matmul(s2[:], ones_col[:], xsq[:],
                                 start=(c == 0), stop=(c == CH - 1))
            t1 = sb2.tile([1, 512], F32, tag="t1", name="t1")
            nc.scalar.activation(t1[:], s1[:], AF.Square)
            vv = sb2.tile([1, 512], F32, tag="vv", name="vv")
            nc.vector.scalar_tensor_tensor(vv[:], s2[:], float(D), t1[:],
                                           op0=OP.mult, op1=OP.subtract)
            stdh = sb2.tile([1, 512], F32, tag="stdh", name="stdh")
            nc.scalar.activation(stdh[:], vv[:], AF.Sqrt,
                                 scale=1.0 / (D * D), bias=eps_col[:])
            a_half = a_row[:, tt * 512:(tt + 1) * 512]
            nc.vector.reciprocal(a_half, stdh[:])
            nc.vector.scalar_tensor_tensor(
                b_row[:, tt * 512:(tt + 1) * 512], s1[:], -1.0 / D, a_half,
                op0=OP.mult, op1=OP.mult)
        nc.gpsimd.partition_broadcast(arep[:], a_row[:])
        nc.gpsimd.partition_broadcast(brep[:], b_row[:])
        masks.make_identity(nc, ident16[:])
        masks.make_identity(nc, ident8[:])

        # ---- LayerNorm apply: yT = xT * a + b (f16), 512-col pieces so
        # the first projections start after the nt=0 halves ----
        for nt in range(2):
            for c in range(CH):
                eng = nc.gpsimd if c == 1 else nc.vector
                xs = xT_sb[:, c * T + nt * 512: c * T + nt * 512 + 512]
                ys = yT_sb[:, c * T + nt * 512: c * T + nt * 512 + 512]
                ar = arep[:, nt * 512:(nt + 1) * 512]
                br = brep[:, nt * 512:(nt + 1) * 512]
                eng.tensor_tensor(ys, xs, ar, op=OP.mult)
                eng.tensor_tensor(ys, ys, br, op=OP.add)

        # pad pT columns
        z8 = sb.tile([128, 4], F8)
        nc.vector.memset(z8[:], 0.0)
        zrow = sb.tile([128, 2], F16)
        nc.vector.memset(zrow[:], 0.0)
        nc.vector.tensor_copy(posT_sb[:, CH * L:], zrow[:])

        # ---- projection emitters ----
        # extraction engines are spread (qc->Act, qp->DVE, k/p->Pool,
        # v->DVE) so consecutive users of the psum ping-pong release in
        # parallel queues.
        def q_proj(p, nt, pool=None, ptag="sc"):
            prj = (pool or ps_sc).tile([128, 512], F32, tag=ptag,
                                       name="prj")
            for ci, c in enumerate((0, 2, 1, 3)):
                nc.tensor.matmul(
                    prj[:],
                    wq_sb[:, c * 256 + p * 128: c * 256 + p * 128 + 128],
                    yT_sb[:, c * T + nt * 512: c * T + nt * 512 + 512],
                    start=(c == 0), stop=(c == CH - 1))
            o = p * T + nt * 512
            nc.scalar.activation(qcT_sb[:, o:o + 512], prj[:],
                                 AF.Identity, bias=qcb_sb[:, p:p + 1])
            nc.vector.tensor_scalar_add(qpT_sb[:, o:o + 512], prj[:],
                                        qpb_sb[:, p:p + 1])

        def k_proj(p, nt, pool=None, ptag="sc"):
            prjk = (pool or ps_sc).tile([128, 512], F32, tag=ptag,
                                        name="prjk")
            for ci, c in enumerate((0, 2, 1, 3)):
                nc.tensor.matmul(
                    prjk[:],
                    wk_sb[:, c * 256 + p * 128: c * 256 + p * 128 + 128],
                    yT_sb[:, c * T + nt * 512: c * T + nt * 512 + 512],
                    start=(ci == 0), stop=(ci == CH - 1))
            nc.vector.tensor_scalar_add(kT_sb[:, p * T + nt * 512:
                                              p * T + nt * 512 + 512],
                                        prjk[:], kb_sb[:, p:p + 1])

        def p_proj(p, nt, pool=None, ptag="sc"):
            pps = (pool or ps_sc).tile([128, 512], F32, tag=ptag,
                                       name="pps")
            for ci, c in enumerate((0, 2, 1, 3)):
                nc.tensor.matmul(
                    pps[:],
                    wp_sb[:, c * 256 + p * 128: c * 256 + p * 128 + 128],
                    posT_sb[:, c * L + nt * 512: c * L + nt * 512 + 512],
                    start=(ci == 0), stop=(ci == CH - 1))
            nc.scalar.copy(
                pT_sb[:, p * PL + nt * 512: p * PL + nt * 512 + 512],
                pps[:])
            if nt == 3:
                nc.gpsimd.tensor_copy(pT_sb[:, p * PL + L: (p + 1) * PL],
                                      z8[:, :PL - L])

        def v_proj(t8, pool=None, ptag="sc"):
            vps = (pool or ps_sc).tile([128, 256], F32, tag=ptag,
                                       name="vps")
            for c in range(CH):
                nc.tensor.matmul(
                    vps[:],
                    yT_sb[:, c * T + t8 * 128: c * T + t8 * 128 + 128],
                    wv_sb[:, c * 256:(c + 1) * 256],
                    start=(c == 0), stop=(c == CH - 1))
            nc.vector.tensor_tensor(
                v_sb[:, t8 * 256:(t8 + 1) * 256], vps[:], vb_sb[:],
                op=OP.add)

        # ---- attention emitters ----
        def hsl(tile_, h, lo, w, stride=T):
            p, off = h // 2, (h % 2) * 64
            return tile_[off:off + 64, p * stride + lo: p * stride + lo + w]

        def band_qb(h, qb, shifted):
            """Band scores for (h, qb), bounce out, and the skewed read of
            this qb's block straight back into `shifted`. Three psum tiles,
            one extraction engine each, so the next band matmul waits on
            exactly one engine per tile."""
            ping = h % 2
            s0 = 897 - qb * 128
            qp_blk = hsl(qpT_sb, h, qb * 128, 128)
            bps0 = ps_b.tile([128, 768], F32, tag="band0")
            nc.tensor.matmul(bps0[:, :512], qp_blk,
                             hsl(pT_sb, h, s0, 512, stride=PL),
                             start=True, stop=True)
            nc.tensor.matmul(bps0[:, 512:], qp_blk,
                             hsl(pT_sb, h, s0 + 512, 256, stride=PL),
                             start=True, stop=True)
            bps1 = ps_b.tile([128, 384], F32, tag="band1")
            nc.tensor.matmul(bps1[:], qp_blk,
                             hsl(pT_sb, h, s0 + 768, 384, stride=PL),
                             start=True, stop=True)
            b8 = sb3.tile([128, BAND], F8, tag="band8", name="b8")
            nc.vector.tensor_copy(b8[:, :768], bps0[:])
            nc.scalar.copy(b8[:, 768:], bps1[:])
            nc.sync.dma_start(bounce[ping, qb], b8[:])
            if qb % 2 == 1:
                src = bass.AP(bounce[:].tensor,
                              (ping * QB + qb - 1) * (128 * BAND) + 127,
                              [[BAND - 1, 128], [128 * BAND, 2], [1, T]])
                nc.sync.dma_start(
                    shifted[:, (qb - 1) * T:(qb + 1) * T], src)

        def shifted_tile():
            shifted = sb2.tile([128, QB * T], F8, tag="shifted",
                               name="shifted")
            return shifted

        def scores_qb(h, qb, shifted):
            """Scores+exp+normalize for (h, qb); returns the E tile.
            Transposes are emitted one qb later (transpose_qb) so the PE
            queue never waits on the Act/DVE chain of the same qb."""
            E_sb = sbE.tile([128, T], F16, tag="E", name="E")
            den = sb3.tile([128, 2], F32, tag="den", name="den")
            for half in range(2):
                sps = ps_sc.tile([128, 512], F32, tag="sc")
                nc.tensor.matmul(
                    sps[:],
                    hsl(qcT_sb, h, qb * 128, 128),
                    hsl(kT_sb, h, half * 512, 512),
                    start=True, stop=False)
                if qb == 0 and half == 1:
                    # scores[0, 1023] += (q+pos_bias)[1] . p[0]
                    # (the reference rel_shift reshape wraps this element)
                    nc.tensor.matmul(
                        sps[0:1, 511:512],
                        hsl(qpT_sb, h, 1, 1),
                        hsl(pT_sb, h, 0, 1, stride=PL),
                        start=False, stop=False)
                nc.tensor.matmul(
                    sps[:],
                    ident8[:],
                    shifted[:, qb * T + half * 512:
                            qb * T + half * 512 + 512],
                    start=False, stop=True)
                nc.scalar.activation(
                    E_sb[:, half * 512:(half + 1) * 512], sps[:], AF.Exp,
                    scale=0.125, accum_out=den[:, half:half + 1])
            rec = sb3.tile([128, 1], F32, tag="rec", name="rec")
            nc.gpsimd.tensor_tensor(rec[:], den[:, 0:1], den[:, 1:2],
                                    op=OP.add)
            nc.vector.reciprocal(rec[:], rec[:])
            nc.vector.tensor_scalar_mul(E_sb[:], E_sb[:], rec[:])
            return E_sb

        def transpose_qb(qb, E_sb, ET_sb):
            # transpose E (f16): 8 PE transposes -> one f16 psum bank
            etps = ps_et.tile([128, T], F16, tag="et")
            for kc in range(QB):
                nc.tensor.transpose(
                    etps[:, kc * 128: kc * 128 + 128],
                    E_sb[:, kc * 128: kc * 128 + 128],
                    ident16[:])
            # scatter-extract: ET_sb[:, kc*T + qb*128 ...] = etps block kc
            dst = bass.AP(ET_sb[:].tensor, qb * 128,
                          [[QB * T, 128], [T, QB], [1, 128]])
            nc.vector.tensor_copy(dst, etps[:])

        def attnv_chunk(p, ib, hh, kh, ET_pair, otps):
            hloc = 2 * p + hh
            for kc in range(4 * kh, 4 * kh + 4):
                nc.tensor.matmul(
                    otps[hh * 64:hh * 64 + 64, :],
                    v_sb[:, kc * 256 + hloc * 64: kc * 256 + hloc * 64 + 64],
                    ET_pair[hh][:, kc * T + ib * 512: kc * T + ib * 512 + 512],
                    start=(kc == 0), stop=(kc == QB - 1))

        def attnv_extract(p, ib, otps):
            dst = oT_sb[:, p * T + ib * 512: p * T + ib * 512 + 512]
            if (p + ib) % 2 == 0:
                nc.vector.tensor_copy(dst, otps[:])
            else:
                nc.scalar.copy(dst, otps[:])

        def out_proj(t8):
            ops_ = ps_sc.tile([128, 512], F32, tag="sc")
            for p in range(NP):
                nc.tensor.matmul(
                    ops_[:],
                    oT_sb[:, p * T + t8 * 128: p * T + t8 * 128 + 128],
                    wo_sb[:, p * D:(p + 1) * D],
                    start=(p == 0), stop=(p == NP - 1))
            osb = sb2.tile([128, 512], F16, tag="osb", name="osb")
            nc.vector.tensor_copy(osb[:, :256], ops_[:, :256])
            nc.scalar.copy(osb[:, 256:], ops_[:, 256:])
            nc.sync.dma_start(out_d[t8 * 128:(t8 + 1) * 128, :], osb[:])

        # ---- emission schedule ----
        # p proj first (needs no LayerNorm -> fills the stats-chain
        # latency). Only pair-0 projections + head-0 bands run before the
        # attention loop; pair-1 projections and v-proj are deferred as
        # per-q-block filler inside the head-0/1 loops so the in-order PE
        # queue reaches head-0 scores ~12us earlier.
        for nt in range(4):
            p_proj(0, nt)
        q_proj(0, 0)
        k_proj(0, 0)
        q_proj(0, 1)
        k_proj(0, 1)
        shifted = {0: shifted_tile()}
        for qb in range(QB):
            band_qb(0, qb, shifted[0])

        filler = {
            0: [lambda: q_proj(1, 0, ps_o, "o"),
                lambda: q_proj(1, 1, ps_o, "o"),
                lambda: k_proj(1, 0, ps_o, "o"),
                lambda: k_proj(1, 1, ps_o, "o"),
                lambda: p_proj(1, 0, ps_o, "o"),
                lambda: p_proj(1, 1, ps_o, "o"),
                lambda: p_proj(1, 2, ps_o, "o"),
                lambda: p_proj(1, 3, ps_o, "o")],
            1: [(lambda t8=t8: v_proj(t8, ps_o, "o")) for t8 in range(QB)],
        }

        ET_tiles = {}
        attnv_work = []   # deferred attn@V chunks for the previous pair
        for h in range(NH):
            # bufs=3: head h+2 must not wait on the deferred attn@V reads
            # of head h's ET (they interleave into head h+2's score loop)
            ET_sb = sb3.tile([128, QB * T], F16, tag="ET", name="ET")
            ET_tiles[h] = ET_sb
            if h + 1 < NH:
                shifted[h + 1] = shifted_tile()
            E_hist = []
            for qb in range(QB):
                if h + 1 < NH:
                    band_qb(h + 1, qb, shifted[h + 1])
                E_hist.append(scores_qb(h, qb, shifted[h]))
                if qb >= 2:
                    transpose_qb(qb - 2, E_hist[qb - 2], ET_sb)
                for fn_ in filler.get(h, [])[qb:qb + 1]:
                    fn_()
                # slot one deferred attn@V chunk of the previous pair
                if attnv_work:
                    attnv_work.pop(0)()
            transpose_qb(QB - 2, E_hist[QB - 2], ET_sb)
            transpose_qb(QB - 1, E_hist[QB - 1], ET_sb)
            if h % 2 == 1:
                p = h // 2
                ET_pair = (ET_tiles[2 * p], ET_tiles[2 * p + 1])
                chunks = []
                state = {}

                def make_chunk(p_, ib_, hh_, kh_, ET_pair_):
                    def go():
                        if ("ot", p_, ib_) not in state:
                            state[("ot", p_, ib_)] = ps_o.tile(
                                [128, 512], F32, tag="o", name="otps")
                        otps = state[("ot", p_, ib_)]
                        attnv_chunk(p_, ib_, hh_, kh_, ET_pair_, otps)
                        if hh_ == 1 and kh_ == 1:
                            attnv_extract(p_, ib_, otps)
                    return go

                for ib in range(2):
                    for hh in range(2):
                        for kh in range(2):
                            chunks.append(make_chunk(p, ib, hh, kh, ET_pair))
                if h == NH - 1:
                    for cfn in chunks[:4]:
                        cfn()
                    for i, cfn in enumerate(chunks[4:]):
                        out_proj(i)
                        cfn()
                    for t8 in range(4, QB):
                        out_proj(t8)
                else:
                    attnv_work.extend(chunks)

    nc.compile()
    return nc


_PROGRAM_CACHE: dict = {}


def _get_program() -> bass.Bass:
    if "nc" not in _PROGRAM_CACHE:
        _PROGRAM_CACHE["nc"] = _build_program()
    return _PROGRAM_CACHE["nc"]


def _prepare_in_maps(x, pos, content_bias, pos_bias, gamma, beta,
                     Wq, bq, Wk, bk, Wv, bv, Wp, Wo, bo):
    x = np.asarray(x, np.float32)
    pos = np.asarray(pos, np.float32)
    gamma = np.asarray(gamma, np.float32)
    beta = np.asarray(beta, np.float32)

    # gamma folding: y = yln*gamma + beta  =>  y@W = yln@(gamma*W) + beta@W
    def fold(W):
        W = np.asarray(W, np.float32)
        return W * gamma[:, None, None], np.einsum("d,dhk->hk", beta, W)

    Wq_f, bq_f = fold(Wq)
    Wk_f, bk_f = fold(Wk)
    Wv_f, bv_f = fold(Wv)
    Wp = np.asarray(Wp, np.float32)
    Wo = np.asarray(Wo, np.float32)

    in_maps = []
    for core in range(8):
        b = core // 2
        g = core % 2
        hs = slice(4 * g, 4 * g + 4)
        qcb = (np.asarray(bq) + np.asarray(content_bias) + bq_f)[hs]
        qpb = (np.asarray(bq) + np.asarray(pos_bias) + bq_f)[hs]
        kb = (np.asarray(bk) + bk_f)[hs]
        vb = (np.asarray(bv) + bv_f)[hs]
        # Wo pair-stacked: [128, NP*D]; pair p rows = Wo[2p] ++ Wo[2p+1]
        Wo_h = np.asarray(Wo)[hs]          # [4, DK, D]
        wo2 = np.concatenate(
            [np.concatenate([Wo_h[2 * p], Wo_h[2 * p + 1]], axis=0)
             for p in range(NP)], axis=1)  # [128, NP*D]
        in_maps.append({
            "xT": np.ascontiguousarray(x[b].T).astype(np.float16),
            "posT": np.ascontiguousarray(pos[b].T).astype(np.float16),
            "wq": np.ascontiguousarray(
                Wq_f[:, hs, :].reshape(D, NH * DK)).astype(np.float16),
            "wk": np.ascontiguousarray(
                Wk_f[:, hs, :].reshape(D, NH * DK)).astype(np.float16),
            "wv": np.ascontiguousarray(
                Wv_f[:, hs, :].reshape(D, NH * DK)).astype(np.float16),
            "wp": np.ascontiguousarray(
                Wp[:, hs, :].reshape(D, NH * DK)).astype(np.float16),
            "wo": np.ascontiguousarray(wo2).astype(np.float16),
            "qc_bias": np.ascontiguousarray(qcb.reshape(2, 128).T),
            "qp_bias": np.ascontiguousarray(qpb.reshape(2, 128).T),
            "k_bias": np.ascontiguousarray(kb.reshape(2, 128).T),
            "v_bias": np.ascontiguousarray(vb.reshape(NH * DK)),
        })

    return in_maps


def _combine(x, bo, results):
    parts = [r["out_partial"] for r in results]
    out = np.asarray(x, np.float32) + np.asarray(bo, np.float32)[None, None, :]
    for b in range(B):
        out[b] += parts[2 * b].astype(np.float32)
        out[b] += parts[2 * b + 1].astype(np.float32)
    return out.astype(np.float32)


def kernel(x, pos, content_bias, pos_bias, gamma, beta,
           Wq, bq, Wk, bk, Wv, bv, Wp, Wo, bo) -> np.ndarray:
    in_maps = _prepare_in_maps(x, pos, content_bias, pos_bias, gamma, beta,
                               Wq, bq, Wk, bk, Wv, bv, Wp, Wo, bo)
    nc = _get_program()
    res = run_bass_kernel_spmd(nc, in_maps, core_ids=list(range(8)))
    return _combine(x, bo, res.results)


# revision 6
# speedup vs baseline: 153.0879x; 1.0191x over previous
"""Trainium2 Bass kernel v2 for Transformer-XL style MHSA (nn_MHSAModule).

Problem (hardcoded):
  B=4, T=1024, D=512, H=8, DK=64, L=2*T-1=2047, eps=1e-3
  out = x + (MHSA(LayerNorm(x), pos) @ Wo + bo)

Sharding: 8 cores = 4 batches x 2 head-groups (4 heads each). Each core
returns a partial output [T, D] f16 (its heads' contribution); the host
sums the two partials per batch and adds the residual x + bo.

v2 design notes (vs v1):
  - f16 activations/weights on the matmul path; f8e4m3 only for matmul
    B-operands no vector engine reads back: kT, pT, and the rel-shift
    band bounce (halves its DMA volume).
  - exp runs on Act straight from PSUM with accum_out giving the softmax
    denominator for free; E is normalized by one per-partition f16
    tensor_scalar instead of v1's psum-copy + scale chain.
  - PSUM extraction is the scarce resource (~4x an SBUF read): band
    extraction is split across DVE/Act/Pool, scores are extracted by the
    exp itself, ET extracted as f16 (2x cheaper than f32).
  - LayerNorm stats via f16 ones-matmuls; a/b rows reach all partitions
    via gpsimd.partition_broadcast instead of a DRAM round-trip.
  - attention@V stacks head pairs on 128 PSUM partitions; Wo is
    pair-stacked so the output projection contracts 128 rows per step.
  - engines execute in-order, so emission order IS the schedule: x/wq
    load first, the band of head h+1 and the attn@V of the previous pair
    are interleaved into head h's per-q-block score loop.
"""
import numpy as np
from contextlib import ExitStack

import concourse.bass as bass
import concourse.bacc as bacc
import concourse.tile as tile
from concourse import mybir
from concourse import masks
from concourse.bass_utils import run_bass_kernel_spmd

F32 = mybir.dt.float32
F16 = mybir.dt.float16
F8 = mybir.dt.float8e4
AF = mybir.ActivationFunctionType
OP = mybir.AluOpType

B, T, D, H, DK = 4, 1024, 512, 8, 64
L = 2 * T - 1
EPS = 1e-3
NH = 4          # heads per core
NP = 2          # head pairs per core
CH = D // 128   # 4 contraction chunks
QB = T // 128   # 8 q blocks
BAND = 1152    # positional band width per q block
PL = L + 2      # padded pT free size (2 zero pad cols)


def _build_program() -> bass.Bass:
    nc = bacc.Bacc("TRN2", target_bir_lowering=False, debug=False)

    # ---- DRAM I/O ----
    xT = nc.dram_tensor("xT", [D, T], F16, kind="ExternalInput")
    posT = nc.dram_tensor("posT", [D, L], F16, kind="ExternalInput")
    wq = nc.dram_tensor("wq", [D, NH * DK], F16, kind="ExternalInput")
    wk = nc.dram_tensor("wk", [D, NH * DK], F16, kind="ExternalInput")
    wv = nc.dram_tensor("wv", [D, NH * DK], F16, kind="ExternalInput")
    wp = nc.dram_tensor("wp", [D, NH * DK], F16, kind="ExternalInput")
    wo = nc.dram_tensor("wo", [128, NP * D], F16, kind="ExternalInput")
    qc_bias = nc.dram_tensor("qc_bias", [128, NP], F32, kind="ExternalInput")
    qp_bias = nc.dram_tensor("qp_bias", [128, NP], F32, kind="ExternalInput")
    k_bias = nc.dram_tensor("k_bias", [128, NP], F32, kind="ExternalInput")
    v_bias = nc.dram_tensor("v_bias", [NH * DK], F32, kind="ExternalInput")
    out_d = nc.dram_tensor("out_partial", [T, D], F16, kind="ExternalOutput")

    # internal scratch: rel-shift bounce, f8, double buffered
    bounce = nc.dram_tensor("bounce", [2, QB, 128, BAND], F8)

    with tile.TileContext(nc) as tc, ExitStack() as ctx:
        sb = ctx.enter_context(tc.tile_pool(name="sb", bufs=1))
        sb2 = ctx.enter_context(tc.tile_pool(name="sb2", bufs=2))
        sb3 = ctx.enter_context(tc.tile_pool(name="sb3", bufs=3))
        sbE = ctx.enter_context(tc.tile_pool(name="sbE", bufs=6))
        # PSUM: sc 2x[128,512]f32 (2 banks) + band 1x[128,1152]f32
        # (3 banks) + et 2x[128,1024]f16 (2 banks) + o 1x[128,512]f32
        # (1 bank) = 8 banks.
        ps_sc = ctx.enter_context(tc.tile_pool(name="ps_sc", bufs=2,
                                               space="PSUM"))
        # band psum: three independently-released tiles so the next band
        # matmul only waits on the one engine that extracts each slice
        ps_b = ctx.enter_context(tc.tile_pool(name="ps_b", bufs=1,
                                              space="PSUM"))
        ps_et = ctx.enter_context(tc.tile_pool(name="ps_et", bufs=2,
                                               space="PSUM"))
        ps_o = ctx.enter_context(tc.tile_pool(name="ps_o", bufs=1,
                                              space="PSUM"))

        # ---- persistent SBUF ----
        xT_sb = sb.tile([128, CH * T], F16)
        yT_sb = sb.tile([128, CH * T], F16)
        posT_sb = sb.tile([128, CH * L + 2], F16)
        pT_sb = sb.tile([128, NP * PL], F8)
        qcT_sb = sb.tile([128, NP * T], F8)
        qpT_sb = sb.tile([128, NP * T], F8)
        kT_sb = sb.tile([128, NP * T], F8)
        v_sb = sb.tile([128, QB * NH * DK], F16)
        oT_sb = sb.tile([128, NP * T], F16)
        wq_sb = sb.tile([128, CH * 256], F16)
        wk_sb = sb.tile([128, CH * 256], F16)
        wv_sb = sb.tile([128, CH * 256], F16)
        wp_sb = sb.tile([128, CH * 256], F16)
        wo_sb = sb.tile([128, NP * D], F16)
        qcb_sb = sb.tile([128, NP], F32)
        qpb_sb = sb.tile([128, NP], F32)
        kb_sb = sb.tile([128, NP], F32)
        vb_sb = sb.tile([128, 256], F32)
        arep = sb.tile([128, T], F32)
        brep = sb.tile([128, T], F32)
        ident16 = sb.tile([128, 128], F16)
        ident8 = sb.tile([128, 128], F8)
        ones_col = sb.tile([128, 1], F16)
        eps_col = sb.tile([1, 1], F32)

        nc.vector.memset(ones_col[:], 1.0)
        nc.vector.memset(eps_col[:], EPS)

        # ---- loads: x first (stats), then pos+wp (p proj), then q/k ----
        for c in range(CH):
            nc.sync.dma_start(xT_sb[:, c * T:(c + 1) * T],
                              xT[c * 128:(c + 1) * 128, :])
        for c in range(CH):
            nc.sync.dma_start(posT_sb[:, c * L:(c + 1) * L],
                              posT[c * 128:(c + 1) * 128, :])
            nc.sync.dma_start(wp_sb[:, c * 256:(c + 1) * 256],
                              wp[c * 128:(c + 1) * 128, :])
        for c in range(CH):
            nc.sync.dma_start(wq_sb[:, c * 256:(c + 1) * 256],
                              wq[c * 128:(c + 1) * 128, :])
            nc.sync.dma_start(wk_sb[:, c * 256:(c + 1) * 256],
                              wk[c * 128:(c + 1) * 128, :])
        nc.sync.dma_start(qcb_sb[:], qc_bias[:])
        nc.sync.dma_start(qpb_sb[:], qp_bias[:])
        nc.sync.dma_start(kb_sb[:], k_bias[:])
        for c in range(CH):
            nc.sync.dma_start(wv_sb[:, c * 256:(c + 1) * 256],
                              wv[c * 128:(c + 1) * 128, :])
        nc.sync.dma_start(wo_sb[:], wo[:])
        nc.sync.dma_start(
            vb_sb[:], bass.AP(v_bias[:].tensor, 0, [[0, 128], [1, 256]]))

        # ---- LayerNorm stats; short [1,512]-row chain (row ops are
        # lane-serial, so every op counts): a = rsqrt(var+eps),
        # b = -mu*a, computed as
        #   t1 = s1*s1; v' = D*s2 - t1; a = Rsqrt(v'/D^2 + eps);
        #   b = (s1 * -1/D) * a
        a_row = sb.tile([1, T], F32)
        b_row = sb.tile([1, T], F32)
        for tt in range(2):
            s1 = ps_b.tile([1, 512], F32, tag="band0")
            for c in range(CH):
                xt = xT_sb[:, c * T + tt * 512: c * T + tt * 512 + 512]
                nc.tensor.matmul(s1[:], ones_col[:], xt,
                                 start=(c == 0), stop=(c == CH - 1))
            s2 = ps_b.tile([1, 512], F32, tag="band1")
            for c in range(CH):
                xsq = sb3.tile([128, 512], F16, tag="xsq", name="xsq")
                xt = xT_sb[:, c * T + tt * 512: c * T + tt * 512 + 512]
                nc.vector.tensor_tensor(xsq[:], xt, xt, op=OP.mult)
                nc.tensor.matmul(s2[:], ones_col[:], xsq[:],
                                 start=(c == 0), stop=(c == CH - 1))
            t1 = sb2.tile([1, 512], F32, tag="t1", name="t1")
            nc.scalar.activation(t1[:], s1[:], AF.Square)
            vv = sb2.tile([1, 512], F32, tag="vv", name="vv")
            nc.vector.scalar_tensor_tensor(vv[:], s2[:], float(D), t1[:],
                                           op0=OP.mult, op1=OP.subtract)
            stdh = sb2.tile([1, 512], F32, tag="stdh", name="stdh")
            nc.scalar.activation(stdh[:], vv[:], AF.Sqrt,
                                 scale=1.0 / (D * D), bias=eps_col[:])
            a_half = a_row[:, tt * 512:(tt + 1) * 512]
            nc.vector.reciprocal(a_half, stdh[:])
            nc.vector.scalar_tensor_tensor(
                b_row[:, tt * 512:(tt + 1) * 512], s1[:], -1.0 / D, a_half,
                op0=OP.mult, op1=OP.mult)
        nc.gpsimd.partition_broadcast(arep[:], a_row[:])
        nc.gpsimd.partition_broadcast(brep[:], b_row[:])
        masks.make_identity(nc, ident16[:])
        masks.make_identity(nc, ident8[:])

        # ---- LayerNorm apply: yT = xT * a + b (f16), 512-col pieces so
        # the first projections start after the nt=0 halves ----
        for nt in range(2):
            for c in range(CH):
                eng = nc.gpsimd if c == 1 else nc.vector
                xs = xT_sb[:, c * T + nt * 512: c * T + nt * 512 + 512]
                ys = yT_sb[:, c * T + nt * 512: c * T + nt * 512 + 512]
                ar = arep[:, nt * 512:(nt + 1) * 512]
                br = brep[:, nt * 512:(nt + 1) * 512]
                eng.tensor_tensor(ys, xs, ar, op=OP.mult)
                eng.tensor_tensor(ys, ys, br, op=OP.add)

        # pad pT columns
        z8 = sb.tile([128, 4], F8)
        nc.vector.memset(z8[:], 0.0)
        zrow = sb.tile([128, 2], F16)
        nc.vector.memset(zrow[:], 0.0)
        nc.vector.tensor_copy(posT_sb[:, CH * L:], zrow[:])

        # ---- projection emitters ----
        # extraction engines are spread (qc->Act, qp->DVE, k/p->Pool,
        # v->DVE) so consecutive users of the psum ping-pong release in
        # parallel queues.
        def q_proj(p, nt, pool=None, ptag="sc"):
            prj = (pool or ps_sc).tile([128, 512], F32, tag=ptag,
                                       name="prj")
            for ci, c in enumerate((0, 2, 1, 3)):
                nc.tensor.matmul(
                    prj[:],
                    wq_sb[:, c * 256 + p * 128: c * 256 + p * 128 + 128],
                    yT_sb[:, c * T + nt * 512: c * T + nt * 512 + 512],
                    start=(c == 0), stop=(c == CH - 1))
            o = p * T + nt * 512
            nc.scalar.activation(qcT_sb[:, o:o + 512], prj[:],
                                 AF.Identity, bias=qcb_sb[:, p:p + 1])
            nc.vector.tensor_scalar_add(qpT_sb[:, o:o + 512], prj[:],
                                        qpb_sb[:, p:p + 1])

        def k_proj(p, nt, pool=None, ptag="sc"):
            prjk = (pool or ps_sc).tile([128, 512], F32, tag=ptag,
                                        name="prjk")
            for ci, c in enumerate((0, 2, 1, 3)):
                nc.tensor.matmul(
                    prjk[:],
                    wk_sb[:, c * 256 + p * 128: c * 256 + p * 128 + 128],
                    yT_sb[:, c * T + nt * 512: c * T + nt * 512 + 512],
                    start=(ci == 0), stop=(ci == CH - 1))
            nc.vector.tensor_scalar_add(kT_sb[:, p * T + nt * 512:
                                              p * T + nt * 512 + 512],
                                        prjk[:], kb_sb[:, p:p + 1])

        def p_proj(p, nt, pool=None, ptag="sc"):
            pps = (pool or ps_sc).tile([128, 512], F32, tag=ptag,
                                       name="pps")
            for ci, c in enumerate((0, 2, 1, 3)):
                nc.tensor.matmul(
                    pps[:],
                    wp_sb[:, c * 256 + p * 128: c * 256 + p * 128 + 128],
                    posT_sb[:, c * L + nt * 512: c * L + nt * 512 + 512],
                    start=(ci == 0), stop=(ci == CH - 1))
            nc.scalar.copy(
                pT_sb[:, p * PL + nt * 512: p * PL + nt * 512 + 512],
                pps[:])
            if nt == 3:
                nc.gpsimd.tensor_copy(pT_sb[:, p * PL + L: (p + 1) * PL],
                                      z8[:, :PL - L])

        def v_proj(t8, pool=None, ptag="sc"):
            vps = (pool or ps_sc).tile([128, 256], F32, tag=ptag,
                                       name="vps")
            for c in range(CH):
                nc.tensor.matmul(
                    vps[:],
                    yT_sb[:, c * T + t8 * 128: c * T + t8 * 128 + 128],
                    wv_sb[:, c * 256:(c + 1) * 256],
                    start=(c == 0), stop=(c == CH - 1))
            nc.vector.tensor_tensor(
                v_sb[:, t8 * 256:(t8 + 1) * 256], vps[:], vb_sb[:],
                op=OP.add)

        # ---- attention emitters ----
        def hsl(tile_, h, lo, w, stride=T):
            p, off = h // 2, (h % 2) * 64
            return tile_[off:off + 64, p * stride + lo: p * stride + lo + w]

        def band_qb(h, qb, shifted):
            """Band scores for (h, qb), bounce out, and the skewed read of
            this qb's block straight back into `shifted`. Three psum tiles,
            one extraction engine each, so the next band matmul waits on
            exactly one engine per tile."""
            ping = h % 2
            s0 = 897 - qb * 128
            qp_blk = hsl(qpT_sb, h, qb * 128, 128)
            bps0 = ps_b.tile([128, 768], F32, tag="band0")
            nc.tensor.matmul(bps0[:, :512], qp_blk,
                             hsl(pT_sb, h, s0, 512, stride=PL),
                             start=True, stop=True)
            nc.tensor.matmul(bps0[:, 512:], qp_blk,
                             hsl(pT_sb, h, s0 + 512, 256, stride=PL),
                             start=True, stop=True)
            bps1 = ps_b.tile([128, 384], F32, tag="band1")
            nc.tensor.matmul(bps1[:], qp_blk,
                             hsl(pT_sb, h, s0 + 768, 384, stride=PL),
                             start=True, stop=True)
            b8 = sb3.tile([128, BAND], F8, tag="band8", name="b8")
            nc.vector.tensor_copy(b8[:, :768], bps0[:])
            nc.scalar.copy(b8[:, 768:], bps1[:])
            nc.sync.dma_start(bounce[ping, qb], b8[:])
            if qb % 2 == 1:
                src = bass.AP(bounce[:].tensor,
                              (ping * QB + qb - 1) * (128 * BAND) + 127,
                              [[BAND - 1, 128], [128 * BAND, 2], [1, T]])
                nc.sync.dma_start(
                    shifted[:, (qb - 1) * T:(qb + 1) * T], src)

        def shifted_tile():
            shifted = sb2.tile([128, QB * T], F8, tag="shifted",
                               name="shifted")
            return shifted

        def scores_qb(h, qb, shifted):
            """Scores+exp+normalize for (h, qb); returns the E tile.
            Transposes are emitted one qb later (transpose_qb) so the PE
            queue never waits on the Act/DVE chain of the same qb."""
            E_sb = sbE.tile([128, T], F16, tag="E", name="E")
            den = sb3.tile([128, 2], F32, tag="den", name="den")
            for half in range(2):
                sps = ps_sc.tile([128, 512], F32, tag="sc")
                nc.tensor.matmul(
                    sps[:],
                    hsl(qcT_sb, h, qb * 128, 128),
                    hsl(kT_sb, h, half * 512, 512),
                    start=True, stop=False)
                if qb == 0 and half == 1:
                    # scores[0, 1023] += (q+pos_bias)[1] . p[0]
                    # (the reference rel_shift reshape wraps this element)
                    nc.tensor.matmul(
                        sps[0:1, 511:512],
                        hsl(qpT_sb, h, 1, 1),
                        hsl(pT_sb, h, 0, 1, stride=PL),
                        start=False, stop=False)
                nc.tensor.matmul(
                    sps[:],
                    ident8[:],
                    shifted[:, qb * T + half * 512:
                            qb * T + half * 512 + 512],
                    start=False, stop=True)
                nc.scalar.activation(
                    E_sb[:, half * 512:(half + 1) * 512], sps[:], AF.Exp,
                    scale=0.125, accum_out=den[:, half:half + 1])
            rec = sb3.tile([128, 1], F32, tag="rec", name="rec")
            nc.vector.tensor_tensor(rec[:], den[:, 0:1], den[:, 1:2],
                                    op=OP.add)
            nc.vector.reciprocal(rec[:], rec[:])
            nc.vector.tensor_scalar_mul(E_sb[:], E_sb[:], rec[:])
            return E_sb

        def transpose_qb(qb, E_sb, ET_sb):
            # transpose E (f16): 8 PE transposes -> one f16 psum bank
            etps = ps_et.tile([128, T], F16, tag="et")
            for kc in range(QB):
                nc.tensor.transpose(
                    etps[:, kc * 128: kc * 128 + 128],
                    E_sb[:, kc * 128: kc * 128 + 128],
                    ident16[:])
            # scatter-extract: ET_sb[:, kc*T + qb*128 ...] = etps block kc
            dst = bass.AP(ET_sb[:].tensor, qb * 128,
                          [[QB * T, 128], [T, QB], [1, 128]])
            nc.vector.tensor_copy(dst, etps[:])

        def attnv_chunk(p, ib, hh, kh, ET_pair, otps):
            hloc = 2 * p + hh
            for kc in range(4 * kh, 4 * kh + 4):
                nc.tensor.matmul(
                    otps[hh * 64:hh * 64 + 64, :],
                    v_sb[:, kc * 256 + hloc * 64: kc * 256 + hloc * 64 + 64],
                    ET_pair[hh][:, kc * T + ib * 512: kc * T + ib * 512 + 512],
                    start=(kc == 0), stop=(kc == QB - 1))

        def attnv_extract(p, ib, otps):
            dst = oT_sb[:, p * T + ib * 512: p * T + ib * 512 + 512]
            if (p + ib) % 2 == 0:
                nc.vector.tensor_copy(dst, otps[:])
            else:
                nc.scalar.copy(dst, otps[:])

        def out_proj(t8):
            ops_ = ps_sc.tile([128, 512], F32, tag="sc")
            for p in range(NP):
                nc.tensor.matmul(
                    ops_[:],
                    oT_sb[:, p * T + t8 * 128: p * T + t8 * 128 + 128],
                    wo_sb[:, p * D:(p + 1) * D],
                    start=(p == 0), stop=(p == NP - 1))
            osb = sb2.tile([128, 512], F16, tag="osb", name="osb")
            if t8 % 2 == 0:
                nc.vector.tensor_copy(osb[:], ops_[:])
            else:
                nc.scalar.copy(osb[:], ops_[:])
            nc.sync.dma_start(out_d[t8 * 128:(t8 + 1) * 128, :], osb[:])

        # ---- emission schedule ----
        # p proj first (needs no LayerNorm -> fills the stats-chain
        # latency). Only pair-0 projections + head-0 bands run before the
        # attention loop; pair-1 projections and v-proj are deferred as
        # per-q-block filler inside the head-0/1 loops so the in-order PE
        # queue reaches head-0 scores ~12us earlier.
        for nt in range(4):
            p_proj(0, nt)
        q_proj(0, 0)
        k_proj(0, 0)
        q_proj(0, 1)
        k_proj(0, 1)
        shifted = {0: shifted_tile()}
        for qb in range(QB):
            band_qb(0, qb, shifted[0])

        filler = {
            0: [lambda: q_proj(1, 0, ps_o, "o"),
                lambda: q_proj(1, 1, ps_o, "o"),
                lambda: k_proj(1, 0, ps_o, "o"),
                lambda: k_proj(1, 1, ps_o, "o"),
                lambda: p_proj(1, 0, ps_o, "o"),
                lambda: p_proj(1, 1, ps_o, "o"),
                lambda: p_proj(1, 2, ps_o, "o"),
                lambda: p_proj(1, 3, ps_o, "o")],
            1: [(lambda t8=t8: v_proj(t8, ps_o, "o")) for t8 in range(QB)],
        }

        ET_tiles = {}
        attnv_work = []   # deferred attn@V chunks for the previous pair
        for h in range(NH):
            # bufs=3: head h+2 must not wait on the deferred attn@V reads
            # of head h's ET (they interleave into head h+2's score loop)
            ET_sb = sb3.tile([128, QB * T], F16, tag="ET", name="ET")
            ET_tiles[h] = ET_sb
            if h + 1 < NH:
                shifted[h + 1] = shifted_tile()
            E_hist = []
            for qb in range(QB):
                if h + 1 < NH:
                    band_qb(h + 1, qb, shifted[h + 1])
                E_hist.append(scores_qb(h, qb, shifted[h]))
                if qb >= 2:
                    transpose_qb(qb - 2, E_hist[qb - 2], ET_sb)
                for fn_ in filler.get(h, [])[qb:qb + 1]:
                    fn_()
                # slot one deferred attn@V chunk of the previous pair
                if attnv_work:
                    attnv_work.pop(0)()
            transpose_qb(QB - 2, E_hist[QB - 2], ET_sb)
            transpose_qb(QB - 1, E_hist[QB - 1], ET_sb)
            if h % 2 == 1:
                p = h // 2
                ET_pair = (ET_tiles[2 * p], ET_tiles[2 * p + 1])
                chunks = []
                state = {}

                def make_chunk(p_, ib_, hh_, kh_, ET_pair_):
                    def go():
                        if ("ot", p_, ib_) not in state:
                            state[("ot", p_, ib_)] = ps_o.tile(
                                [128, 512], F32, tag="o", name="otps")
                        otps = state[("ot", p_, ib_)]
                        attnv_chunk(p_, ib_, hh_, kh_, ET_pair_, otps)
                        if hh_ == 1 and kh_ == 1:
                            attnv_extract(p_, ib_, otps)
                    return go

                for ib in range(2):
                    for hh in range(2):
                        for kh in range(2):
                            chunks.append(make_chunk(p, ib, hh, kh, ET_pair))
                if h == NH - 1:
                    for cfn in chunks[:4]:
                        cfn()
                    for i, cfn in enumerate(chunks[4:]):
                        out_proj(i)
                        cfn()
                    for t8 in range(4, QB):
                        out_proj(t8)
                else:
                    attnv_work.extend(chunks)

    nc.compile()
    return nc


_PROGRAM_CACHE: dict = {}


def _get_program() -> bass.Bass:
    if "nc" not in _PROGRAM_CACHE:
        _PROGRAM_CACHE["nc"] = _build_program()
    return _PROGRAM_CACHE["nc"]


def _prepare_in_maps(x, pos, content_bias, pos_bias, gamma, beta,
                     Wq, bq, Wk, bk, Wv, bv, Wp, Wo, bo):
    x = np.asarray(x, np.float32)
    pos = np.asarray(pos, np.float32)
    gamma = np.asarray(gamma, np.float32)
    beta = np.asarray(beta, np.float32)

    # gamma folding: y = yln*gamma + beta  =>  y@W = yln@(gamma*W) + beta@W
    def fold(W):
        W = np.asarray(W, np.float32)
        return W * gamma[:, None, None], np.einsum("d,dhk->hk", beta, W)

    Wq_f, bq_f = fold(Wq)
    Wk_f, bk_f = fold(Wk)
    Wv_f, bv_f = fold(Wv)
    Wp = np.asarray(Wp, np.float32)
    Wo = np.asarray(Wo, np.float32)

    in_maps = []
    for core in range(8):
        b = core // 2
        g = core % 2
        hs = slice(4 * g, 4 * g + 4)
        qcb = (np.asarray(bq) + np.asarray(content_bias) + bq_f)[hs]
        qpb = (np.asarray(bq) + np.asarray(pos_bias) + bq_f)[hs]
        kb = (np.asarray(bk) + bk_f)[hs]
        vb = (np.asarray(bv) + bv_f)[hs]
        # Wo pair-stacked: [128, NP*D]; pair p rows = Wo[2p] ++ Wo[2p+1]
        Wo_h = np.asarray(Wo)[hs]          # [4, DK, D]
        wo2 = np.concatenate(
            [np.concatenate([Wo_h[2 * p], Wo_h[2 * p + 1]], axis=0)
             for p in range(NP)], axis=1)  # [128, NP*D]
        in_maps.append({
            "xT": np.ascontiguousarray(x[b].T).astype(np.float16),
            "posT": np.ascontiguousarray(pos[b].T).astype(np.float16),
            "wq": np.ascontiguousarray(
                Wq_f[:, hs, :].reshape(D, NH * DK)).astype(np.float16),
            "wk": np.ascontiguousarray(
                Wk_f[:, hs, :].reshape(D, NH * DK)).astype(np.float16),
            "wv": np.ascontiguousarray(
                Wv_f[:, hs, :].reshape(D, NH * DK)).astype(np.float16),
            "wp": np.ascontiguousarray(
                Wp[:, hs, :].reshape(D, NH * DK)).astype(np.float16),
            "wo": np.ascontiguousarray(wo2).astype(np.float16),
            "qc_bias": np.ascontiguousarray(qcb.reshape(2, 128).T),
            "qp_bias": np.ascontiguousarray(qpb.reshape(2, 128).T),
            "k_bias": np.ascontiguousarray(kb.reshape(2, 128).T),
            "v_bias": np.ascontiguousarray(vb.reshape(NH * DK)),
        })

    return in_maps


def _combine(x, bo, results):
    parts = [r["out_partial"] for r in results]
    out = np.asarray(x, np.float32) + np.asarray(bo, np.float32)[None, None, :]
    for b in range(B):
        out[b] += parts[2 * b].astype(np.float32)
        out[b] += parts[2 * b + 1].astype(np.float32)
    return out.astype(np.float32)


def kernel(x, pos, content_bias, pos_bias, gamma, beta,
           Wq, bq, Wk, bk, Wv, bv, Wp, Wo, bo) -> np.ndarray:
    in_maps = _prepare_in_maps(x, pos, content_bias, pos_bias, gamma, beta,
                               Wq, bq, Wk, bk, Wv, bv, Wp, Wo, bo)
    nc = _get_program()
    res = run_bass_kernel_spmd(nc, in_maps, core_ids=list(range(8)))
    return _combine(x, bo, res.results)


# revision 7
# speedup vs baseline: 154.5006x; 1.0092x over previous
"""Trainium2 Bass kernel v2 for Transformer-XL style MHSA (nn_MHSAModule).

Problem (hardcoded):
  B=4, T=1024, D=512, H=8, DK=64, L=2*T-1=2047, eps=1e-3
  out = x + (MHSA(LayerNorm(x), pos) @ Wo + bo)

Sharding: 8 cores = 4 batches x 2 head-groups (4 heads each). Each core
returns a partial output [T, D] f16 (its heads' contribution); the host
sums the two partials per batch and adds the residual x + bo.

v2 design notes (vs v1):
  - f16 activations/weights on the matmul path; f8e4m3 only for matmul
    B-operands no vector engine reads back: kT, pT, and the rel-shift
    band bounce (halves its DMA volume).
  - exp runs on Act straight from PSUM with accum_out giving the softmax
    denominator for free; E is normalized by one per-partition f16
    tensor_scalar instead of v1's psum-copy + scale chain.
  - PSUM extraction is the scarce resource (~4x an SBUF read): band
    extraction is split across DVE/Act/Pool, scores are extracted by the
    exp itself, ET extracted as f16 (2x cheaper than f32).
  - LayerNorm stats via f16 ones-matmuls; a/b rows reach all partitions
    via gpsimd.partition_broadcast instead of a DRAM round-trip.
  - attention@V stacks head pairs on 128 PSUM partitions; Wo is
    pair-stacked so the output projection contracts 128 rows per step.
  - engines execute in-order, so emission order IS the schedule: x/wq
    load first, the band of head h+1 and the attn@V of the previous pair
    are interleaved into head h's per-q-block score loop.
"""
import numpy as np
from contextlib import ExitStack

import concourse.bass as bass
import concourse.bacc as bacc
import concourse.tile as tile
from concourse import mybir
from concourse import masks
from concourse.bass_utils import run_bass_kernel_spmd

F32 = mybir.dt.float32
F16 = mybir.dt.float16
F8 = mybir.dt.float8e4
AF = mybir.ActivationFunctionType
OP = mybir.AluOpType

B, T, D, H, DK = 4, 1024, 512, 8, 64
L = 2 * T - 1
EPS = 1e-3
NH = 4          # heads per core
NP = 2          # head pairs per core
CH = D // 128   # 4 contraction chunks
QB = T // 128   # 8 q blocks
BAND = 1152    # positional band width per q block
PL = L + 2      # padded pT free size (2 zero pad cols)


def _build_program() -> bass.Bass:
    nc = bacc.Bacc("TRN2", target_bir_lowering=False, debug=False)

    # ---- DRAM I/O ----
    xT = nc.dram_tensor("xT", [D, T], F16, kind="ExternalInput")
    posT = nc.dram_tensor("posT", [D, L], F16, kind="ExternalInput")
    wq = nc.dram_tensor("wq", [D, NH * DK], F16, kind="ExternalInput")
    wk = nc.dram_tensor("wk", [D, NH * DK], F16, kind="ExternalInput")
    wv = nc.dram_tensor("wv", [D, NH * DK], F16, kind="ExternalInput")
    wp = nc.dram_tensor("wp", [D, NH * DK], F16, kind="ExternalInput")
    wo = nc.dram_tensor("wo", [128, NP * D], F16, kind="ExternalInput")
    qc_bias = nc.dram_tensor("qc_bias", [128, NP], F32, kind="ExternalInput")
    qp_bias = nc.dram_tensor("qp_bias", [128, NP], F32, kind="ExternalInput")
    k_bias = nc.dram_tensor("k_bias", [128, NP], F32, kind="ExternalInput")
    v_bias = nc.dram_tensor("v_bias", [NH * DK], F32, kind="ExternalInput")
    out_d = nc.dram_tensor("out_partial", [T, D], F16, kind="ExternalOutput")

    # internal scratch: rel-shift bounce, f8, double buffered
    bounce = nc.dram_tensor("bounce", [2, QB, 128, BAND], F8)

    with tile.TileContext(nc) as tc, ExitStack() as ctx:
        sb = ctx.enter_context(tc.tile_pool(name="sb", bufs=1))
        sb2 = ctx.enter_context(tc.tile_pool(name="sb2", bufs=2))
        sb3 = ctx.enter_context(tc.tile_pool(name="sb3", bufs=3))
        sbE = ctx.enter_context(tc.tile_pool(name="sbE", bufs=6))
        # PSUM: sc 2x[128,512]f32 (2 banks) + band 1x[128,1152]f32
        # (3 banks) + et 2x[128,1024]f16 (2 banks) + o 1x[128,512]f32
        # (1 bank) = 8 banks.
        ps_sc = ctx.enter_context(tc.tile_pool(name="ps_sc", bufs=2,
                                               space="PSUM"))
        # band psum: three independently-released tiles so the next band
        # matmul only waits on the one engine that extracts each slice
        ps_b = ctx.enter_context(tc.tile_pool(name="ps_b", bufs=1,
                                              space="PSUM"))
        ps_et = ctx.enter_context(tc.tile_pool(name="ps_et", bufs=2,
                                               space="PSUM"))
        ps_o = ctx.enter_context(tc.tile_pool(name="ps_o", bufs=1,
                                              space="PSUM"))

        # ---- persistent SBUF ----
        xT_sb = sb.tile([128, CH * T], F16)
        yT_sb = sb.tile([128, CH * T], F16)
        posT_sb = sb.tile([128, CH * L + 2], F16)
        pT_sb = sb.tile([128, NP * PL], F8)
        qcT_sb = sb.tile([128, NP * T], F8)
        qpT_sb = sb.tile([128, NP * T], F8)
        kT_sb = sb.tile([128, NP * T], F8)
        v_sb = sb.tile([128, QB * NH * DK], F16)
        oT_sb = sb.tile([128, NP * T], F16)
        wq_sb = sb.tile([128, CH * 256], F16)
        wk_sb = sb.tile([128, CH * 256], F16)
        wv_sb = sb.tile([128, CH * 256], F16)
        wp_sb = sb.tile([128, CH * 256], F16)
        wo_sb = sb.tile([128, NP * D], F16)
        qcb_sb = sb.tile([128, NP], F32)
        qpb_sb = sb.tile([128, NP], F32)
        kb_sb = sb.tile([128, NP], F32)
        vb_sb = sb.tile([128, 256], F32)
        arep = sb.tile([128, T], F32)
        brep = sb.tile([128, T], F32)
        ident16 = sb.tile([128, 128], F16)
        ident8 = sb.tile([128, 128], F8)
        ones_col = sb.tile([128, 1], F16)
        eps_col = sb.tile([1, 1], F32)

        nc.vector.memset(ones_col[:], 1.0)
        nc.vector.memset(eps_col[:], EPS)

        # ---- loads: x first (stats), then pos+wp (p proj), then q/k ----
        for c in range(CH):
            nc.sync.dma_start(xT_sb[:, c * T:(c + 1) * T],
                              xT[c * 128:(c + 1) * 128, :])
        for c in range(CH):
            nc.sync.dma_start(posT_sb[:, c * L:(c + 1) * L],
                              posT[c * 128:(c + 1) * 128, :])
            nc.sync.dma_start(wp_sb[:, c * 256:(c + 1) * 256],
                              wp[c * 128:(c + 1) * 128, :])
        for c in range(CH):
            nc.sync.dma_start(wq_sb[:, c * 256:(c + 1) * 256],
                              wq[c * 128:(c + 1) * 128, :])
            nc.sync.dma_start(wk_sb[:, c * 256:(c + 1) * 256],
                              wk[c * 128:(c + 1) * 128, :])
        nc.sync.dma_start(qcb_sb[:], qc_bias[:])
        nc.sync.dma_start(qpb_sb[:], qp_bias[:])
        nc.sync.dma_start(kb_sb[:], k_bias[:])
        for c in range(CH):
            nc.sync.dma_start(wv_sb[:, c * 256:(c + 1) * 256],
                              wv[c * 128:(c + 1) * 128, :])
        nc.sync.dma_start(wo_sb[:], wo[:])
        nc.sync.dma_start(
            vb_sb[:], bass.AP(v_bias[:].tensor, 0, [[0, 128], [1, 256]]))

        # ---- LayerNorm stats; short [1,512]-row chain (row ops are
        # lane-serial, so every op counts): a = rsqrt(var+eps),
        # b = -mu*a, computed as
        #   t1 = s1*s1; v' = D*s2 - t1; a = Rsqrt(v'/D^2 + eps);
        #   b = (s1 * -1/D) * a
        a_row = sb.tile([1, T], F32)
        b_row = sb.tile([1, T], F32)
        for tt in range(2):
            s1 = ps_b.tile([1, 512], F32, tag="band0")
            for c in range(CH):
                xt = xT_sb[:, c * T + tt * 512: c * T + tt * 512 + 512]
                nc.tensor.matmul(s1[:], ones_col[:], xt,
                                 start=(c == 0), stop=(c == CH - 1))
            s2 = ps_b.tile([1, 512], F32, tag="band1")
            for c in range(CH):
                xsq = sb3.tile([128, 512], F16, tag="xsq", name="xsq")
                xt = xT_sb[:, c * T + tt * 512: c * T + tt * 512 + 512]
                nc.vector.tensor_tensor(xsq[:], xt, xt, op=OP.mult)
                nc.tensor.matmul(s2[:], ones_col[:], xsq[:],
                                 start=(c == 0), stop=(c == CH - 1))
            t1 = sb2.tile([1, 512], F32, tag="t1", name="t1")
            nc.scalar.activation(t1[:], s1[:], AF.Square)
            vv = sb2.tile([1, 512], F32, tag="vv", name="vv")
            nc.vector.scalar_tensor_tensor(vv[:], s2[:], float(D), t1[:],
                                           op0=OP.mult, op1=OP.subtract)
            stdh = sb2.tile([1, 512], F32, tag="stdh", name="stdh")
            nc.scalar.activation(stdh[:], vv[:], AF.Sqrt,
                                 scale=1.0 / (D * D), bias=eps_col[:])
            a_half = a_row[:, tt * 512:(tt + 1) * 512]
            nc.vector.reciprocal(a_half, stdh[:])
            nc.vector.scalar_tensor_tensor(
                b_row[:, tt * 512:(tt + 1) * 512], s1[:], -1.0 / D, a_half,
                op0=OP.mult, op1=OP.mult)
        nc.gpsimd.partition_broadcast(arep[:], a_row[:])
        nc.gpsimd.partition_broadcast(brep[:], b_row[:])
        masks.make_identity(nc, ident16[:])
        masks.make_identity(nc, ident8[:])

        # ---- LayerNorm apply: yT = xT * a + b (f16), 512-col pieces so
        # the first projections start after the nt=0 halves ----
        for nt in range(2):
            for c in range(CH):
                eng = nc.gpsimd if c == 1 else nc.vector
                xs = xT_sb[:, c * T + nt * 512: c * T + nt * 512 + 512]
                ys = yT_sb[:, c * T + nt * 512: c * T + nt * 512 + 512]
                ar = arep[:, nt * 512:(nt + 1) * 512]
                br = brep[:, nt * 512:(nt + 1) * 512]
                eng.tensor_tensor(ys, xs, ar, op=OP.mult)
                eng.tensor_tensor(ys, ys, br, op=OP.add)

        # pad pT columns
        z8 = sb.tile([128, 4], F8)
        nc.vector.memset(z8[:], 0.0)
        zrow = sb.tile([128, 2], F16)
        nc.vector.memset(zrow[:], 0.0)
        nc.vector.tensor_copy(posT_sb[:, CH * L:], zrow[:])

        # ---- projection emitters ----
        # extraction engines are spread (qc->Act, qp->DVE, k/p->Pool,
        # v->DVE) so consecutive users of the psum ping-pong release in
        # parallel queues.
        def q_proj(p, nt, pool=None, ptag="sc"):
            prj = (pool or ps_sc).tile([128, 512], F32, tag=ptag,
                                       name="prj")
            for ci, c in enumerate((0, 2, 1, 3)):
                nc.tensor.matmul(
                    prj[:],
                    wq_sb[:, c * 256 + p * 128: c * 256 + p * 128 + 128],
                    yT_sb[:, c * T + nt * 512: c * T + nt * 512 + 512],
                    start=(c == 0), stop=(c == CH - 1))
            o = p * T + nt * 512
            nc.scalar.activation(qcT_sb[:, o:o + 512], prj[:],
                                 AF.Identity, bias=qcb_sb[:, p:p + 1])
            nc.vector.tensor_scalar_add(qpT_sb[:, o:o + 512], prj[:],
                                        qpb_sb[:, p:p + 1])

        def k_proj(p, nt, pool=None, ptag="sc"):
            prjk = (pool or ps_sc).tile([128, 512], F32, tag=ptag,
                                        name="prjk")
            for ci, c in enumerate((0, 2, 1, 3)):
                nc.tensor.matmul(
                    prjk[:],
                    wk_sb[:, c * 256 + p * 128: c * 256 + p * 128 + 128],
                    yT_sb[:, c * T + nt * 512: c * T + nt * 512 + 512],
                    start=(ci == 0), stop=(ci == CH - 1))
            nc.scalar.activation(kT_sb[:, p * T + nt * 512:
                                       p * T + nt * 512 + 512],
                                 prjk[:], AF.Identity,
                                 bias=kb_sb[:, p:p + 1])

        def p_proj(p, nt, pool=None, ptag="sc"):
            pps = (pool or ps_sc).tile([128, 512], F32, tag=ptag,
                                       name="pps")
            for ci, c in enumerate((0, 2, 1, 3)):
                nc.tensor.matmul(
                    pps[:],
                    wp_sb[:, c * 256 + p * 128: c * 256 + p * 128 + 128],
                    posT_sb[:, c * L + nt * 512: c * L + nt * 512 + 512],
                    start=(ci == 0), stop=(ci == CH - 1))
            nc.scalar.copy(
                pT_sb[:, p * PL + nt * 512: p * PL + nt * 512 + 512],
                pps[:])
            if nt == 3:
                nc.gpsimd.tensor_copy(pT_sb[:, p * PL + L: (p + 1) * PL],
                                      z8[:, :PL - L])

        def v_proj(t8, pool=None, ptag="sc"):
            vps = (pool or ps_sc).tile([128, 256], F32, tag=ptag,
                                       name="vps")
            for c in range(CH):
                nc.tensor.matmul(
                    vps[:],
                    yT_sb[:, c * T + t8 * 128: c * T + t8 * 128 + 128],
                    wv_sb[:, c * 256:(c + 1) * 256],
                    start=(c == 0), stop=(c == CH - 1))
            nc.vector.tensor_tensor(
                v_sb[:, t8 * 256:(t8 + 1) * 256], vps[:], vb_sb[:],
                op=OP.add)

        # ---- attention emitters ----
        def hsl(tile_, h, lo, w, stride=T):
            p, off = h // 2, (h % 2) * 64
            return tile_[off:off + 64, p * stride + lo: p * stride + lo + w]

        def band_qb(h, qb, shifted):
            """Band scores for (h, qb), bounce out, and the skewed read of
            this qb's block straight back into `shifted`. Three psum tiles,
            one extraction engine each, so the next band matmul waits on
            exactly one engine per tile."""
            ping = h % 2
            s0 = 897 - qb * 128
            qp_blk = hsl(qpT_sb, h, qb * 128, 128)
            bps0 = ps_b.tile([128, 768], F32, tag="band0")
            nc.tensor.matmul(bps0[:, :512], qp_blk,
                             hsl(pT_sb, h, s0, 512, stride=PL),
                             start=True, stop=True)
            nc.tensor.matmul(bps0[:, 512:], qp_blk,
                             hsl(pT_sb, h, s0 + 512, 256, stride=PL),
                             start=True, stop=True)
            bps1 = ps_b.tile([128, 384], F32, tag="band1")
            nc.tensor.matmul(bps1[:], qp_blk,
                             hsl(pT_sb, h, s0 + 768, 384, stride=PL),
                             start=True, stop=True)
            b8 = sb3.tile([128, BAND], F8, tag="band8", name="b8")
            nc.vector.tensor_copy(b8[:, :768], bps0[:])
            nc.scalar.copy(b8[:, 768:], bps1[:])
            nc.sync.dma_start(bounce[ping, qb], b8[:])
            if qb % 2 == 1:
                src = bass.AP(bounce[:].tensor,
                              (ping * QB + qb - 1) * (128 * BAND) + 127,
                              [[BAND - 1, 128], [128 * BAND, 2], [1, T]])
                nc.sync.dma_start(
                    shifted[:, (qb - 1) * T:(qb + 1) * T], src)

        def shifted_tile():
            shifted = sb2.tile([128, QB * T], F8, tag="shifted",
                               name="shifted")
            return shifted

        def scores_qb(h, qb, shifted):
            """Scores+exp+normalize for (h, qb); returns the E tile.
            Transposes are emitted one qb later (transpose_qb) so the PE
            queue never waits on the Act/DVE chain of the same qb."""
            E_sb = sbE.tile([128, T], F16, tag="E", name="E")
            den = sb3.tile([128, 2], F32, tag="den", name="den")
            for half in range(2):
                sps = ps_sc.tile([128, 512], F32, tag="sc")
                nc.tensor.matmul(
                    sps[:],
                    hsl(qcT_sb, h, qb * 128, 128),
                    hsl(kT_sb, h, half * 512, 512),
                    start=True, stop=False)
                if qb == 0 and half == 1:
                    # scores[0, 1023] += (q+pos_bias)[1] . p[0]
                    # (the reference rel_shift reshape wraps this element)
                    nc.tensor.matmul(
                        sps[0:1, 511:512],
                        hsl(qpT_sb, h, 1, 1),
                        hsl(pT_sb, h, 0, 1, stride=PL),
                        start=False, stop=False)
                nc.tensor.matmul(
                    sps[:],
                    ident8[:],
                    shifted[:, qb * T + half * 512:
                            qb * T + half * 512 + 512],
                    start=False, stop=True)
                nc.scalar.activation(
                    E_sb[:, half * 512:(half + 1) * 512], sps[:], AF.Exp,
                    scale=0.125, accum_out=den[:, half:half + 1])
            rec = sb3.tile([128, 1], F32, tag="rec", name="rec")
            nc.vector.tensor_tensor(rec[:], den[:, 0:1], den[:, 1:2],
                                    op=OP.add)
            nc.vector.reciprocal(rec[:], rec[:])
            nc.vector.tensor_scalar_mul(E_sb[:], E_sb[:], rec[:])
            return E_sb

        def transpose_qb(qb, E_sb, ET_sb):
            # transpose E (f16): 8 PE transposes -> one f16 psum bank
            etps = ps_et.tile([128, T], F16, tag="et")
            for kc in range(QB):
                nc.tensor.transpose(
                    etps[:, kc * 128: kc * 128 + 128],
                    E_sb[:, kc * 128: kc * 128 + 128],
                    ident16[:])
            # scatter-extract: ET_sb[:, kc*T + qb*128 ...] = etps block kc
            dst = bass.AP(ET_sb[:].tensor, qb * 128,
                          [[QB * T, 128], [T, QB], [1, 128]])
            nc.vector.tensor_copy(dst, etps[:])

        def attnv_chunk(p, ib, hh, kh, ET_pair, otps):
            hloc = 2 * p + hh
            for kc in range(4 * kh, 4 * kh + 4):
                nc.tensor.matmul(
                    otps[hh * 64:hh * 64 + 64, :],
                    v_sb[:, kc * 256 + hloc * 64: kc * 256 + hloc * 64 + 64],
                    ET_pair[hh][:, kc * T + ib * 512: kc * T + ib * 512 + 512],
                    start=(kc == 0), stop=(kc == QB - 1))

        def attnv_extract(p, ib, otps):
            dst = oT_sb[:, p * T + ib * 512: p * T + ib * 512 + 512]
            if (p + ib) % 2 == 0:
                nc.vector.tensor_copy(dst, otps[:])
            else:
                nc.scalar.copy(dst, otps[:])

        def out_proj(t8):
            ops_ = ps_sc.tile([128, 512], F32, tag="sc")
            for p in range(NP):
                nc.tensor.matmul(
                    ops_[:],
                    oT_sb[:, p * T + t8 * 128: p * T + t8 * 128 + 128],
                    wo_sb[:, p * D:(p + 1) * D],
                    start=(p == 0), stop=(p == NP - 1))
            osb = sb2.tile([128, 512], F16, tag="osb", name="osb")
            if t8 % 2 == 0:
                nc.vector.tensor_copy(osb[:], ops_[:])
            else:
                nc.scalar.copy(osb[:], ops_[:])
            nc.sync.dma_start(out_d[t8 * 128:(t8 + 1) * 128, :], osb[:])

        # ---- emission schedule ----
        # p proj first (needs no LayerNorm -> fills the stats-chain
        # latency). Only pair-0 projections + head-0 bands run before the
        # attention loop; pair-1 projections and v-proj are deferred as
        # per-q-block filler inside the head-0/1 loops so the in-order PE
        # queue reaches head-0 scores ~12us earlier.
        for nt in range(4):
            p_proj(0, nt)
        q_proj(0, 0)
        q_proj(0, 1)
        shifted = {0: shifted_tile()}
        for qb in range(QB):
            band_qb(0, qb, shifted[0])
        k_proj(0, 0)
        k_proj(0, 1)

        filler = {
            0: [lambda: q_proj(1, 0, ps_o, "o"),
                lambda: q_proj(1, 1, ps_o, "o"),
                lambda: k_proj(1, 0, ps_o, "o"),
                lambda: k_proj(1, 1, ps_o, "o"),
                lambda: p_proj(1, 0, ps_o, "o"),
                lambda: p_proj(1, 1, ps_o, "o"),
                lambda: p_proj(1, 2, ps_o, "o"),
                lambda: p_proj(1, 3, ps_o, "o")],
            1: [(lambda t8=t8: v_proj(t8, ps_o, "o")) for t8 in range(QB)],
        }

        ET_tiles = {}
        attnv_work = []   # deferred attn@V chunks for the previous pair
        for h in range(NH):
            # bufs=3: head h+2 must not wait on the deferred attn@V reads
            # of head h's ET (they interleave into head h+2's score loop)
            ET_sb = sb3.tile([128, QB * T], F16, tag="ET", name="ET")
            ET_tiles[h] = ET_sb
            if h + 1 < NH:
                shifted[h + 1] = shifted_tile()
            E_hist = []
            for qb in range(QB):
                if h + 1 < NH:
                    band_qb(h + 1, qb, shifted[h + 1])
                E_hist.append(scores_qb(h, qb, shifted[h]))
                if qb >= 2:
                    transpose_qb(qb - 2, E_hist[qb - 2], ET_sb)
                for fn_ in filler.get(h, [])[qb:qb + 1]:
                    fn_()
                # slot one deferred attn@V chunk of the previous pair
                if attnv_work:
                    attnv_work.pop(0)()
            transpose_qb(QB - 2, E_hist[QB - 2], ET_sb)
            transpose_qb(QB - 1, E_hist[QB - 1], ET_sb)
            if h % 2 == 1:
                p = h // 2
                ET_pair = (ET_tiles[2 * p], ET_tiles[2 * p + 1])
                chunks = []
                state = {}

                def make_chunk(p_, ib_, hh_, kh_, ET_pair_):
                    def go():
                        if ("ot", p_, ib_) not in state:
                            state[("ot", p_, ib_)] = ps_o.tile(
                                [128, 512], F32, tag="o", name="otps")
                        otps = state[("ot", p_, ib_)]
                        attnv_chunk(p_, ib_, hh_, kh_, ET_pair_, otps)
                        if hh_ == 1 and kh_ == 1:
                            attnv_extract(p_, ib_, otps)
                    return go

                for ib in range(2):
                    for hh in range(2):
                        for kh in range(2):
                            chunks.append(make_chunk(p, ib, hh, kh, ET_pair))
                if h == NH - 1:
                    for cfn in chunks[:4]:
                        cfn()
                    for i, cfn in enumerate(chunks[4:]):
                        out_proj(i)
                        cfn()
                    for t8 in range(4, QB):
                        out_proj(t8)
                else:
                    attnv_work.extend(chunks)

    nc.compile()
    return nc


_PROGRAM_CACHE: dict = {}


def _get_program() -> bass.Bass:
    if "nc" not in _PROGRAM_CACHE:
        _PROGRAM_CACHE["nc"] = _build_program()
    return _PROGRAM_CACHE["nc"]


def _prepare_in_maps(x, pos, content_bias, pos_bias, gamma, beta,
                     Wq, bq, Wk, bk, Wv, bv, Wp, Wo, bo):
    x = np.asarray(x, np.float32)
    pos = np.asarray(pos, np.float32)
    gamma = np.asarray(gamma, np.float32)
    beta = np.asarray(beta, np.float32)

    # gamma folding: y = yln*gamma + beta  =>  y@W = yln@(gamma*W) + beta@W
    def fold(W):
        W = np.asarray(W, np.float32)
        return W * gamma[:, None, None], np.einsum("d,dhk->hk", beta, W)

    Wq_f, bq_f = fold(Wq)
    Wk_f, bk_f = fold(Wk)
    Wv_f, bv_f = fold(Wv)
    Wp = np.asarray(Wp, np.float32)
    Wo = np.asarray(Wo, np.float32)

    in_maps = []
    for core in range(8):
        b = core // 2
        g = core % 2
        hs = slice(4 * g, 4 * g + 4)
        qcb = (np.asarray(bq) + np.asarray(content_bias) + bq_f)[hs]
        qpb = (np.asarray(bq) + np.asarray(pos_bias) + bq_f)[hs]
        kb = (np.asarray(bk) + bk_f)[hs]
        vb = (np.asarray(bv) + bv_f)[hs]
        # Wo pair-stacked: [128, NP*D]; pair p rows = Wo[2p] ++ Wo[2p+1]
        Wo_h = np.asarray(Wo)[hs]          # [4, DK, D]
        wo2 = np.concatenate(
            [np.concatenate([Wo_h[2 * p], Wo_h[2 * p + 1]], axis=0)
             for p in range(NP)], axis=1)  # [128, NP*D]
        in_maps.append({
            "xT": np.ascontiguousarray(x[b].T).astype(np.float16),
            "posT": np.ascontiguousarray(pos[b].T).astype(np.float16),
            "wq": np.ascontiguousarray(
                Wq_f[:, hs, :].reshape(D, NH * DK)).astype(np.float16),
            "wk": np.ascontiguousarray(
                Wk_f[:, hs, :].reshape(D, NH * DK)).astype(np.float16),
            "wv": np.ascontiguousarray(
                Wv_f[:, hs, :].reshape(D, NH * DK)).astype(np.float16),
            "wp": np.ascontiguousarray(
                Wp[:, hs, :].reshape(D, NH * DK)).astype(np.float16),
            "wo": np.ascontiguousarray(wo2).astype(np.float16),
            "qc_bias": np.ascontiguousarray(qcb.reshape(2, 128).T),
            "qp_bias": np.ascontiguousarray(qpb.reshape(2, 128).T),
            "k_bias": np.ascontiguousarray(kb.reshape(2, 128).T),
            "v_bias": np.ascontiguousarray(vb.reshape(NH * DK)),
        })

    return in_maps


def _combine(x, bo, results):
    parts = [r["out_partial"] for r in results]
    out = np.asarray(x, np.float32) + np.asarray(bo, np.float32)[None, None, :]
    for b in range(B):
        out[b] += parts[2 * b].astype(np.float32)
        out[b] += parts[2 * b + 1].astype(np.float32)
    return out.astype(np.float32)


def kernel(x, pos, content_bias, pos_bias, gamma, beta,
           Wq, bq, Wk, bk, Wv, bv, Wp, Wo, bo) -> np.ndarray:
    in_maps = _prepare_in_maps(x, pos, content_bias, pos_bias, gamma, beta,
                               Wq, bq, Wk, bk, Wv, bv, Wp, Wo, bo)
    nc = _get_program()
    res = run_bass_kernel_spmd(nc, in_maps, core_ids=list(range(8)))
    return _combine(x, bo, res.results)


# revision 8
# speedup vs baseline: 160.1568x; 1.0366x over previous
"""Trainium2 Bass kernel v2 for Transformer-XL style MHSA (nn_MHSAModule).

Problem (hardcoded):
  B=4, T=1024, D=512, H=8, DK=64, L=2*T-1=2047, eps=1e-3
  out = x + (MHSA(LayerNorm(x), pos) @ Wo + bo)

Sharding: 8 cores = 4 batches x 2 head-groups (4 heads each). Each core
returns a partial output [T, D] f16 (its heads' contribution); the host
sums the two partials per batch and adds the residual x + bo.

v2 design notes (vs v1):
  - f16 activations/weights on the matmul path; f8e4m3 only for matmul
    B-operands no vector engine reads back: kT, pT, and the rel-shift
    band bounce (halves its DMA volume).
  - exp runs on Act straight from PSUM with accum_out giving the softmax
    denominator for free; E is normalized by one per-partition f16
    tensor_scalar instead of v1's psum-copy + scale chain.
  - PSUM extraction is the scarce resource (~4x an SBUF read): band
    extraction is split across DVE/Act/Pool, scores are extracted by the
    exp itself, ET extracted as f16 (2x cheaper than f32).
  - LayerNorm stats via f16 ones-matmuls; a/b rows reach all partitions
    via gpsimd.partition_broadcast instead of a DRAM round-trip.
  - attention@V stacks head pairs on 128 PSUM partitions; Wo is
    pair-stacked so the output projection contracts 128 rows per step.
  - engines execute in-order, so emission order IS the schedule: x/wq
    load first, the band of head h+1 and the attn@V of the previous pair
    are interleaved into head h's per-q-block score loop.
"""
import numpy as np
from contextlib import ExitStack

import concourse.bass as bass
import concourse.bacc as bacc
import concourse.tile as tile
from concourse import mybir
from concourse import masks
from concourse.bass_utils import run_bass_kernel_spmd

F32 = mybir.dt.float32
F16 = mybir.dt.float16
F8 = mybir.dt.float8e4
AF = mybir.ActivationFunctionType
OP = mybir.AluOpType

B, T, D, H, DK = 4, 1024, 512, 8, 64
L = 2 * T - 1
EPS = 1e-3
NH = 4          # heads per core
NP = 2          # head pairs per core
CH = D // 128   # 4 contraction chunks
QB = T // 128   # 8 q blocks
BAND = 1152    # positional band width per q block
PL = L + 2      # padded pT free size (2 zero pad cols)


def _build_program() -> bass.Bass:
    nc = bacc.Bacc("TRN2", target_bir_lowering=False, debug=False)

    # ---- DRAM I/O ----
    xT = nc.dram_tensor("xT", [D, T], F16, kind="ExternalInput")
    posT = nc.dram_tensor("posT", [D, L], F16, kind="ExternalInput")
    wq = nc.dram_tensor("wq", [D, NH * DK], F16, kind="ExternalInput")
    wk = nc.dram_tensor("wk", [D, NH * DK], F16, kind="ExternalInput")
    wv = nc.dram_tensor("wv", [D, NH * DK], F16, kind="ExternalInput")
    wp = nc.dram_tensor("wp", [D, NH * DK], F16, kind="ExternalInput")
    wo = nc.dram_tensor("wo", [128, NP * D], F16, kind="ExternalInput")
    qc_bias = nc.dram_tensor("qc_bias", [128, NP], F32, kind="ExternalInput")
    qp_bias = nc.dram_tensor("qp_bias", [128, NP], F32, kind="ExternalInput")
    k_bias = nc.dram_tensor("k_bias", [128, NP], F32, kind="ExternalInput")
    v_bias = nc.dram_tensor("v_bias", [NH * DK], F32, kind="ExternalInput")
    out_d = nc.dram_tensor("out_partial", [T, D], F16, kind="ExternalOutput")

    # internal scratch: rel-shift bounce, f8, double buffered
    bounce = nc.dram_tensor("bounce", [2, QB, 128, BAND], F8)

    with tile.TileContext(nc) as tc, ExitStack() as ctx:
        sb = ctx.enter_context(tc.tile_pool(name="sb", bufs=1))
        sb2 = ctx.enter_context(tc.tile_pool(name="sb2", bufs=2))
        sb3 = ctx.enter_context(tc.tile_pool(name="sb3", bufs=3))
        sbE = ctx.enter_context(tc.tile_pool(name="sbE", bufs=6))
        # PSUM: sc 2x[128,512]f32 (2 banks) + band 1x[128,1152]f32
        # (3 banks) + et 2x[128,1024]f16 (2 banks) + o 1x[128,512]f32
        # (1 bank) = 8 banks.
        ps_sc = ctx.enter_context(tc.tile_pool(name="ps_sc", bufs=2,
                                               space="PSUM"))
        # band psum: three independently-released tiles so the next band
        # matmul only waits on the one engine that extracts each slice
        ps_b = ctx.enter_context(tc.tile_pool(name="ps_b", bufs=1,
                                              space="PSUM"))
        ps_et = ctx.enter_context(tc.tile_pool(name="ps_et", bufs=2,
                                               space="PSUM"))
        ps_o = ctx.enter_context(tc.tile_pool(name="ps_o", bufs=1,
                                              space="PSUM"))

        # ---- persistent SBUF ----
        xT_sb = sb.tile([128, CH * T], F16)
        yT_sb = sb.tile([128, CH * T], F16)
        posT_sb = sb.tile([128, CH * L + 2], F16)
        pT_sb = sb.tile([128, NP * PL], F8)
        qcT_sb = sb.tile([128, NP * T], F8)
        qpT_sb = sb.tile([128, NP * T], F8)
        kT_sb = sb.tile([128, NP * T], F8)
        v_sb = sb.tile([128, QB * NH * DK], F16)
        oT_sb = sb.tile([128, NP * T], F16)
        wq_sb = sb.tile([128, CH * 256], F16)
        wk_sb = sb.tile([128, CH * 256], F16)
        wv_sb = sb.tile([128, CH * 256], F16)
        wp_sb = sb.tile([128, CH * 256], F16)
        wo_sb = sb.tile([128, NP * D], F16)
        qcb_sb = sb.tile([128, NP], F32)
        qpb_sb = sb.tile([128, NP], F32)
        kb_sb = sb.tile([128, NP], F32)
        vb_sb = sb.tile([128, 256], F32)
        arep = sb.tile([128, T], F32)
        brep = sb.tile([128, T], F32)
        ident16 = sb.tile([128, 128], F16)
        ident8 = sb.tile([128, 128], F8)
        ones_col = sb.tile([128, 1], F16)
        eps_col = sb.tile([1, 1], F32)

        nc.vector.memset(ones_col[:], 1.0)
        nc.vector.memset(eps_col[:], EPS)

        # ---- loads: x first (stats), then pos+wp (p proj), then q/k ----
        for c in range(CH):
            nc.sync.dma_start(xT_sb[:, c * T:(c + 1) * T],
                              xT[c * 128:(c + 1) * 128, :])
        for c in range(CH):
            nc.sync.dma_start(posT_sb[:, c * L:(c + 1) * L],
                              posT[c * 128:(c + 1) * 128, :])
            nc.sync.dma_start(wp_sb[:, c * 256:(c + 1) * 256],
                              wp[c * 128:(c + 1) * 128, :])
        for c in range(CH):
            nc.sync.dma_start(wq_sb[:, c * 256:(c + 1) * 256],
                              wq[c * 128:(c + 1) * 128, :])
            nc.sync.dma_start(wk_sb[:, c * 256:(c + 1) * 256],
                              wk[c * 128:(c + 1) * 128, :])
        nc.sync.dma_start(qcb_sb[:], qc_bias[:])
        nc.sync.dma_start(qpb_sb[:], qp_bias[:])
        nc.sync.dma_start(kb_sb[:], k_bias[:])
        for c in range(CH):
            nc.sync.dma_start(wv_sb[:, c * 256:(c + 1) * 256],
                              wv[c * 128:(c + 1) * 128, :])
        nc.sync.dma_start(wo_sb[:], wo[:])
        nc.sync.dma_start(
            vb_sb[:], bass.AP(v_bias[:].tensor, 0, [[0, 128], [1, 256]]))

        # ---- LayerNorm stats; short [1,512]-row chain (row ops are
        # lane-serial, so every op counts): a = rsqrt(var+eps),
        # b = -mu*a, computed as
        #   t1 = s1*s1; v' = D*s2 - t1; a = Rsqrt(v'/D^2 + eps);
        #   b = (s1 * -1/D) * a
        a_row = sb.tile([1, T], F32)
        b_row = sb.tile([1, T], F32)
        for tt in range(2):
            s1 = ps_b.tile([1, 512], F32, tag="band0")
            for c in range(CH):
                xt = xT_sb[:, c * T + tt * 512: c * T + tt * 512 + 512]
                nc.tensor.matmul(s1[:], ones_col[:], xt,
                                 start=(c == 0), stop=(c == CH - 1))
            s2 = ps_b.tile([1, 512], F32, tag="band1")
            for c in range(CH):
                xsq = sb3.tile([128, 512], F16, tag="xsq", name="xsq")
                xt = xT_sb[:, c * T + tt * 512: c * T + tt * 512 + 512]
                nc.vector.tensor_tensor(xsq[:], xt, xt, op=OP.mult)
                nc.tensor.matmul(s2[:], ones_col[:], xsq[:],
                                 start=(c == 0), stop=(c == CH - 1))
            t1 = sb2.tile([1, 512], F32, tag="t1", name="t1")
            nc.scalar.activation(t1[:], s1[:], AF.Square)
            vv = sb2.tile([1, 512], F32, tag="vv", name="vv")
            nc.vector.scalar_tensor_tensor(vv[:], s2[:], float(D), t1[:],
                                           op0=OP.mult, op1=OP.subtract)
            stdh = sb2.tile([1, 512], F32, tag="stdh", name="stdh")
            nc.scalar.activation(stdh[:], vv[:], AF.Sqrt,
                                 scale=1.0 / (D * D), bias=eps_col[:])
            a_half = a_row[:, tt * 512:(tt + 1) * 512]
            nc.vector.reciprocal(a_half, stdh[:])
            nc.vector.scalar_tensor_tensor(
                b_row[:, tt * 512:(tt + 1) * 512], s1[:], -1.0 / D, a_half,
                op0=OP.mult, op1=OP.mult)
        nc.gpsimd.partition_broadcast(arep[:], a_row[:])
        nc.gpsimd.partition_broadcast(brep[:], b_row[:])
        masks.make_identity(nc, ident16[:])
        masks.make_identity(nc, ident8[:])

        # ---- LayerNorm apply: yT = xT * a + b (f16), 512-col pieces so
        # the first projections start after the nt=0 halves ----
        for nt in range(2):
            for c in range(CH):
                eng = nc.gpsimd if c == 1 else nc.vector
                xs = xT_sb[:, c * T + nt * 512: c * T + nt * 512 + 512]
                ys = yT_sb[:, c * T + nt * 512: c * T + nt * 512 + 512]
                ar = arep[:, nt * 512:(nt + 1) * 512]
                br = brep[:, nt * 512:(nt + 1) * 512]
                eng.tensor_tensor(ys, xs, ar, op=OP.mult)
                eng.tensor_tensor(ys, ys, br, op=OP.add)

        # pad pT columns
        z8 = sb.tile([128, 4], F8)
        nc.vector.memset(z8[:], 0.0)
        zrow = sb.tile([128, 2], F16)
        nc.vector.memset(zrow[:], 0.0)
        nc.vector.tensor_copy(posT_sb[:, CH * L:], zrow[:])

        # ---- projection emitters ----
        # extraction engines are spread (qc->Act, qp->DVE, k/p->Pool,
        # v->DVE) so consecutive users of the psum ping-pong release in
        # parallel queues.
        def q_proj(p, nt, pool=None, ptag="sc"):
            prj = (pool or ps_sc).tile([128, 512], F32, tag=ptag,
                                       name="prj")
            for ci, c in enumerate((0, 2, 1, 3)):
                nc.tensor.matmul(
                    prj[:],
                    wq_sb[:, c * 256 + p * 128: c * 256 + p * 128 + 128],
                    yT_sb[:, c * T + nt * 512: c * T + nt * 512 + 512],
                    start=(c == 0), stop=(c == CH - 1))
            o = p * T + nt * 512
            nc.scalar.activation(qcT_sb[:, o:o + 512], prj[:],
                                 AF.Identity, bias=qcb_sb[:, p:p + 1])
            nc.vector.tensor_scalar_add(qpT_sb[:, o:o + 512], prj[:],
                                        qpb_sb[:, p:p + 1])

        def k_proj(p, nt, pool=None, ptag="sc"):
            prjk = (pool or ps_sc).tile([128, 512], F32, tag=ptag,
                                        name="prjk")
            for ci, c in enumerate((0, 2, 1, 3)):
                nc.tensor.matmul(
                    prjk[:],
                    wk_sb[:, c * 256 + p * 128: c * 256 + p * 128 + 128],
                    yT_sb[:, c * T + nt * 512: c * T + nt * 512 + 512],
                    start=(ci == 0), stop=(ci == CH - 1))
            nc.scalar.activation(kT_sb[:, p * T + nt * 512:
                                       p * T + nt * 512 + 512],
                                 prjk[:], AF.Identity,
                                 bias=kb_sb[:, p:p + 1])

        def p_proj(p, nt, pool=None, ptag="sc"):
            pps = (pool or ps_sc).tile([128, 512], F32, tag=ptag,
                                       name="pps")
            for ci, c in enumerate((0, 2, 1, 3)):
                nc.tensor.matmul(
                    pps[:],
                    wp_sb[:, c * 256 + p * 128: c * 256 + p * 128 + 128],
                    posT_sb[:, c * L + nt * 512: c * L + nt * 512 + 512],
                    start=(ci == 0), stop=(ci == CH - 1))
            nc.scalar.copy(
                pT_sb[:, p * PL + nt * 512: p * PL + nt * 512 + 512],
                pps[:])
            if nt == 3:
                nc.gpsimd.tensor_copy(pT_sb[:, p * PL + L: (p + 1) * PL],
                                      z8[:, :PL - L])

        def v_proj(t8, pool=None, ptag="sc"):
            vps = (pool or ps_sc).tile([128, 256], F32, tag=ptag,
                                       name="vps")
            for c in range(CH):
                nc.tensor.matmul(
                    vps[:],
                    yT_sb[:, c * T + t8 * 128: c * T + t8 * 128 + 128],
                    wv_sb[:, c * 256:(c + 1) * 256],
                    start=(c == 0), stop=(c == CH - 1))
            nc.vector.tensor_tensor(
                v_sb[:, t8 * 256:(t8 + 1) * 256], vps[:], vb_sb[:],
                op=OP.add)

        # ---- attention emitters ----
        def hsl(tile_, h, lo, w, stride=T):
            p, off = h // 2, (h % 2) * 64
            return tile_[off:off + 64, p * stride + lo: p * stride + lo + w]

        def band_qb(h, qb, shifted):
            """Band scores for (h, qb), bounce out, and the skewed read of
            this qb's block straight back into `shifted`. Three psum tiles,
            one extraction engine each, so the next band matmul waits on
            exactly one engine per tile."""
            ping = h % 2
            s0 = 897 - qb * 128
            qp_blk = hsl(qpT_sb, h, qb * 128, 128)
            bps0 = ps_b.tile([128, 768], F32, tag="band0")
            nc.tensor.matmul(bps0[:, :512], qp_blk,
                             hsl(pT_sb, h, s0, 512, stride=PL),
                             start=True, stop=True)
            nc.tensor.matmul(bps0[:, 512:], qp_blk,
                             hsl(pT_sb, h, s0 + 512, 256, stride=PL),
                             start=True, stop=True)
            bps1 = ps_b.tile([128, 384], F32, tag="band1")
            nc.tensor.matmul(bps1[:], qp_blk,
                             hsl(pT_sb, h, s0 + 768, 384, stride=PL),
                             start=True, stop=True)
            b8 = sbE.tile([128, BAND], F8, tag="band8", name="b8")
            nc.vector.tensor_copy(b8[:, :768], bps0[:])
            nc.scalar.copy(b8[:, 768:], bps1[:])
            nc.sync.dma_start(bounce[ping, qb], b8[:])
            if qb % 2 == 1:
                src = bass.AP(bounce[:].tensor,
                              (ping * QB + qb - 1) * (128 * BAND) + 127,
                              [[BAND - 1, 128], [128 * BAND, 2], [1, T]])
                nc.sync.dma_start(
                    shifted[:, (qb - 1) * T:(qb + 1) * T], src)

        def shifted_tile():
            shifted = sb2.tile([128, QB * T], F8, tag="shifted",
                               name="shifted")
            return shifted

        def scores_qb(h, qb, shifted):
            """Scores+exp+normalize for (h, qb); returns the E tile.
            Transposes are emitted one qb later (transpose_qb) so the PE
            queue never waits on the Act/DVE chain of the same qb."""
            E_sb = sbE.tile([128, T], F16, tag="E", name="E")
            den = sb3.tile([128, 2], F32, tag="den", name="den")
            for half in range(2):
                sps = ps_sc.tile([128, 512], F32, tag="sc")
                nc.tensor.matmul(
                    sps[:],
                    hsl(qcT_sb, h, qb * 128, 128),
                    hsl(kT_sb, h, half * 512, 512),
                    start=True, stop=False)
                if qb == 0 and half == 1:
                    # scores[0, 1023] += (q+pos_bias)[1] . p[0]
                    # (the reference rel_shift reshape wraps this element)
                    nc.tensor.matmul(
                        sps[0:1, 511:512],
                        hsl(qpT_sb, h, 1, 1),
                        hsl(pT_sb, h, 0, 1, stride=PL),
                        start=False, stop=False)
                nc.tensor.matmul(
                    sps[:],
                    ident8[:],
                    shifted[:, qb * T + half * 512:
                            qb * T + half * 512 + 512],
                    start=False, stop=True)
                nc.scalar.activation(
                    E_sb[:, half * 512:(half + 1) * 512], sps[:], AF.Exp,
                    scale=0.125, accum_out=den[:, half:half + 1])
            rec = sb3.tile([128, 1], F32, tag="rec", name="rec")
            nc.vector.tensor_tensor(rec[:], den[:, 0:1], den[:, 1:2],
                                    op=OP.add)
            nc.vector.reciprocal(rec[:], rec[:])
            nc.vector.tensor_scalar_mul(E_sb[:], E_sb[:], rec[:])
            return E_sb

        def transpose_qb(qb, E_sb, ET_sb):
            # transpose E (f16): 8 PE transposes -> one f16 psum bank
            etps = ps_et.tile([128, T], F16, tag="et")
            for kc in range(QB):
                nc.tensor.transpose(
                    etps[:, kc * 128: kc * 128 + 128],
                    E_sb[:, kc * 128: kc * 128 + 128],
                    ident16[:])
            # scatter-extract: ET_sb[:, kc*T + qb*128 ...] = etps block kc
            dst = bass.AP(ET_sb[:].tensor, qb * 128,
                          [[QB * T, 128], [T, QB], [1, 128]])
            nc.vector.tensor_copy(dst, etps[:])

        def attnv_chunk(p, ib, hh, kh, ET_pair, otps):
            hloc = 2 * p + hh
            for kc in range(4 * kh, 4 * kh + 4):
                nc.tensor.matmul(
                    otps[hh * 64:hh * 64 + 64, :],
                    v_sb[:, kc * 256 + hloc * 64: kc * 256 + hloc * 64 + 64],
                    ET_pair[hh][:, kc * T + ib * 512: kc * T + ib * 512 + 512],
                    start=(kc == 0), stop=(kc == QB - 1))

        def attnv_extract(p, ib, otps):
            dst = oT_sb[:, p * T + ib * 512: p * T + ib * 512 + 512]
            if (p + ib) % 2 == 0:
                nc.vector.tensor_copy(dst, otps[:])
            else:
                nc.scalar.copy(dst, otps[:])

        def out_proj(t8):
            ops_ = ps_sc.tile([128, 512], F32, tag="sc")
            for p in range(NP):
                nc.tensor.matmul(
                    ops_[:],
                    oT_sb[:, p * T + t8 * 128: p * T + t8 * 128 + 128],
                    wo_sb[:, p * D:(p + 1) * D],
                    start=(p == 0), stop=(p == NP - 1))
            osb = sbE.tile([128, 512], F16, tag="osb", name="osb")
            if t8 % 2 == 0:
                nc.vector.tensor_copy(osb[:], ops_[:])
            else:
                nc.scalar.copy(osb[:], ops_[:])
            nc.sync.dma_start(out_d[t8 * 128:(t8 + 1) * 128, :], osb[:])

        # ---- emission schedule ----
        # p proj first (needs no LayerNorm -> fills the stats-chain
        # latency). Only pair-0 projections + head-0 bands run before the
        # attention loop; pair-1 projections and v-proj are deferred as
        # per-q-block filler inside the head-0/1 loops so the in-order PE
        # queue reaches head-0 scores ~12us earlier.
        for nt in range(4):
            p_proj(0, nt)
        q_proj(0, 0)
        q_proj(0, 1)
        shifted = {0: shifted_tile()}
        for qb in range(QB):
            band_qb(0, qb, shifted[0])
        k_proj(0, 0)
        k_proj(0, 1)

        filler = {
            0: [lambda: q_proj(1, 0, ps_o, "o"),
                lambda: q_proj(1, 1, ps_o, "o"),
                lambda: k_proj(1, 0, ps_o, "o"),
                lambda: k_proj(1, 1, ps_o, "o"),
                lambda: p_proj(1, 0, ps_o, "o"),
                lambda: p_proj(1, 1, ps_o, "o"),
                lambda: p_proj(1, 2, ps_o, "o"),
                lambda: p_proj(1, 3, ps_o, "o")],
            1: [(lambda t8=t8: v_proj(t8, ps_o, "o")) for t8 in range(QB)],
        }

        ET_tiles = {}
        attnv_work = []   # deferred attn@V chunks for the previous pair
        for h in range(NH):
            # bufs=3: head h+2 must not wait on the deferred attn@V reads
            # of head h's ET (they interleave into head h+2's score loop)
            ET_sb = sb3.tile([128, QB * T], F16, tag="ET", name="ET")
            ET_tiles[h] = ET_sb
            if h + 1 < NH:
                shifted[h + 1] = shifted_tile()
            E_hist = []
            for qb in range(QB):
                if h + 1 < NH:
                    band_qb(h + 1, qb, shifted[h + 1])
                E_hist.append(scores_qb(h, qb, shifted[h]))
                if qb >= 2:
                    transpose_qb(qb - 2, E_hist[qb - 2], ET_sb)
                for fn_ in filler.get(h, [])[qb:qb + 1]:
                    fn_()
                # slot one deferred attn@V chunk of the previous pair
                if attnv_work:
                    attnv_work.pop(0)()
            transpose_qb(QB - 2, E_hist[QB - 2], ET_sb)
            transpose_qb(QB - 1, E_hist[QB - 1], ET_sb)
            if h % 2 == 1:
                p = h // 2
                ET_pair = (ET_tiles[2 * p], ET_tiles[2 * p + 1])
                chunks = []
                state = {}

                def make_chunk(p_, ib_, hh_, kh_, ET_pair_):
                    def go():
                        if ("ot", p_, ib_) not in state:
                            state[("ot", p_, ib_)] = ps_o.tile(
                                [128, 512], F32, tag="o", name="otps")
                        otps = state[("ot", p_, ib_)]
                        attnv_chunk(p_, ib_, hh_, kh_, ET_pair_, otps)
                        if hh_ == 1 and kh_ == 1:
                            attnv_extract(p_, ib_, otps)
                    return go

                for ib in range(2):
                    for hh in range(2):
                        for kh in range(2):
                            chunks.append(make_chunk(p, ib, hh, kh, ET_pair))
                if h == NH - 1:
                    for cfn in chunks[:4]:
                        cfn()
                    for i, cfn in enumerate(chunks[4:]):
                        out_proj(i)
                        cfn()
                    for t8 in range(4, QB):
                        out_proj(t8)
                else:
                    attnv_work.extend(chunks)

    nc.compile()
    return nc


_PROGRAM_CACHE: dict = {}


def _get_program() -> bass.Bass:
    if "nc" not in _PROGRAM_CACHE:
        _PROGRAM_CACHE["nc"] = _build_program()
    return _PROGRAM_CACHE["nc"]


def _prepare_in_maps(x, pos, content_bias, pos_bias, gamma, beta,
                     Wq, bq, Wk, bk, Wv, bv, Wp, Wo, bo):
    x = np.asarray(x, np.float32)
    pos = np.asarray(pos, np.float32)
    gamma = np.asarray(gamma, np.float32)
    beta = np.asarray(beta, np.float32)

    # gamma folding: y = yln*gamma + beta  =>  y@W = yln@(gamma*W) + beta@W
    def fold(W):
        W = np.asarray(W, np.float32)
        return W * gamma[:, None, None], np.einsum("d,dhk->hk", beta, W)

    Wq_f, bq_f = fold(Wq)
    Wk_f, bk_f = fold(Wk)
    Wv_f, bv_f = fold(Wv)
    Wp = np.asarray(Wp, np.float32)
    Wo = np.asarray(Wo, np.float32)

    in_maps = []
    for core in range(8):
        b = core // 2
        g = core % 2
        hs = slice(4 * g, 4 * g + 4)
        qcb = (np.asarray(bq) + np.asarray(content_bias) + bq_f)[hs]
        qpb = (np.asarray(bq) + np.asarray(pos_bias) + bq_f)[hs]
        kb = (np.asarray(bk) + bk_f)[hs]
        vb = (np.asarray(bv) + bv_f)[hs]
        # Wo pair-stacked: [128, NP*D]; pair p rows = Wo[2p] ++ Wo[2p+1]
        Wo_h = np.asarray(Wo)[hs]          # [4, DK, D]
        wo2 = np.concatenate(
            [np.concatenate([Wo_h[2 * p], Wo_h[2 * p + 1]], axis=0)
             for p in range(NP)], axis=1)  # [128, NP*D]
        in_maps.append({
            "xT": np.ascontiguousarray(x[b].T).astype(np.float16),
            "posT": np.ascontiguousarray(pos[b].T).astype(np.float16),
            "wq": np.ascontiguousarray(
                Wq_f[:, hs, :].reshape(D, NH * DK)).astype(np.float16),
            "wk": np.ascontiguousarray(
                Wk_f[:, hs, :].reshape(D, NH * DK)).astype(np.float16),
            "wv": np.ascontiguousarray(
                Wv_f[:, hs, :].reshape(D, NH * DK)).astype(np.float16),
            "wp": np.ascontiguousarray(
                Wp[:, hs, :].reshape(D, NH * DK)).astype(np.float16),
            "wo": np.ascontiguousarray(wo2).astype(np.float16),
            "qc_bias": np.ascontiguousarray(qcb.reshape(2, 128).T),
            "qp_bias": np.ascontiguousarray(qpb.reshape(2, 128).T),
            "k_bias": np.ascontiguousarray(kb.reshape(2, 128).T),
            "v_bias": np.ascontiguousarray(vb.reshape(NH * DK)),
        })

    return in_maps


def _combine(x, bo, results):
    parts = [r["out_partial"] for r in results]
    out = np.asarray(x, np.float32) + np.asarray(bo, np.float32)[None, None, :]
    for b in range(B):
        out[b] += parts[2 * b].astype(np.float32)
        out[b] += parts[2 * b + 1].astype(np.float32)
    return out.astype(np.float32)


def kernel(x, pos, content_bias, pos_bias, gamma, beta,
           Wq, bq, Wk, bk, Wv, bv, Wp, Wo, bo) -> np.ndarray:
    in_maps = _prepare_in_maps(x, pos, content_bias, pos_bias, gamma, beta,
                               Wq, bq, Wk, bk, Wv, bv, Wp, Wo, bo)
    nc = _get_program()
    res = run_bass_kernel_spmd(nc, in_maps, core_ids=list(range(8)))
    return _combine(x, bo, res.results)


# revision 9
# speedup vs baseline: 160.9419x; 1.0049x over previous
"""Trainium2 Bass kernel v2 for Transformer-XL style MHSA (nn_MHSAModule).

Problem (hardcoded):
  B=4, T=1024, D=512, H=8, DK=64, L=2*T-1=2047, eps=1e-3
  out = x + (MHSA(LayerNorm(x), pos) @ Wo + bo)

Sharding: 8 cores = 4 batches x 2 head-groups (4 heads each). Each core
returns a partial output [T, D] f16 (its heads' contribution); the host
sums the two partials per batch and adds the residual x + bo.

v2 design notes (vs v1):
  - f16 activations/weights on the matmul path; f8e4m3 only for matmul
    B-operands no vector engine reads back: kT, pT, and the rel-shift
    band bounce (halves its DMA volume).
  - exp runs on Act straight from PSUM with accum_out giving the softmax
    denominator for free; E is normalized by one per-partition f16
    tensor_scalar instead of v1's psum-copy + scale chain.
  - PSUM extraction is the scarce resource (~4x an SBUF read): band
    extraction is split across DVE/Act/Pool, scores are extracted by the
    exp itself, ET extracted as f16 (2x cheaper than f32).
  - LayerNorm stats via f16 ones-matmuls; a/b rows reach all partitions
    via gpsimd.partition_broadcast instead of a DRAM round-trip.
  - attention@V stacks head pairs on 128 PSUM partitions; Wo is
    pair-stacked so the output projection contracts 128 rows per step.
  - engines execute in-order, so emission order IS the schedule: x/wq
    load first, the band of head h+1 and the attn@V of the previous pair
    are interleaved into head h's per-q-block score loop.
"""
import numpy as np
from contextlib import ExitStack

import concourse.bass as bass
import concourse.bacc as bacc
import concourse.tile as tile
from concourse import mybir
from concourse import masks
from concourse.bass_utils import run_bass_kernel_spmd

F32 = mybir.dt.float32
F16 = mybir.dt.float16
F8 = mybir.dt.float8e4
AF = mybir.ActivationFunctionType
OP = mybir.AluOpType

B, T, D, H, DK = 4, 1024, 512, 8, 64
L = 2 * T - 1
EPS = 1e-3
NH = 4          # heads per core
NP = 2          # head pairs per core
CH = D // 128   # 4 contraction chunks
QB = T // 128   # 8 q blocks
BAND = 1152    # positional band width per q block
PL = L + 2      # padded pT free size (2 zero pad cols)


def _build_program() -> bass.Bass:
    nc = bacc.Bacc("TRN2", target_bir_lowering=False, debug=False)

    # ---- DRAM I/O ----
    xT = nc.dram_tensor("xT", [D, T], F16, kind="ExternalInput")
    posT = nc.dram_tensor("posT", [D, L], F16, kind="ExternalInput")
    wq = nc.dram_tensor("wq", [D, NH * DK], F16, kind="ExternalInput")
    wk = nc.dram_tensor("wk", [D, NH * DK], F16, kind="ExternalInput")
    wv = nc.dram_tensor("wv", [D, NH * DK], F16, kind="ExternalInput")
    wp = nc.dram_tensor("wp", [D, NH * DK], F16, kind="ExternalInput")
    wo = nc.dram_tensor("wo", [128, NP * D], F16, kind="ExternalInput")
    qc_bias = nc.dram_tensor("qc_bias", [128, NP], F32, kind="ExternalInput")
    qp_bias = nc.dram_tensor("qp_bias", [128, NP], F32, kind="ExternalInput")
    k_bias = nc.dram_tensor("k_bias", [128, NP], F32, kind="ExternalInput")
    v_bias = nc.dram_tensor("v_bias", [NH * DK], F32, kind="ExternalInput")
    out_d = nc.dram_tensor("out_partial", [T, D], F16, kind="ExternalOutput")

    # internal scratch: rel-shift bounce, f8, double buffered
    bounce = nc.dram_tensor("bounce", [2, QB, 128, BAND], F8)

    with tile.TileContext(nc) as tc, ExitStack() as ctx:
        sb = ctx.enter_context(tc.tile_pool(name="sb", bufs=1))
        sb2 = ctx.enter_context(tc.tile_pool(name="sb2", bufs=2))
        sb3 = ctx.enter_context(tc.tile_pool(name="sb3", bufs=3))
        sbE = ctx.enter_context(tc.tile_pool(name="sbE", bufs=6))
        # PSUM: sc 2x[128,512]f32 (2 banks) + band 1x[128,1152]f32
        # (3 banks) + et 2x[128,1024]f16 (2 banks) + o 1x[128,512]f32
        # (1 bank) = 8 banks.
        ps_sc = ctx.enter_context(tc.tile_pool(name="ps_sc", bufs=2,
                                               space="PSUM"))
        # band psum: three independently-released tiles so the next band
        # matmul only waits on the one engine that extracts each slice
        ps_b = ctx.enter_context(tc.tile_pool(name="ps_b", bufs=1,
                                              space="PSUM"))
        ps_et = ctx.enter_context(tc.tile_pool(name="ps_et", bufs=2,
                                               space="PSUM"))
        ps_o = ctx.enter_context(tc.tile_pool(name="ps_o", bufs=1,
                                              space="PSUM"))

        # ---- persistent SBUF ----
        xT_sb = sb.tile([128, CH * T], F16)
        yT_sb = sb.tile([128, CH * T], F16)
        posT_sb = sb.tile([128, CH * L + 2], F16)
        pT_sb = sb.tile([128, NP * PL], F8)
        qcT_sb = sb.tile([128, NP * T], F8)
        qpT_sb = sb.tile([128, NP * T], F8)
        kT_sb = sb.tile([128, NP * T], F8)
        v_sb = sb.tile([128, QB * NH * DK], F16)
        oT_sb = sb.tile([128, NP * T], F16)
        wq_sb = sb.tile([128, CH * 256], F16)
        wk_sb = sb.tile([128, CH * 256], F16)
        wv_sb = sb.tile([128, CH * 256], F16)
        wp_sb = sb.tile([128, CH * 256], F16)
        wo_sb = sb.tile([128, NP * D], F16)
        qcb_sb = sb.tile([128, NP], F32)
        qpb_sb = sb.tile([128, NP], F32)
        kb_sb = sb.tile([128, NP], F32)
        vb_sb = sb.tile([128, 256], F32)
        arep = sb.tile([128, T], F32)
        brep = sb.tile([128, T], F32)
        ident16 = sb.tile([128, 128], F16)
        ident8 = sb.tile([128, 128], F8)
        ones_col = sb.tile([128, 1], F16)
        eps_col = sb.tile([1, 1], F32)

        nc.vector.memset(ones_col[:], 1.0)
        nc.vector.memset(eps_col[:], EPS)

        # ---- loads: x first (stats), then pos+wp (p proj), then q/k ----
        for c in range(CH):
            nc.sync.dma_start(xT_sb[:, c * T:(c + 1) * T],
                              xT[c * 128:(c + 1) * 128, :])
        for c in range(CH):
            nc.sync.dma_start(posT_sb[:, c * L:(c + 1) * L],
                              posT[c * 128:(c + 1) * 128, :])
            nc.sync.dma_start(wp_sb[:, c * 256:(c + 1) * 256],
                              wp[c * 128:(c + 1) * 128, :])
            nc.sync.dma_start(wq_sb[:, c * 256:(c + 1) * 256],
                              wq[c * 128:(c + 1) * 128, :])
            nc.sync.dma_start(wk_sb[:, c * 256:(c + 1) * 256],
                              wk[c * 128:(c + 1) * 128, :])
        nc.sync.dma_start(qcb_sb[:], qc_bias[:])
        nc.sync.dma_start(qpb_sb[:], qp_bias[:])
        nc.sync.dma_start(kb_sb[:], k_bias[:])
        for c in range(CH):
            nc.sync.dma_start(wv_sb[:, c * 256:(c + 1) * 256],
                              wv[c * 128:(c + 1) * 128, :])
        nc.sync.dma_start(wo_sb[:], wo[:])
        nc.sync.dma_start(
            vb_sb[:], bass.AP(v_bias[:].tensor, 0, [[0, 128], [1, 256]]))

        # ---- LayerNorm stats; short [1,512]-row chain (row ops are
        # lane-serial, so every op counts): a = rsqrt(var+eps),
        # b = -mu*a, computed as
        #   t1 = s1*s1; v' = D*s2 - t1; a = Rsqrt(v'/D^2 + eps);
        #   b = (s1 * -1/D) * a
        a_row = sb.tile([1, T], F32)
        b_row = sb.tile([1, T], F32)
        for tt in range(2):
            s1 = ps_b.tile([1, 512], F32, tag="band0")
            for c in range(CH):
                xt = xT_sb[:, c * T + tt * 512: c * T + tt * 512 + 512]
                nc.tensor.matmul(s1[:], ones_col[:], xt,
                                 start=(c == 0), stop=(c == CH - 1))
            s2 = ps_b.tile([1, 512], F32, tag="band1")
            for c in range(CH):
                xsq = sb3.tile([128, 512], F16, tag="xsq", name="xsq")
                xt = xT_sb[:, c * T + tt * 512: c * T + tt * 512 + 512]
                nc.vector.tensor_tensor(xsq[:], xt, xt, op=OP.mult)
                nc.tensor.matmul(s2[:], ones_col[:], xsq[:],
                                 start=(c == 0), stop=(c == CH - 1))
            t1 = sb2.tile([1, 512], F32, tag="t1", name="t1")
            nc.scalar.activation(t1[:], s1[:], AF.Square)
            vv = sb2.tile([1, 512], F32, tag="vv", name="vv")
            nc.vector.scalar_tensor_tensor(vv[:], s2[:], float(D), t1[:],
                                           op0=OP.mult, op1=OP.subtract)
            stdh = sb2.tile([1, 512], F32, tag="stdh", name="stdh")
            nc.scalar.activation(stdh[:], vv[:], AF.Sqrt,
                                 scale=1.0 / (D * D), bias=eps_col[:])
            a_half = a_row[:, tt * 512:(tt + 1) * 512]
            nc.vector.reciprocal(a_half, stdh[:])
            nc.vector.scalar_tensor_tensor(
                b_row[:, tt * 512:(tt + 1) * 512], s1[:], -1.0 / D, a_half,
                op0=OP.mult, op1=OP.mult)
        nc.gpsimd.partition_broadcast(arep[:], a_row[:])
        nc.gpsimd.partition_broadcast(brep[:], b_row[:])
        masks.make_identity(nc, ident16[:])
        masks.make_identity(nc, ident8[:])

        # ---- LayerNorm apply: yT = xT * a + b (f16), 512-col pieces so
        # the first projections start after the nt=0 halves ----
        for nt in range(2):
            for c in range(CH):
                eng = nc.gpsimd if (c == 1 and nt == 1) else nc.vector
                xs = xT_sb[:, c * T + nt * 512: c * T + nt * 512 + 512]
                ys = yT_sb[:, c * T + nt * 512: c * T + nt * 512 + 512]
                ar = arep[:, nt * 512:(nt + 1) * 512]
                br = brep[:, nt * 512:(nt + 1) * 512]
                eng.tensor_tensor(ys, xs, ar, op=OP.mult)
                eng.tensor_tensor(ys, ys, br, op=OP.add)

        # pad pT columns
        z8 = sb.tile([128, 4], F8)
        nc.vector.memset(z8[:], 0.0)
        zrow = sb.tile([128, 2], F16)
        nc.vector.memset(zrow[:], 0.0)
        nc.vector.tensor_copy(posT_sb[:, CH * L:], zrow[:])

        # ---- projection emitters ----
        # extraction engines are spread (qc->Act, qp->DVE, k/p->Pool,
        # v->DVE) so consecutive users of the psum ping-pong release in
        # parallel queues.
        def q_proj(p, nt, pool=None, ptag="sc"):
            prj = (pool or ps_sc).tile([128, 512], F32, tag=ptag,
                                       name="prj")
            for ci, c in enumerate((0, 2, 1, 3)):
                nc.tensor.matmul(
                    prj[:],
                    wq_sb[:, c * 256 + p * 128: c * 256 + p * 128 + 128],
                    yT_sb[:, c * T + nt * 512: c * T + nt * 512 + 512],
                    start=(c == 0), stop=(c == CH - 1))
            o = p * T + nt * 512
            nc.scalar.activation(qcT_sb[:, o:o + 512], prj[:],
                                 AF.Identity, bias=qcb_sb[:, p:p + 1])
            nc.scalar.activation(qpT_sb[:, o:o + 512], prj[:],
                                 AF.Identity, bias=qpb_sb[:, p:p + 1])

        def k_proj(p, nt, pool=None, ptag="sc"):
            prjk = (pool or ps_sc).tile([128, 512], F32, tag=ptag,
                                        name="prjk")
            for ci, c in enumerate((0, 2, 1, 3)):
                nc.tensor.matmul(
                    prjk[:],
                    wk_sb[:, c * 256 + p * 128: c * 256 + p * 128 + 128],
                    yT_sb[:, c * T + nt * 512: c * T + nt * 512 + 512],
                    start=(ci == 0), stop=(ci == CH - 1))
            nc.scalar.activation(kT_sb[:, p * T + nt * 512:
                                       p * T + nt * 512 + 512],
                                 prjk[:], AF.Identity,
                                 bias=kb_sb[:, p:p + 1])

        def p_proj(p, nt, pool=None, ptag="sc"):
            pps = (pool or ps_sc).tile([128, 512], F32, tag=ptag,
                                       name="pps")
            for ci, c in enumerate((0, 2, 1, 3)):
                nc.tensor.matmul(
                    pps[:],
                    wp_sb[:, c * 256 + p * 128: c * 256 + p * 128 + 128],
                    posT_sb[:, c * L + nt * 512: c * L + nt * 512 + 512],
                    start=(ci == 0), stop=(ci == CH - 1))
            nc.scalar.copy(
                pT_sb[:, p * PL + nt * 512: p * PL + nt * 512 + 512],
                pps[:])
            if nt == 3:
                nc.gpsimd.tensor_copy(pT_sb[:, p * PL + L: (p + 1) * PL],
                                      z8[:, :PL - L])

        def v_proj(t8, pool=None, ptag="sc"):
            vps = (pool or ps_sc).tile([128, 256], F32, tag=ptag,
                                       name="vps")
            for c in range(CH):
                nc.tensor.matmul(
                    vps[:],
                    yT_sb[:, c * T + t8 * 128: c * T + t8 * 128 + 128],
                    wv_sb[:, c * 256:(c + 1) * 256],
                    start=(c == 0), stop=(c == CH - 1))
            nc.vector.tensor_tensor(
                v_sb[:, t8 * 256:(t8 + 1) * 256], vps[:], vb_sb[:],
                op=OP.add)

        # ---- attention emitters ----
        def hsl(tile_, h, lo, w, stride=T):
            p, off = h // 2, (h % 2) * 64
            return tile_[off:off + 64, p * stride + lo: p * stride + lo + w]

        def band_qb(h, qb, shifted):
            """Band scores for (h, qb), bounce out, and the skewed read of
            this qb's block straight back into `shifted`. Three psum tiles,
            one extraction engine each, so the next band matmul waits on
            exactly one engine per tile."""
            ping = h % 2
            s0 = 897 - qb * 128
            qp_blk = hsl(qpT_sb, h, qb * 128, 128)
            bps0 = ps_b.tile([128, 768], F32, tag="band0")
            nc.tensor.matmul(bps0[:, :512], qp_blk,
                             hsl(pT_sb, h, s0, 512, stride=PL),
                             start=True, stop=True)
            nc.tensor.matmul(bps0[:, 512:], qp_blk,
                             hsl(pT_sb, h, s0 + 512, 256, stride=PL),
                             start=True, stop=True)
            bps1 = ps_b.tile([128, 384], F32, tag="band1")
            nc.tensor.matmul(bps1[:], qp_blk,
                             hsl(pT_sb, h, s0 + 768, 384, stride=PL),
                             start=True, stop=True)
            b8 = sbE.tile([128, BAND], F8, tag="band8", name="b8")
            nc.vector.tensor_copy(b8[:, :768], bps0[:])
            nc.scalar.copy(b8[:, 768:], bps1[:])
            nc.sync.dma_start(bounce[ping, qb], b8[:])
            if qb % 2 == 1:
                src = bass.AP(bounce[:].tensor,
                              (ping * QB + qb - 1) * (128 * BAND) + 127,
                              [[BAND - 1, 128], [128 * BAND, 2], [1, T]])
                nc.sync.dma_start(
                    shifted[:, (qb - 1) * T:(qb + 1) * T], src)

        def shifted_tile():
            shifted = sb2.tile([128, QB * T], F8, tag="shifted",
                               name="shifted")
            return shifted

        def scores_qb(h, qb, shifted):
            """Scores+exp+normalize for (h, qb); returns the E tile.
            Transposes are emitted one qb later (transpose_qb) so the PE
            queue never waits on the Act/DVE chain of the same qb."""
            E_sb = sbE.tile([128, T], F16, tag="E", name="E")
            den = sb3.tile([128, 2], F32, tag="den", name="den")
            for half in range(2):
                sps = ps_sc.tile([128, 512], F32, tag="sc")
                nc.tensor.matmul(
                    sps[:],
                    hsl(qcT_sb, h, qb * 128, 128),
                    hsl(kT_sb, h, half * 512, 512),
                    start=True, stop=False)
                if qb == 0 and half == 1:
                    # scores[0, 1023] += (q+pos_bias)[1] . p[0]
                    # (the reference rel_shift reshape wraps this element)
                    nc.tensor.matmul(
                        sps[0:1, 511:512],
                        hsl(qpT_sb, h, 1, 1),
                        hsl(pT_sb, h, 0, 1, stride=PL),
                        start=False, stop=False)
                nc.tensor.matmul(
                    sps[:],
                    ident8[:],
                    shifted[:, qb * T + half * 512:
                            qb * T + half * 512 + 512],
                    start=False, stop=True)
                nc.scalar.activation(
                    E_sb[:, half * 512:(half + 1) * 512], sps[:], AF.Exp,
                    scale=0.125, accum_out=den[:, half:half + 1])
            rec = sb3.tile([128, 1], F32, tag="rec", name="rec")
            nc.vector.tensor_tensor(rec[:], den[:, 0:1], den[:, 1:2],
                                    op=OP.add)
            nc.vector.reciprocal(rec[:], rec[:])
            nc.vector.tensor_scalar_mul(E_sb[:], E_sb[:], rec[:])
            return E_sb

        def transpose_qb(qb, E_sb, ET_sb):
            # transpose E (f16): 8 PE transposes -> one f16 psum bank
            etps = ps_et.tile([128, T], F16, tag="et")
            for kc in range(QB):
                nc.tensor.transpose(
                    etps[:, kc * 128: kc * 128 + 128],
                    E_sb[:, kc * 128: kc * 128 + 128],
                    ident16[:])
            # scatter-extract: ET_sb[:, kc*T + qb*128 ...] = etps block kc
            dst = bass.AP(ET_sb[:].tensor, qb * 128,
                          [[QB * T, 128], [T, QB], [1, 128]])
            nc.vector.tensor_copy(dst, etps[:])

        def attnv_chunk(p, ib, hh, kh, ET_pair, otps):
            hloc = 2 * p + hh
            for kc in range(4 * kh, 4 * kh + 4):
                nc.tensor.matmul(
                    otps[hh * 64:hh * 64 + 64, :],
                    v_sb[:, kc * 256 + hloc * 64: kc * 256 + hloc * 64 + 64],
                    ET_pair[hh][:, kc * T + ib * 512: kc * T + ib * 512 + 512],
                    start=(kc == 0), stop=(kc == QB - 1))

        def attnv_extract(p, ib, otps):
            dst = oT_sb[:, p * T + ib * 512: p * T + ib * 512 + 512]
            if (p + ib) % 2 == 0:
                nc.vector.tensor_copy(dst, otps[:])
            else:
                nc.scalar.copy(dst, otps[:])

        def out_proj(t8):
            ops_ = ps_sc.tile([128, 512], F32, tag="sc")
            for p in range(NP):
                nc.tensor.matmul(
                    ops_[:],
                    oT_sb[:, p * T + t8 * 128: p * T + t8 * 128 + 128],
                    wo_sb[:, p * D:(p + 1) * D],
                    start=(p == 0), stop=(p == NP - 1))
            osb = sbE.tile([128, 512], F16, tag="osb", name="osb")
            if t8 % 2 == 0:
                nc.vector.tensor_copy(osb[:], ops_[:])
            else:
                nc.scalar.copy(osb[:], ops_[:])
            nc.sync.dma_start(out_d[t8 * 128:(t8 + 1) * 128, :], osb[:])

        # ---- emission schedule ----
        # p proj first (needs no LayerNorm -> fills the stats-chain
        # latency). Only pair-0 projections + head-0 bands run before the
        # attention loop; pair-1 projections and v-proj are deferred as
        # per-q-block filler inside the head-0/1 loops so the in-order PE
        # queue reaches head-0 scores ~12us earlier.
        for nt in range(4):
            p_proj(0, nt)
        q_proj(0, 0)
        q_proj(0, 1)
        shifted = {0: shifted_tile()}
        for qb in range(QB):
            band_qb(0, qb, shifted[0])
        k_proj(0, 0)
        k_proj(0, 1)

        filler = {
            0: [lambda: q_proj(1, 0, ps_o, "o"),
                lambda: q_proj(1, 1, ps_o, "o"),
                lambda: k_proj(1, 0, ps_o, "o"),
                lambda: k_proj(1, 1, ps_o, "o"),
                lambda: p_proj(1, 0, ps_o, "o"),
                lambda: p_proj(1, 1, ps_o, "o"),
                lambda: p_proj(1, 2, ps_o, "o"),
                lambda: p_proj(1, 3, ps_o, "o")],
            1: [(lambda t8=t8: v_proj(t8, ps_o, "o")) for t8 in range(QB)],
        }

        ET_tiles = {}
        attnv_work = []   # deferred attn@V chunks for the previous pair
        for h in range(NH):
            # bufs=3: head h+2 must not wait on the deferred attn@V reads
            # of head h's ET (they interleave into head h+2's score loop)
            ET_sb = sb3.tile([128, QB * T], F16, tag="ET", name="ET")
            ET_tiles[h] = ET_sb
            if h + 1 < NH:
                shifted[h + 1] = shifted_tile()
            E_hist = []
            for qb in range(QB):
                if h + 1 < NH:
                    band_qb(h + 1, qb, shifted[h + 1])
                E_hist.append(scores_qb(h, qb, shifted[h]))
                if qb >= 2:
                    transpose_qb(qb - 2, E_hist[qb - 2], ET_sb)
                for fn_ in filler.get(h, [])[qb:qb + 1]:
                    fn_()
                # slot one deferred attn@V chunk of the previous pair
                if attnv_work:
                    attnv_work.pop(0)()
            transpose_qb(QB - 2, E_hist[QB - 2], ET_sb)
            transpose_qb(QB - 1, E_hist[QB - 1], ET_sb)
            if h % 2 == 1:
                p = h // 2
                ET_pair = (ET_tiles[2 * p], ET_tiles[2 * p + 1])
                chunks = []
                state = {}

                def make_chunk(p_, ib_, hh_, kh_, ET_pair_):
                    def go():
                        if ("ot", p_, ib_) not in state:
                            state[("ot", p_, ib_)] = ps_o.tile(
                                [128, 512], F32, tag="o", name="otps")
                        otps = state[("ot", p_, ib_)]
                        attnv_chunk(p_, ib_, hh_, kh_, ET_pair_, otps)
                        if hh_ == 1 and kh_ == 1:
                            attnv_extract(p_, ib_, otps)
                    return go

                for ib in range(2):
                    for hh in range(2):
                        for kh in range(2):
                            chunks.append(make_chunk(p, ib, hh, kh, ET_pair))
                if h == NH - 1:
                    for cfn in chunks[:4]:
                        cfn()
                    for i, cfn in enumerate(chunks[4:]):
                        out_proj(i)
                        cfn()
                    for t8 in range(4, QB):
                        out_proj(t8)
                else:
                    attnv_work.extend(chunks)

    nc.compile()
    return nc


_PROGRAM_CACHE: dict = {}


def _get_program() -> bass.Bass:
    if "nc" not in _PROGRAM_CACHE:
        _PROGRAM_CACHE["nc"] = _build_program()
    return _PROGRAM_CACHE["nc"]


def _prepare_in_maps(x, pos, content_bias, pos_bias, gamma, beta,
                     Wq, bq, Wk, bk, Wv, bv, Wp, Wo, bo):
    x = np.asarray(x, np.float32)
    pos = np.asarray(pos, np.float32)
    gamma = np.asarray(gamma, np.float32)
    beta = np.asarray(beta, np.float32)

    # gamma folding: y = yln*gamma + beta  =>  y@W = yln@(gamma*W) + beta@W
    def fold(W):
        W = np.asarray(W, np.float32)
        return W * gamma[:, None, None], np.einsum("d,dhk->hk", beta, W)

    Wq_f, bq_f = fold(Wq)
    Wk_f, bk_f = fold(Wk)
    Wv_f, bv_f = fold(Wv)
    Wp = np.asarray(Wp, np.float32)
    Wo = np.asarray(Wo, np.float32)

    in_maps = []
    for core in range(8):
        b = core // 2
        g = core % 2
        hs = slice(4 * g, 4 * g + 4)
        qcb = (np.asarray(bq) + np.asarray(content_bias) + bq_f)[hs]
        qpb = (np.asarray(bq) + np.asarray(pos_bias) + bq_f)[hs]
        kb = (np.asarray(bk) + bk_f)[hs]
        vb = (np.asarray(bv) + bv_f)[hs]
        # Wo pair-stacked: [128, NP*D]; pair p rows = Wo[2p] ++ Wo[2p+1]
        Wo_h = np.asarray(Wo)[hs]          # [4, DK, D]
        wo2 = np.concatenate(
            [np.concatenate([Wo_h[2 * p], Wo_h[2 * p + 1]], axis=0)
             for p in range(NP)], axis=1)  # [128, NP*D]
        in_maps.append({
            "xT": np.ascontiguousarray(x[b].T).astype(np.float16),
            "posT": np.ascontiguousarray(pos[b].T).astype(np.float16),
            "wq": np.ascontiguousarray(
                Wq_f[:, hs, :].reshape(D, NH * DK)).astype(np.float16),
            "wk": np.ascontiguousarray(
                Wk_f[:, hs, :].reshape(D, NH * DK)).astype(np.float16),
            "wv": np.ascontiguousarray(
                Wv_f[:, hs, :].reshape(D, NH * DK)).astype(np.float16),
            "wp": np.ascontiguousarray(
                Wp[:, hs, :].reshape(D, NH * DK)).astype(np.float16),
            "wo": np.ascontiguousarray(wo2).astype(np.float16),
            "qc_bias": np.ascontiguousarray(qcb.reshape(2, 128).T),
            "qp_bias": np.ascontiguousarray(qpb.reshape(2, 128).T),
            "k_bias": np.ascontiguousarray(kb.reshape(2, 128).T),
            "v_bias": np.ascontiguousarray(vb.reshape(NH * DK)),
        })

    return in_maps


def _combine(x, bo, results):
    parts = [r["out_partial"] for r in results]
    out = np.asarray(x, np.float32) + np.asarray(bo, np.float32)[None, None, :]
    for b in range(B):
        out[b] += parts[2 * b].astype(np.float32)
        out[b] += parts[2 * b + 1].astype(np.float32)
    return out.astype(np.float32)


def kernel(x, pos, content_bias, pos_bias, gamma, beta,
           Wq, bq, Wk, bk, Wv, bv, Wp, Wo, bo) -> np.ndarray:
    in_maps = _prepare_in_maps(x, pos, content_bias, pos_bias, gamma, beta,
                               Wq, bq, Wk, bk, Wv, bv, Wp, Wo, bo)
    nc = _get_program()
    res = run_bass_kernel_spmd(nc, in_maps, core_ids=list(range(8)))
    return _combine(x, bo, res.results)


# revision 10
# speedup vs baseline: 164.2999x; 1.0209x over previous
"""Trainium2 Bass kernel v2 for Transformer-XL style MHSA (nn_MHSAModule).

Problem (hardcoded):
  B=4, T=1024, D=512, H=8, DK=64, L=2*T-1=2047, eps=1e-3
  out = x + (MHSA(LayerNorm(x), pos) @ Wo + bo)

Sharding: 8 cores = 4 batches x 2 head-groups (4 heads each). Each core
returns a partial output [T, D] f16 (its heads' contribution); the host
sums the two partials per batch and adds the residual x + bo.

v2 design notes (vs v1):
  - f16 activations/weights on the matmul path; f8e4m3 only for matmul
    B-operands no vector engine reads back: kT, pT, and the rel-shift
    band bounce (halves its DMA volume).
  - exp runs on Act straight from PSUM with accum_out giving the softmax
    denominator for free; E is normalized by one per-partition f16
    tensor_scalar instead of v1's psum-copy + scale chain.
  - PSUM extraction is the scarce resource (~4x an SBUF read): band
    extraction is split across DVE/Act/Pool, scores are extracted by the
    exp itself, ET extracted as f16 (2x cheaper than f32).
  - LayerNorm stats via f16 ones-matmuls; a/b rows reach all partitions
    via gpsimd.partition_broadcast instead of a DRAM round-trip.
  - attention@V stacks head pairs on 128 PSUM partitions; Wo is
    pair-stacked so the output projection contracts 128 rows per step.
  - engines execute in-order, so emission order IS the schedule: x/wq
    load first, the band of head h+1 and the attn@V of the previous pair
    are interleaved into head h's per-q-block score loop.
"""
import numpy as np
from contextlib import ExitStack

import concourse.bass as bass
import concourse.bacc as bacc
import concourse.tile as tile
from concourse import mybir
from concourse import masks
from concourse.bass_utils import run_bass_kernel_spmd

F32 = mybir.dt.float32
F16 = mybir.dt.float16
F8 = mybir.dt.float8e4
AF = mybir.ActivationFunctionType
OP = mybir.AluOpType

B, T, D, H, DK = 4, 1024, 512, 8, 64
L = 2 * T - 1
EPS = 1e-3
NH = 4          # heads per core
NP = 2          # head pairs per core
CH = D // 128   # 4 contraction chunks
QB = T // 128   # 8 q blocks
BAND = 1152    # positional band width per q block
PL = L + 2      # padded pT free size (2 zero pad cols)


def _build_program() -> bass.Bass:
    nc = bacc.Bacc("TRN2", target_bir_lowering=False, debug=False)

    # ---- DRAM I/O ----
    xT = nc.dram_tensor("xT", [D, T], F16, kind="ExternalInput")
    posT = nc.dram_tensor("posT", [D, L], F16, kind="ExternalInput")
    wq = nc.dram_tensor("wq", [D, NH * DK], F16, kind="ExternalInput")
    wk = nc.dram_tensor("wk", [D, NH * DK], F16, kind="ExternalInput")
    wv = nc.dram_tensor("wv", [D, NH * DK], F16, kind="ExternalInput")
    wp = nc.dram_tensor("wp", [D, NH * DK], F16, kind="ExternalInput")
    wo = nc.dram_tensor("wo", [128, NP * D], F16, kind="ExternalInput")
    qc_bias = nc.dram_tensor("qc_bias", [128, NP], F32, kind="ExternalInput")
    qp_bias = nc.dram_tensor("qp_bias", [128, NP], F32, kind="ExternalInput")
    k_bias = nc.dram_tensor("k_bias", [128, NP], F32, kind="ExternalInput")
    v_bias = nc.dram_tensor("v_bias", [NH * DK], F32, kind="ExternalInput")
    out_d = nc.dram_tensor("out_partial", [T, D], F16, kind="ExternalOutput")

    # internal scratch: rel-shift bounce, f8, double buffered
    bounce = nc.dram_tensor("bounce", [2, QB, 128, BAND], F8)

    with tile.TileContext(nc) as tc, ExitStack() as ctx:
        sb = ctx.enter_context(tc.tile_pool(name="sb", bufs=1))
        sb2 = ctx.enter_context(tc.tile_pool(name="sb2", bufs=2))
        sb3 = ctx.enter_context(tc.tile_pool(name="sb3", bufs=3))
        sbE = ctx.enter_context(tc.tile_pool(name="sbE", bufs=6))
        # PSUM: sc 2x[128,512]f32 (2 banks) + band 1x[128,1152]f32
        # (3 banks) + et 2x[128,1024]f16 (2 banks) + o 1x[128,512]f32
        # (1 bank) = 8 banks.
        ps_sc = ctx.enter_context(tc.tile_pool(name="ps_sc", bufs=2,
                                               space="PSUM"))
        # band psum: three independently-released tiles so the next band
        # matmul only waits on the one engine that extracts each slice
        ps_b = ctx.enter_context(tc.tile_pool(name="ps_b", bufs=1,
                                              space="PSUM"))
        ps_et = ctx.enter_context(tc.tile_pool(name="ps_et", bufs=2,
                                               space="PSUM"))
        ps_o = ctx.enter_context(tc.tile_pool(name="ps_o", bufs=1,
                                              space="PSUM"))

        # ---- persistent SBUF ----
        xT_sb = sb.tile([128, CH * T], F16)
        yT_sb = sb.tile([128, CH * T], F16)
        posT_sb = sb.tile([128, CH * L + 2], F16)
        pT_sb = sb.tile([128, NP * PL], F8)
        qcT_sb = sb.tile([128, NP * T], F8)
        qpT_sb = sb.tile([128, NP * T], F8)
        kT_sb = sb.tile([128, NP * T], F8)
        v_sb = sb.tile([128, QB * NH * DK], F16)
        oT_sb = sb.tile([128, NP * T], F16)
        wq_sb = sb.tile([128, CH * 256], F16)
        wk_sb = sb.tile([128, CH * 256], F16)
        wv_sb = sb.tile([128, CH * 256], F16)
        wp_sb = sb.tile([128, CH * 256], F16)
        wo_sb = sb.tile([128, NP * D], F16)
        qcb_sb = sb.tile([128, NP], F32)
        qpb_sb = sb.tile([128, NP], F32)
        kb_sb = sb.tile([128, NP], F32)
        vb_sb = sb.tile([128, 256], F32)
        arep = sb.tile([128, T], F32)
        brep = sb.tile([128, T], F32)
        ident16 = sb.tile([128, 128], F16)
        ident8 = sb.tile([128, 128], F8)
        ones_col = sb.tile([128, 1], F16)
        eps_col = sb.tile([1, 1], F32)

        nc.vector.memset(ones_col[:], 1.0)
        nc.vector.memset(eps_col[:], EPS)

        # ---- loads: x first (stats), then pos+wp (p proj), then q/k ----
        for tt in range(2):
            for c in range(CH):
                nc.sync.dma_start(
                    xT_sb[:, c * T + tt * 512: c * T + tt * 512 + 512],
                    xT[c * 128:(c + 1) * 128, tt * 512:(tt + 1) * 512])
        for c in range(CH):
            nc.sync.dma_start(wp_sb[:, c * 256:(c + 1) * 256],
                              wp[c * 128:(c + 1) * 128, :])
            nc.sync.dma_start(posT_sb[:, c * L:(c + 1) * L],
                              posT[c * 128:(c + 1) * 128, :])
        for c in range(CH):
            nc.sync.dma_start(wq_sb[:, c * 256:(c + 1) * 256],
                              wq[c * 128:(c + 1) * 128, :])
            nc.sync.dma_start(wk_sb[:, c * 256:(c + 1) * 256],
                              wk[c * 128:(c + 1) * 128, :])
        nc.sync.dma_start(qcb_sb[:], qc_bias[:])
        nc.sync.dma_start(qpb_sb[:], qp_bias[:])
        nc.sync.dma_start(kb_sb[:], k_bias[:])
        for c in range(CH):
            nc.sync.dma_start(wv_sb[:, c * 256:(c + 1) * 256],
                              wv[c * 128:(c + 1) * 128, :])
        nc.sync.dma_start(wo_sb[:], wo[:])
        nc.sync.dma_start(
            vb_sb[:], bass.AP(v_bias[:].tensor, 0, [[0, 128], [1, 256]]))

        # ---- LayerNorm stats; short [1,512]-row chain (row ops are
        # lane-serial, so every op counts): a = rsqrt(var+eps),
        # b = -mu*a, computed as
        #   t1 = s1*s1; v' = D*s2 - t1; a = Rsqrt(v'/D^2 + eps);
        #   b = (s1 * -1/D) * a
        a_row = sb.tile([1, T], F32)
        b_row = sb.tile([1, T], F32)
        for tt in range(2):
            s1 = ps_b.tile([1, 512], F32, tag="band0")
            for c in range(CH):
                xt = xT_sb[:, c * T + tt * 512: c * T + tt * 512 + 512]
                nc.tensor.matmul(s1[:], ones_col[:], xt,
                                 start=(c == 0), stop=(c == CH - 1))
            s2 = ps_b.tile([1, 512], F32, tag="band1")
            for c in range(CH):
                xsq = sb3.tile([128, 512], F16, tag="xsq", name="xsq")
                xt = xT_sb[:, c * T + tt * 512: c * T + tt * 512 + 512]
                nc.vector.tensor_tensor(xsq[:], xt, xt, op=OP.mult)
                nc.tensor.matmul(s2[:], ones_col[:], xsq[:],
                                 start=(c == 0), stop=(c == CH - 1))
            t1 = sb2.tile([1, 512], F32, tag="t1", name="t1")
            nc.scalar.activation(t1[:], s1[:], AF.Square)
            vv = sb2.tile([1, 512], F32, tag="vv", name="vv")
            nc.vector.scalar_tensor_tensor(vv[:], s2[:], float(D), t1[:],
                                           op0=OP.mult, op1=OP.subtract)
            stdh = sb2.tile([1, 512], F32, tag="stdh", name="stdh")
            nc.scalar.activation(stdh[:], vv[:], AF.Sqrt,
                                 scale=1.0 / (D * D), bias=eps_col[:])
            a_half = a_row[:, tt * 512:(tt + 1) * 512]
            nc.vector.reciprocal(a_half, stdh[:])
            nc.vector.scalar_tensor_tensor(
                b_row[:, tt * 512:(tt + 1) * 512], s1[:], -1.0 / D, a_half,
                op0=OP.mult, op1=OP.mult)
            nc.gpsimd.partition_broadcast(
                arep[:, tt * 512:(tt + 1) * 512], a_half)
            nc.gpsimd.partition_broadcast(
                brep[:, tt * 512:(tt + 1) * 512],
                b_row[:, tt * 512:(tt + 1) * 512])
        masks.make_identity(nc, ident16[:])
        masks.make_identity(nc, ident8[:])

        # ---- LayerNorm apply: yT = xT * a + b (f16), 512-col pieces so
        # the first projections start after the nt=0 halves ----
        for nt in range(2):
            for c in range(CH):
                eng = nc.gpsimd if (c == 1 and nt == 1) else nc.vector
                xs = xT_sb[:, c * T + nt * 512: c * T + nt * 512 + 512]
                ys = yT_sb[:, c * T + nt * 512: c * T + nt * 512 + 512]
                ar = arep[:, nt * 512:(nt + 1) * 512]
                br = brep[:, nt * 512:(nt + 1) * 512]
                eng.tensor_tensor(ys, xs, ar, op=OP.mult)
                eng.tensor_tensor(ys, ys, br, op=OP.add)

        # pad pT columns
        z8 = sb.tile([128, 4], F8)
        nc.vector.memset(z8[:], 0.0)
        zrow = sb.tile([128, 2], F16)
        nc.vector.memset(zrow[:], 0.0)
        nc.vector.tensor_copy(posT_sb[:, CH * L:], zrow[:])

        # ---- projection emitters ----
        # extraction engines are spread (qc->Act, qp->DVE, k/p->Pool,
        # v->DVE) so consecutive users of the psum ping-pong release in
        # parallel queues.
        def q_proj(p, nt, pool=None, ptag="sc"):
            prj = (pool or ps_sc).tile([128, 512], F32, tag=ptag,
                                       name="prj")
            for ci, c in enumerate((0, 2, 1, 3)):
                nc.tensor.matmul(
                    prj[:],
                    wq_sb[:, c * 256 + p * 128: c * 256 + p * 128 + 128],
                    yT_sb[:, c * T + nt * 512: c * T + nt * 512 + 512],
                    start=(c == 0), stop=(c == CH - 1))
            o = p * T + nt * 512
            nc.scalar.activation(qcT_sb[:, o:o + 512], prj[:],
                                 AF.Identity, bias=qcb_sb[:, p:p + 1])
            nc.scalar.activation(qpT_sb[:, o:o + 512], prj[:],
                                 AF.Identity, bias=qpb_sb[:, p:p + 1])

        def k_proj(p, nt, pool=None, ptag="sc"):
            prjk = (pool or ps_sc).tile([128, 512], F32, tag=ptag,
                                        name="prjk")
            for ci, c in enumerate((0, 2, 1, 3)):
                nc.tensor.matmul(
                    prjk[:],
                    wk_sb[:, c * 256 + p * 128: c * 256 + p * 128 + 128],
                    yT_sb[:, c * T + nt * 512: c * T + nt * 512 + 512],
                    start=(ci == 0), stop=(ci == CH - 1))
            nc.scalar.activation(kT_sb[:, p * T + nt * 512:
                                       p * T + nt * 512 + 512],
                                 prjk[:], AF.Identity,
                                 bias=kb_sb[:, p:p + 1])

        def p_proj(p, nt, pool=None, ptag="sc"):
            pps = (pool or ps_sc).tile([128, 512], F32, tag=ptag,
                                       name="pps")
            for ci, c in enumerate((0, 2, 1, 3)):
                nc.tensor.matmul(
                    pps[:],
                    wp_sb[:, c * 256 + p * 128: c * 256 + p * 128 + 128],
                    posT_sb[:, c * L + nt * 512: c * L + nt * 512 + 512],
                    start=(ci == 0), stop=(ci == CH - 1))
            nc.scalar.copy(
                pT_sb[:, p * PL + nt * 512: p * PL + nt * 512 + 512],
                pps[:])
            if nt == 3:
                nc.gpsimd.tensor_copy(pT_sb[:, p * PL + L: (p + 1) * PL],
                                      z8[:, :PL - L])

        def v_proj(t8, pool=None, ptag="sc"):
            vps = (pool or ps_sc).tile([128, 256], F32, tag=ptag,
                                       name="vps")
            for c in range(CH):
                nc.tensor.matmul(
                    vps[:],
                    yT_sb[:, c * T + t8 * 128: c * T + t8 * 128 + 128],
                    wv_sb[:, c * 256:(c + 1) * 256],
                    start=(c == 0), stop=(c == CH - 1))
            nc.vector.tensor_tensor(
                v_sb[:, t8 * 256:(t8 + 1) * 256], vps[:], vb_sb[:],
                op=OP.add)

        # ---- attention emitters ----
        def hsl(tile_, h, lo, w, stride=T):
            p, off = h // 2, (h % 2) * 64
            return tile_[off:off + 64, p * stride + lo: p * stride + lo + w]

        def band_qb(h, qb, shifted):
            """Band scores for (h, qb), bounce out, and the skewed read of
            this qb's block straight back into `shifted`. Three psum tiles,
            one extraction engine each, so the next band matmul waits on
            exactly one engine per tile."""
            ping = h % 2
            s0 = 897 - qb * 128
            qp_blk = hsl(qpT_sb, h, qb * 128, 128)
            bps0 = ps_b.tile([128, 768], F32, tag="band0")
            nc.tensor.matmul(bps0[:, :512], qp_blk,
                             hsl(pT_sb, h, s0, 512, stride=PL),
                             start=True, stop=True)
            nc.tensor.matmul(bps0[:, 512:], qp_blk,
                             hsl(pT_sb, h, s0 + 512, 256, stride=PL),
                             start=True, stop=True)
            bps1 = ps_b.tile([128, 384], F32, tag="band1")
            nc.tensor.matmul(bps1[:], qp_blk,
                             hsl(pT_sb, h, s0 + 768, 384, stride=PL),
                             start=True, stop=True)
            b8 = sbE.tile([128, BAND], F8, tag="band8", name="b8")
            nc.vector.tensor_copy(b8[:, :768], bps0[:])
            nc.scalar.copy(b8[:, 768:], bps1[:])
            nc.sync.dma_start(bounce[ping, qb], b8[:])
            if qb % 2 == 1:
                src = bass.AP(bounce[:].tensor,
                              (ping * QB + qb - 1) * (128 * BAND) + 127,
                              [[BAND - 1, 128], [128 * BAND, 2], [1, T]])
                nc.sync.dma_start(
                    shifted[:, (qb - 1) * T:(qb + 1) * T], src)

        def shifted_tile():
            shifted = sb2.tile([128, QB * T], F8, tag="shifted",
                               name="shifted")
            return shifted

        def scores_qb(h, qb, shifted):
            """Scores+exp+normalize for (h, qb); returns the E tile.
            Transposes are emitted one qb later (transpose_qb) so the PE
            queue never waits on the Act/DVE chain of the same qb."""
            E_sb = sbE.tile([128, T], F16, tag="E", name="E")
            den = sb3.tile([128, 2], F32, tag="den", name="den")
            for half in range(2):
                sps = ps_sc.tile([128, 512], F32, tag="sc")
                nc.tensor.matmul(
                    sps[:],
                    hsl(qcT_sb, h, qb * 128, 128),
                    hsl(kT_sb, h, half * 512, 512),
                    start=True, stop=False)
                if qb == 0 and half == 1:
                    # scores[0, 1023] += (q+pos_bias)[1] . p[0]
                    # (the reference rel_shift reshape wraps this element)
                    nc.tensor.matmul(
                        sps[0:1, 511:512],
                        hsl(qpT_sb, h, 1, 1),
                        hsl(pT_sb, h, 0, 1, stride=PL),
                        start=False, stop=False)
                nc.tensor.matmul(
                    sps[:],
                    ident8[:],
                    shifted[:, qb * T + half * 512:
                            qb * T + half * 512 + 512],
                    start=False, stop=True)
                nc.scalar.activation(
                    E_sb[:, half * 512:(half + 1) * 512], sps[:], AF.Exp,
                    scale=0.125, accum_out=den[:, half:half + 1])
            rec = sb3.tile([128, 1], F32, tag="rec", name="rec")
            nc.vector.tensor_tensor(rec[:], den[:, 0:1], den[:, 1:2],
                                    op=OP.add)
            nc.vector.reciprocal(rec[:], rec[:])
            nc.vector.tensor_scalar_mul(E_sb[:], E_sb[:], rec[:])
            return E_sb

        def transpose_qb(qb, E_sb, ET_sb):
            # transpose E (f16): 8 PE transposes -> one f16 psum bank
            etps = ps_et.tile([128, T], F16, tag="et")
            for kc in range(QB):
                nc.tensor.transpose(
                    etps[:, kc * 128: kc * 128 + 128],
                    E_sb[:, kc * 128: kc * 128 + 128],
                    ident16[:])
            # scatter-extract: ET_sb[:, kc*T + qb*128 ...] = etps block kc
            dst = bass.AP(ET_sb[:].tensor, qb * 128,
                          [[QB * T, 128], [T, QB], [1, 128]])
            nc.vector.tensor_copy(dst, etps[:])

        def attnv_chunk(p, ib, hh, kh, ET_pair, otps):
            hloc = 2 * p + hh
            for kc in range(4 * kh, 4 * kh + 4):
                nc.tensor.matmul(
                    otps[hh * 64:hh * 64 + 64, :],
                    v_sb[:, kc * 256 + hloc * 64: kc * 256 + hloc * 64 + 64],
                    ET_pair[hh][:, kc * T + ib * 512: kc * T + ib * 512 + 512],
                    start=(kc == 0), stop=(kc == QB - 1))

        def attnv_extract(p, ib, otps):
            dst = oT_sb[:, p * T + ib * 512: p * T + ib * 512 + 512]
            if (p + ib) % 2 == 0:
                nc.vector.tensor_copy(dst, otps[:])
            else:
                nc.scalar.copy(dst, otps[:])

        def out_proj(t8):
            ops_ = ps_sc.tile([128, 512], F32, tag="sc")
            for p in range(NP):
                nc.tensor.matmul(
                    ops_[:],
                    oT_sb[:, p * T + t8 * 128: p * T + t8 * 128 + 128],
                    wo_sb[:, p * D:(p + 1) * D],
                    start=(p == 0), stop=(p == NP - 1))
            osb = sbE.tile([128, 512], F16, tag="osb", name="osb")
            if t8 % 2 == 0:
                nc.vector.tensor_copy(osb[:], ops_[:])
            else:
                nc.scalar.copy(osb[:], ops_[:])
            nc.sync.dma_start(out_d[t8 * 128:(t8 + 1) * 128, :], osb[:])

        # ---- emission schedule ----
        # p proj first (needs no LayerNorm -> fills the stats-chain
        # latency). Only pair-0 projections + head-0 bands run before the
        # attention loop; pair-1 projections and v-proj are deferred as
        # per-q-block filler inside the head-0/1 loops so the in-order PE
        # queue reaches head-0 scores ~12us earlier.
        for nt in range(4):
            p_proj(0, nt)
        q_proj(0, 0)
        q_proj(0, 1)
        shifted = {0: shifted_tile()}
        for qb in range(QB):
            band_qb(0, qb, shifted[0])
        k_proj(0, 0)
        k_proj(0, 1)

        filler = {
            0: [lambda: q_proj(1, 0, ps_o, "o"),
                lambda: q_proj(1, 1, ps_o, "o"),
                lambda: k_proj(1, 0, ps_o, "o"),
                lambda: k_proj(1, 1, ps_o, "o"),
                lambda: p_proj(1, 0, ps_o, "o"),
                lambda: p_proj(1, 1, ps_o, "o"),
                lambda: p_proj(1, 2, ps_o, "o"),
                lambda: p_proj(1, 3, ps_o, "o")],
            1: [(lambda t8=t8: v_proj(t8, ps_o, "o")) for t8 in range(QB)],
        }

        ET_tiles = {}
        attnv_work = []   # deferred attn@V chunks for the previous pair
        for h in range(NH):
            # bufs=3: head h+2 must not wait on the deferred attn@V reads
            # of head h's ET (they interleave into head h+2's score loop)
            ET_sb = sb3.tile([128, QB * T], F16, tag="ET", name="ET")
            ET_tiles[h] = ET_sb
            if h + 1 < NH:
                shifted[h + 1] = shifted_tile()
            E_hist = []
            for qb in range(QB):
                if h + 1 < NH:
                    band_qb(h + 1, qb, shifted[h + 1])
                E_hist.append(scores_qb(h, qb, shifted[h]))
                if qb >= 2:
                    transpose_qb(qb - 2, E_hist[qb - 2], ET_sb)
                for fn_ in filler.get(h, [])[qb:qb + 1]:
                    fn_()
                # slot one deferred attn@V chunk of the previous pair
                if attnv_work:
                    attnv_work.pop(0)()
            transpose_qb(QB - 2, E_hist[QB - 2], ET_sb)
            transpose_qb(QB - 1, E_hist[QB - 1], ET_sb)
            if h % 2 == 1:
                p = h // 2
                ET_pair = (ET_tiles[2 * p], ET_tiles[2 * p + 1])
                chunks = []
                state = {}

                def make_chunk(p_, ib_, hh_, kh_, ET_pair_):
                    def go():
                        if ("ot", p_, ib_) not in state:
                            state[("ot", p_, ib_)] = ps_o.tile(
                                [128, 512], F32, tag="o", name="otps")
                        otps = state[("ot", p_, ib_)]
                        attnv_chunk(p_, ib_, hh_, kh_, ET_pair_, otps)
                        if hh_ == 1 and kh_ == 1:
                            attnv_extract(p_, ib_, otps)
                    return go

                for ib in range(2):
                    for hh in range(2):
                        for kh in range(2):
                            chunks.append(make_chunk(p, ib, hh, kh, ET_pair))
                if h == NH - 1:
                    for cfn in chunks[:4]:
                        cfn()
                    for i, cfn in enumerate(chunks[4:]):
                        out_proj(i)
                        cfn()
                    for t8 in range(4, QB):
                        out_proj(t8)
                else:
                    attnv_work.extend(chunks)

    nc.compile()
    return nc


_PROGRAM_CACHE: dict = {}


def _get_program() -> bass.Bass:
    if "nc" not in _PROGRAM_CACHE:
        _PROGRAM_CACHE["nc"] = _build_program()
    return _PROGRAM_CACHE["nc"]


def _prepare_in_maps(x, pos, content_bias, pos_bias, gamma, beta,
                     Wq, bq, Wk, bk, Wv, bv, Wp, Wo, bo):
    x = np.asarray(x, np.float32)
    pos = np.asarray(pos, np.float32)
    gamma = np.asarray(gamma, np.float32)
    beta = np.asarray(beta, np.float32)

    # gamma folding: y = yln*gamma + beta  =>  y@W = yln@(gamma*W) + beta@W
    def fold(W):
        W = np.asarray(W, np.float32)
        return W * gamma[:, None, None], np.einsum("d,dhk->hk", beta, W)

    Wq_f, bq_f = fold(Wq)
    Wk_f, bk_f = fold(Wk)
    Wv_f, bv_f = fold(Wv)
    Wp = np.asarray(Wp, np.float32)
    Wo = np.asarray(Wo, np.float32)

    in_maps = []
    for core in range(8):
        b = core // 2
        g = core % 2
        hs = slice(4 * g, 4 * g + 4)
        qcb = (np.asarray(bq) + np.asarray(content_bias) + bq_f)[hs]
        qpb = (np.asarray(bq) + np.asarray(pos_bias) + bq_f)[hs]
        kb = (np.asarray(bk) + bk_f)[hs]
        vb = (np.asarray(bv) + bv_f)[hs]
        # Wo pair-stacked: [128, NP*D]; pair p rows = Wo[2p] ++ Wo[2p+1]
        Wo_h = np.asarray(Wo)[hs]          # [4, DK, D]
        wo2 = np.concatenate(
            [np.concatenate([Wo_h[2 * p], Wo_h[2 * p + 1]], axis=0)
             for p in range(NP)], axis=1)  # [128, NP*D]
        in_maps.append({
            "xT": np.ascontiguousarray(x[b].T).astype(np.float16),
            "posT": np.ascontiguousarray(pos[b].T).astype(np.float16),
            "wq": np.ascontiguousarray(
                Wq_f[:, hs, :].reshape(D, NH * DK)).astype(np.float16),
            "wk": np.ascontiguousarray(
                Wk_f[:, hs, :].reshape(D, NH * DK)).astype(np.float16),
            "wv": np.ascontiguousarray(
                Wv_f[:, hs, :].reshape(D, NH * DK)).astype(np.float16),
            "wp": np.ascontiguousarray(
                Wp[:, hs, :].reshape(D, NH * DK)).astype(np.float16),
            "wo": np.ascontiguousarray(wo2).astype(np.float16),
            "qc_bias": np.ascontiguousarray(qcb.reshape(2, 128).T),
            "qp_bias": np.ascontiguousarray(qpb.reshape(2, 128).T),
            "k_bias": np.ascontiguousarray(kb.reshape(2, 128).T),
            "v_bias": np.ascontiguousarray(vb.reshape(NH * DK)),
        })

    return in_maps


def _combine(x, bo, results):
    parts = [r["out_partial"] for r in results]
    out = np.asarray(x, np.float32) + np.asarray(bo, np.float32)[None, None, :]
    for b in range(B):
        out[b] += parts[2 * b].astype(np.float32)
        out[b] += parts[2 * b + 1].astype(np.float32)
    return out.astype(np.float32)


def kernel(x, pos, content_bias, pos_bias, gamma, beta,
           Wq, bq, Wk, bk, Wv, bv, Wp, Wo, bo) -> np.ndarray:
    in_maps = _prepare_in_maps(x, pos, content_bias, pos_bias, gamma, beta,
                               Wq, bq, Wk, bk, Wv, bv, Wp, Wo, bo)
    nc = _get_program()
    res = run_bass_kernel_spmd(nc, in_maps, core_ids=list(range(8)))
    return _combine(x, bo, res.results)


# revision 11
# speedup vs baseline: 165.2248x; 1.0056x over previous
"""Trainium2 Bass kernel v2 for Transformer-XL style MHSA (nn_MHSAModule).

Problem (hardcoded):
  B=4, T=1024, D=512, H=8, DK=64, L=2*T-1=2047, eps=1e-3
  out = x + (MHSA(LayerNorm(x), pos) @ Wo + bo)

Sharding: 8 cores = 4 batches x 2 head-groups (4 heads each). Each core
returns a partial output [T, D] f16 (its heads' contribution); the host
sums the two partials per batch and adds the residual x + bo.

v2 design notes (vs v1):
  - f16 activations/weights on the matmul path; f8e4m3 only for matmul
    B-operands no vector engine reads back: kT, pT, and the rel-shift
    band bounce (halves its DMA volume).
  - exp runs on Act straight from PSUM with accum_out giving the softmax
    denominator for free; E is normalized by one per-partition f16
    tensor_scalar instead of v1's psum-copy + scale chain.
  - PSUM extraction is the scarce resource (~4x an SBUF read): band
    extraction is split across DVE/Act/Pool, scores are extracted by the
    exp itself, ET extracted as f16 (2x cheaper than f32).
  - LayerNorm stats via f16 ones-matmuls; a/b rows reach all partitions
    via gpsimd.partition_broadcast instead of a DRAM round-trip.
  - attention@V stacks head pairs on 128 PSUM partitions; Wo is
    pair-stacked so the output projection contracts 128 rows per step.
  - engines execute in-order, so emission order IS the schedule: x/wq
    load first, the band of head h+1 and the attn@V of the previous pair
    are interleaved into head h's per-q-block score loop.
"""
import numpy as np
from contextlib import ExitStack

import concourse.bass as bass
import concourse.bacc as bacc
import concourse.tile as tile
from concourse import mybir
from concourse import masks
from concourse.bass_utils import run_bass_kernel_spmd

F32 = mybir.dt.float32
F16 = mybir.dt.float16
F8 = mybir.dt.float8e4
AF = mybir.ActivationFunctionType
OP = mybir.AluOpType

B, T, D, H, DK = 4, 1024, 512, 8, 64
L = 2 * T - 1
EPS = 1e-3
NH = 4          # heads per core
NP = 2          # head pairs per core
CH = D // 128   # 4 contraction chunks
QB = T // 128   # 8 q blocks
BAND = 1152    # positional band width per q block
PL = L + 2      # padded pT free size (2 zero pad cols)


def _build_program() -> bass.Bass:
    nc = bacc.Bacc("TRN2", target_bir_lowering=False, debug=False)

    # ---- DRAM I/O ----
    xT = nc.dram_tensor("xT", [D, T], F16, kind="ExternalInput")
    posT = nc.dram_tensor("posT", [D, L], F16, kind="ExternalInput")
    wq = nc.dram_tensor("wq", [D, NH * DK], F16, kind="ExternalInput")
    wk = nc.dram_tensor("wk", [D, NH * DK], F16, kind="ExternalInput")
    wv = nc.dram_tensor("wv", [D, NH * DK], F16, kind="ExternalInput")
    wp = nc.dram_tensor("wp", [D, NH * DK], F16, kind="ExternalInput")
    wo = nc.dram_tensor("wo", [128, NP * D], F16, kind="ExternalInput")
    qc_bias = nc.dram_tensor("qc_bias", [128, NP], F32, kind="ExternalInput")
    qp_bias = nc.dram_tensor("qp_bias", [128, NP], F32, kind="ExternalInput")
    k_bias = nc.dram_tensor("k_bias", [128, NP], F32, kind="ExternalInput")
    v_bias = nc.dram_tensor("v_bias", [NH * DK], F32, kind="ExternalInput")
    out_d = nc.dram_tensor("out_partial", [T, D], F16, kind="ExternalOutput")

    # internal scratch: rel-shift bounce, f8, double buffered
    bounce = nc.dram_tensor("bounce", [2, QB, 128, BAND], F8)

    with tile.TileContext(nc) as tc, ExitStack() as ctx:
        sb = ctx.enter_context(tc.tile_pool(name="sb", bufs=1))
        sb2 = ctx.enter_context(tc.tile_pool(name="sb2", bufs=2))
        sb3 = ctx.enter_context(tc.tile_pool(name="sb3", bufs=3))
        sbE = ctx.enter_context(tc.tile_pool(name="sbE", bufs=6))
        # PSUM: sc 2x[128,512]f32 (2 banks) + band 1x[128,1152]f32
        # (3 banks) + et 2x[128,1024]f16 (2 banks) + o 1x[128,512]f32
        # (1 bank) = 8 banks.
        ps_sc = ctx.enter_context(tc.tile_pool(name="ps_sc", bufs=2,
                                               space="PSUM"))
        # band psum: three independently-released tiles so the next band
        # matmul only waits on the one engine that extracts each slice
        ps_b = ctx.enter_context(tc.tile_pool(name="ps_b", bufs=1,
                                              space="PSUM"))
        ps_et = ctx.enter_context(tc.tile_pool(name="ps_et", bufs=2,
                                               space="PSUM"))
        ps_o = ctx.enter_context(tc.tile_pool(name="ps_o", bufs=1,
                                              space="PSUM"))

        # ---- persistent SBUF ----
        xT_sb = sb.tile([128, CH * T], F16)
        yT_sb = sb.tile([128, CH * T], F16)
        posT_sb = sb.tile([128, CH * L + 2], F16)
        pT_sb = sb.tile([128, NP * PL], F8)
        qcT_sb = sb.tile([128, NP * T], F8)
        qpT_sb = sb.tile([128, NP * T], F8)
        kT_sb = sb.tile([128, NP * T], F8)
        v_sb = sb.tile([128, QB * NH * DK], F16)
        oT_sb = sb.tile([128, NP * T], F16)
        wq_sb = sb.tile([128, CH * 256], F16)
        wk_sb = sb.tile([128, CH * 256], F16)
        wv_sb = sb.tile([128, CH * 256], F16)
        wp_sb = sb.tile([128, CH * 256], F16)
        wo_sb = sb.tile([128, NP * D], F16)
        qcb_sb = sb.tile([128, NP], F32)
        qpb_sb = sb.tile([128, NP], F32)
        kb_sb = sb.tile([128, NP], F32)
        vb_sb = sb.tile([128, 256], F32)
        arep = sb.tile([128, T], F32)
        brep = sb.tile([128, T], F32)
        ident16 = sb.tile([128, 128], F16)
        ident8 = sb.tile([128, 128], F8)
        ones_col = sb.tile([128, 1], F16)
        eps_col = sb.tile([1, 1], F32)

        nc.vector.memset(ones_col[:], 1.0)
        nc.vector.memset(eps_col[:], EPS)

        # ---- loads: x first (stats), then pos+wp (p proj), then q/k ----
        for tt in range(2):
            for c in range(CH):
                nc.sync.dma_start(
                    xT_sb[:, c * T + tt * 512: c * T + tt * 512 + 512],
                    xT[c * 128:(c + 1) * 128, tt * 512:(tt + 1) * 512])
        for c in range(CH):
            nc.sync.dma_start(wp_sb[:, c * 256:(c + 1) * 256],
                              wp[c * 128:(c + 1) * 128, :])
            nc.sync.dma_start(posT_sb[:, c * L:(c + 1) * L],
                              posT[c * 128:(c + 1) * 128, :])
        for c in range(CH):
            nc.sync.dma_start(wq_sb[:, c * 256:(c + 1) * 256],
                              wq[c * 128:(c + 1) * 128, :])
            nc.sync.dma_start(wk_sb[:, c * 256:(c + 1) * 256],
                              wk[c * 128:(c + 1) * 128, :])
        nc.sync.dma_start(qcb_sb[:], qc_bias[:])
        nc.sync.dma_start(qpb_sb[:], qp_bias[:])
        nc.sync.dma_start(kb_sb[:], k_bias[:])
        for c in range(CH):
            nc.sync.dma_start(wv_sb[:, c * 256:(c + 1) * 256],
                              wv[c * 128:(c + 1) * 128, :])
        nc.sync.dma_start(wo_sb[:], wo[:])
        nc.sync.dma_start(
            vb_sb[:], bass.AP(v_bias[:].tensor, 0, [[0, 128], [1, 256]]))

        # ---- LayerNorm stats; short [1,512]-row chain (row ops are
        # lane-serial, so every op counts): a = rsqrt(var+eps),
        # b = -mu*a, computed as
        #   t1 = s1*s1; v' = D*s2 - t1; a = Rsqrt(v'/D^2 + eps);
        #   b = (s1 * -1/D) * a
        a_row = sb.tile([1, T], F32)
        b_row = sb.tile([1, T], F32)
        for tt in range(2):
            s1 = ps_b.tile([1, 512], F32, tag="band0")
            for c in range(CH):
                xt = xT_sb[:, c * T + tt * 512: c * T + tt * 512 + 512]
                nc.tensor.matmul(s1[:], ones_col[:], xt,
                                 start=(c == 0), stop=(c == CH - 1))
            s2 = ps_b.tile([1, 512], F32, tag="band1")
            for ci, c in enumerate((0, 2, 1, 3)):
                xsq = sb3.tile([128, 512], F16, tag="xsq", name="xsq")
                xt = xT_sb[:, c * T + tt * 512: c * T + tt * 512 + 512]
                eng = nc.vector if c in (0, 2) else nc.gpsimd
                eng.tensor_tensor(xsq[:], xt, xt, op=OP.mult)
                nc.tensor.matmul(s2[:], ones_col[:], xsq[:],
                                 start=(ci == 0), stop=(ci == CH - 1))
            t1 = sb2.tile([1, 512], F32, tag="t1", name="t1")
            nc.scalar.activation(t1[:], s1[:], AF.Square)
            vv = sb2.tile([1, 512], F32, tag="vv", name="vv")
            nc.vector.scalar_tensor_tensor(vv[:], s2[:], float(D), t1[:],
                                           op0=OP.mult, op1=OP.subtract)
            stdh = sb2.tile([1, 512], F32, tag="stdh", name="stdh")
            nc.scalar.activation(stdh[:], vv[:], AF.Sqrt,
                                 scale=1.0 / (D * D), bias=eps_col[:])
            a_half = a_row[:, tt * 512:(tt + 1) * 512]
            nc.vector.reciprocal(a_half, stdh[:])
            nc.vector.scalar_tensor_tensor(
                b_row[:, tt * 512:(tt + 1) * 512], s1[:], -1.0 / D, a_half,
                op0=OP.mult, op1=OP.mult)
            nc.gpsimd.partition_broadcast(
                arep[:, tt * 512:(tt + 1) * 512], a_half)
            nc.gpsimd.partition_broadcast(
                brep[:, tt * 512:(tt + 1) * 512],
                b_row[:, tt * 512:(tt + 1) * 512])
        masks.make_identity(nc, ident16[:])
        masks.make_identity(nc, ident8[:])

        # ---- LayerNorm apply: yT = xT * a + b (f16), 512-col pieces so
        # the first projections start after the nt=0 halves ----
        for nt in range(2):
            for c in range(CH):
                eng = nc.gpsimd if (c == 1 and nt == 1) else nc.vector
                xs = xT_sb[:, c * T + nt * 512: c * T + nt * 512 + 512]
                ys = yT_sb[:, c * T + nt * 512: c * T + nt * 512 + 512]
                ar = arep[:, nt * 512:(nt + 1) * 512]
                br = brep[:, nt * 512:(nt + 1) * 512]
                eng.tensor_tensor(ys, xs, ar, op=OP.mult)
                eng.tensor_tensor(ys, ys, br, op=OP.add)

        # pad pT columns
        z8 = sb.tile([128, 4], F8)
        nc.vector.memset(z8[:], 0.0)
        zrow = sb.tile([128, 2], F16)
        nc.vector.memset(zrow[:], 0.0)
        nc.vector.tensor_copy(posT_sb[:, CH * L:], zrow[:])

        # ---- projection emitters ----
        # extraction engines are spread (qc->Act, qp->DVE, k/p->Pool,
        # v->DVE) so consecutive users of the psum ping-pong release in
        # parallel queues.
        def q_proj(p, nt, pool=None, ptag="sc"):
            prj = (pool or ps_sc).tile([128, 512], F32, tag=ptag,
                                       name="prj")
            for ci, c in enumerate((0, 2, 1, 3)):
                nc.tensor.matmul(
                    prj[:],
                    wq_sb[:, c * 256 + p * 128: c * 256 + p * 128 + 128],
                    yT_sb[:, c * T + nt * 512: c * T + nt * 512 + 512],
                    start=(c == 0), stop=(c == CH - 1))
            o = p * T + nt * 512
            nc.scalar.activation(qcT_sb[:, o:o + 512], prj[:],
                                 AF.Identity, bias=qcb_sb[:, p:p + 1])
            nc.scalar.activation(qpT_sb[:, o:o + 512], prj[:],
                                 AF.Identity, bias=qpb_sb[:, p:p + 1])

        def k_proj(p, nt, pool=None, ptag="sc"):
            prjk = (pool or ps_sc).tile([128, 512], F32, tag=ptag,
                                        name="prjk")
            for ci, c in enumerate((0, 2, 1, 3)):
                nc.tensor.matmul(
                    prjk[:],
                    wk_sb[:, c * 256 + p * 128: c * 256 + p * 128 + 128],
                    yT_sb[:, c * T + nt * 512: c * T + nt * 512 + 512],
                    start=(ci == 0), stop=(ci == CH - 1))
            nc.scalar.activation(kT_sb[:, p * T + nt * 512:
                                       p * T + nt * 512 + 512],
                                 prjk[:], AF.Identity,
                                 bias=kb_sb[:, p:p + 1])

        def p_proj(p, nt, pool=None, ptag="sc"):
            pps = (pool or ps_sc).tile([128, 512], F32, tag=ptag,
                                       name="pps")
            for ci, c in enumerate((0, 2, 1, 3)):
                nc.tensor.matmul(
                    pps[:],
                    wp_sb[:, c * 256 + p * 128: c * 256 + p * 128 + 128],
                    posT_sb[:, c * L + nt * 512: c * L + nt * 512 + 512],
                    start=(ci == 0), stop=(ci == CH - 1))
            nc.scalar.copy(
                pT_sb[:, p * PL + nt * 512: p * PL + nt * 512 + 512],
                pps[:])
            if nt == 3:
                nc.gpsimd.tensor_copy(pT_sb[:, p * PL + L: (p + 1) * PL],
                                      z8[:, :PL - L])

        def v_proj(t8, pool=None, ptag="sc"):
            vps = (pool or ps_sc).tile([128, 256], F32, tag=ptag,
                                       name="vps")
            for c in range(CH):
                nc.tensor.matmul(
                    vps[:],
                    yT_sb[:, c * T + t8 * 128: c * T + t8 * 128 + 128],
                    wv_sb[:, c * 256:(c + 1) * 256],
                    start=(c == 0), stop=(c == CH - 1))
            nc.vector.tensor_tensor(
                v_sb[:, t8 * 256:(t8 + 1) * 256], vps[:], vb_sb[:],
                op=OP.add)

        # ---- attention emitters ----
        def hsl(tile_, h, lo, w, stride=T):
            p, off = h // 2, (h % 2) * 64
            return tile_[off:off + 64, p * stride + lo: p * stride + lo + w]

        def band_qb(h, qb, shifted):
            """Band scores for (h, qb), bounce out, and the skewed read of
            this qb's block straight back into `shifted`. Three psum tiles,
            one extraction engine each, so the next band matmul waits on
            exactly one engine per tile."""
            ping = h % 2
            s0 = 897 - qb * 128
            qp_blk = hsl(qpT_sb, h, qb * 128, 128)
            bps0 = ps_b.tile([128, 768], F32, tag="band0")
            nc.tensor.matmul(bps0[:, :512], qp_blk,
                             hsl(pT_sb, h, s0, 512, stride=PL),
                             start=True, stop=True)
            nc.tensor.matmul(bps0[:, 512:], qp_blk,
                             hsl(pT_sb, h, s0 + 512, 256, stride=PL),
                             start=True, stop=True)
            bps1 = ps_b.tile([128, 384], F32, tag="band1")
            nc.tensor.matmul(bps1[:], qp_blk,
                             hsl(pT_sb, h, s0 + 768, 384, stride=PL),
                             start=True, stop=True)
            b8 = sbE.tile([128, BAND], F8, tag="band8", name="b8")
            nc.vector.tensor_copy(b8[:, :768], bps0[:])
            nc.scalar.copy(b8[:, 768:], bps1[:])
            nc.sync.dma_start(bounce[ping, qb], b8[:])
            if qb % 2 == 1:
                src = bass.AP(bounce[:].tensor,
                              (ping * QB + qb - 1) * (128 * BAND) + 127,
                              [[BAND - 1, 128], [128 * BAND, 2], [1, T]])
                nc.sync.dma_start(
                    shifted[:, (qb - 1) * T:(qb + 1) * T], src)

        def shifted_tile():
            shifted = sb2.tile([128, QB * T], F8, tag="shifted",
                               name="shifted")
            return shifted

        def scores_qb(h, qb, shifted):
            """Scores+exp+normalize for (h, qb); returns the E tile.
            Transposes are emitted one qb later (transpose_qb) so the PE
            queue never waits on the Act/DVE chain of the same qb."""
            E_sb = sbE.tile([128, T], F16, tag="E", name="E")
            den = sb3.tile([128, 2], F32, tag="den", name="den")
            for half in range(2):
                sps = ps_sc.tile([128, 512], F32, tag="sc")
                nc.tensor.matmul(
                    sps[:],
                    hsl(qcT_sb, h, qb * 128, 128),
                    hsl(kT_sb, h, half * 512, 512),
                    start=True, stop=False)
                if qb == 0 and half == 1:
                    # scores[0, 1023] += (q+pos_bias)[1] . p[0]
                    # (the reference rel_shift reshape wraps this element)
                    nc.tensor.matmul(
                        sps[0:1, 511:512],
                        hsl(qpT_sb, h, 1, 1),
                        hsl(pT_sb, h, 0, 1, stride=PL),
                        start=False, stop=False)
                nc.tensor.matmul(
                    sps[:],
                    ident8[:],
                    shifted[:, qb * T + half * 512:
                            qb * T + half * 512 + 512],
                    start=False, stop=True)
                nc.scalar.activation(
                    E_sb[:, half * 512:(half + 1) * 512], sps[:], AF.Exp,
                    scale=0.125, accum_out=den[:, half:half + 1])
            rec = sb3.tile([128, 1], F32, tag="rec", name="rec")
            nc.vector.tensor_tensor(rec[:], den[:, 0:1], den[:, 1:2],
                                    op=OP.add)
            nc.vector.reciprocal(rec[:], rec[:])
            nc.vector.tensor_scalar_mul(E_sb[:], E_sb[:], rec[:])
            return E_sb

        def transpose_qb(qb, E_sb, ET_sb):
            # transpose E (f16): 8 PE transposes -> one f16 psum bank
            etps = ps_et.tile([128, T], F16, tag="et")
            for kc in range(QB):
                nc.tensor.transpose(
                    etps[:, kc * 128: kc * 128 + 128],
                    E_sb[:, kc * 128: kc * 128 + 128],
                    ident16[:])
            # scatter-extract: ET_sb[:, kc*T + qb*128 ...] = etps block kc
            dst = bass.AP(ET_sb[:].tensor, qb * 128,
                          [[QB * T, 128], [T, QB], [1, 128]])
            nc.vector.tensor_copy(dst, etps[:])

        def attnv_chunk(p, ib, hh, kh, ET_pair, otps):
            hloc = 2 * p + hh
            for kc in range(4 * kh, 4 * kh + 4):
                nc.tensor.matmul(
                    otps[hh * 64:hh * 64 + 64, :],
                    v_sb[:, kc * 256 + hloc * 64: kc * 256 + hloc * 64 + 64],
                    ET_pair[hh][:, kc * T + ib * 512: kc * T + ib * 512 + 512],
                    start=(kc == 0), stop=(kc == QB - 1))

        def attnv_extract(p, ib, otps):
            dst = oT_sb[:, p * T + ib * 512: p * T + ib * 512 + 512]
            if (p + ib) % 2 == 0:
                nc.vector.tensor_copy(dst, otps[:])
            else:
                nc.scalar.copy(dst, otps[:])

        def out_proj(t8):
            ops_ = ps_sc.tile([128, 512], F32, tag="sc")
            for p in range(NP):
                nc.tensor.matmul(
                    ops_[:],
                    oT_sb[:, p * T + t8 * 128: p * T + t8 * 128 + 128],
                    wo_sb[:, p * D:(p + 1) * D],
                    start=(p == 0), stop=(p == NP - 1))
            osb = sbE.tile([128, 512], F16, tag="osb", name="osb")
            if t8 % 2 == 0:
                nc.vector.tensor_copy(osb[:], ops_[:])
            else:
                nc.scalar.copy(osb[:], ops_[:])
            nc.sync.dma_start(out_d[t8 * 128:(t8 + 1) * 128, :], osb[:])

        # ---- emission schedule ----
        # p proj first (needs no LayerNorm -> fills the stats-chain
        # latency). Only pair-0 projections + head-0 bands run before the
        # attention loop; pair-1 projections and v-proj are deferred as
        # per-q-block filler inside the head-0/1 loops so the in-order PE
        # queue reaches head-0 scores ~12us earlier.
        for nt in range(4):
            p_proj(0, nt)
        q_proj(0, 0)
        q_proj(0, 1)
        shifted = {0: shifted_tile()}
        for qb in range(QB):
            band_qb(0, qb, shifted[0])
        k_proj(0, 0)
        k_proj(0, 1)

        filler = {
            0: [lambda: q_proj(1, 0, ps_o, "o"),
                lambda: q_proj(1, 1, ps_o, "o"),
                lambda: k_proj(1, 0, ps_o, "o"),
                lambda: k_proj(1, 1, ps_o, "o"),
                lambda: p_proj(1, 0, ps_o, "o"),
                lambda: p_proj(1, 1, ps_o, "o"),
                lambda: p_proj(1, 2, ps_o, "o"),
                lambda: p_proj(1, 3, ps_o, "o")],
            1: [(lambda t8=t8: v_proj(t8, ps_o, "o")) for t8 in range(QB)],
        }

        ET_tiles = {}
        attnv_work = []   # deferred attn@V chunks for the previous pair
        for h in range(NH):
            # bufs=3: head h+2 must not wait on the deferred attn@V reads
            # of head h's ET (they interleave into head h+2's score loop)
            ET_sb = sb3.tile([128, QB * T], F16, tag="ET", name="ET")
            ET_tiles[h] = ET_sb
            if h + 1 < NH:
                shifted[h + 1] = shifted_tile()
            E_hist = []
            for qb in range(QB):
                if h + 1 < NH:
                    band_qb(h + 1, qb, shifted[h + 1])
                E_hist.append(scores_qb(h, qb, shifted[h]))
                if qb >= 2:
                    transpose_qb(qb - 2, E_hist[qb - 2], ET_sb)
                for fn_ in filler.get(h, [])[qb:qb + 1]:
                    fn_()
                # slot one deferred attn@V chunk of the previous pair
                # (head 2's first two slots skip so two chunks spill into
                # head 3's otherwise filler-free loop)
                if attnv_work and not (h == 2 and qb < 2):
                    attnv_work.pop(0)()
            transpose_qb(QB - 2, E_hist[QB - 2], ET_sb)
            transpose_qb(QB - 1, E_hist[QB - 1], ET_sb)
            if h % 2 == 1:
                p = h // 2
                ET_pair = (ET_tiles[2 * p], ET_tiles[2 * p + 1])
                chunks = []
                state = {}

                def make_chunk(p_, ib_, hh_, kh_, ET_pair_):
                    def go():
                        if ("ot", p_, ib_) not in state:
                            state[("ot", p_, ib_)] = ps_o.tile(
                                [128, 512], F32, tag="o", name="otps")
                        otps = state[("ot", p_, ib_)]
                        attnv_chunk(p_, ib_, hh_, kh_, ET_pair_, otps)
                        if hh_ == 1 and kh_ == 1:
                            attnv_extract(p_, ib_, otps)
                    return go

                for ib in range(2):
                    for hh in range(2):
                        for kh in range(2):
                            chunks.append(make_chunk(p, ib, hh, kh, ET_pair))
                if h == NH - 1:
                    for cfn in chunks[:4]:
                        cfn()
                    for i, cfn in enumerate(chunks[4:]):
                        out_proj(i)
                        cfn()
                    for t8 in range(4, QB):
                        out_proj(t8)
                else:
                    attnv_work.extend(chunks)

    nc.compile()
    return nc


_PROGRAM_CACHE: dict = {}


def _get_program() -> bass.Bass:
    if "nc" not in _PROGRAM_CACHE:
        _PROGRAM_CACHE["nc"] = _build_program()
    return _PROGRAM_CACHE["nc"]


def _prepare_in_maps(x, pos, content_bias, pos_bias, gamma, beta,
                     Wq, bq, Wk, bk, Wv, bv, Wp, Wo, bo):
    x = np.asarray(x, np.float32)
    pos = np.asarray(pos, np.float32)
    gamma = np.asarray(gamma, np.float32)
    beta = np.asarray(beta, np.float32)

    # gamma folding: y = yln*gamma + beta  =>  y@W = yln@(gamma*W) + beta@W
    def fold(W):
        W = np.asarray(W, np.float32)
        return W * gamma[:, None, None], np.einsum("d,dhk->hk", beta, W)

    Wq_f, bq_f = fold(Wq)
    Wk_f, bk_f = fold(Wk)
    Wv_f, bv_f = fold(Wv)
    Wp = np.asarray(Wp, np.float32)
    Wo = np.asarray(Wo, np.float32)

    in_maps = []
    for core in range(8):
        b = core // 2
        g = core % 2
        hs = slice(4 * g, 4 * g + 4)
        qcb = (np.asarray(bq) + np.asarray(content_bias) + bq_f)[hs]
        qpb = (np.asarray(bq) + np.asarray(pos_bias) + bq_f)[hs]
        kb = (np.asarray(bk) + bk_f)[hs]
        vb = (np.asarray(bv) + bv_f)[hs]
        # Wo pair-stacked: [128, NP*D]; pair p rows = Wo[2p] ++ Wo[2p+1]
        Wo_h = np.asarray(Wo)[hs]          # [4, DK, D]
        wo2 = np.concatenate(
            [np.concatenate([Wo_h[2 * p], Wo_h[2 * p + 1]], axis=0)
             for p in range(NP)], axis=1)  # [128, NP*D]
        in_maps.append({
            "xT": np.ascontiguousarray(x[b].T).astype(np.float16),
            "posT": np.ascontiguousarray(pos[b].T).astype(np.float16),
            "wq": np.ascontiguousarray(
                Wq_f[:, hs, :].reshape(D, NH * DK)).astype(np.float16),
            "wk": np.ascontiguousarray(
                Wk_f[:, hs, :].reshape(D, NH * DK)).astype(np.float16),
            "wv": np.ascontiguousarray(
                Wv_f[:, hs, :].reshape(D, NH * DK)).astype(np.float16),
            "wp": np.ascontiguousarray(
                Wp[:, hs, :].reshape(D, NH * DK)).astype(np.float16),
            "wo": np.ascontiguousarray(wo2).astype(np.float16),
            "qc_bias": np.ascontiguousarray(qcb.reshape(2, 128).T),
            "qp_bias": np.ascontiguousarray(qpb.reshape(2, 128).T),
            "k_bias": np.ascontiguousarray(kb.reshape(2, 128).T),
            "v_bias": np.ascontiguousarray(vb.reshape(NH * DK)),
        })

    return in_maps


def _combine(x, bo, results):
    parts = [r["out_partial"] for r in results]
    out = np.asarray(x, np.float32) + np.asarray(bo, np.float32)[None, None, :]
    for b in range(B):
        out[b] += parts[2 * b].astype(np.float32)
        out[b] += parts[2 * b + 1].astype(np.float32)
    return out.astype(np.float32)


def kernel(x, pos, content_bias, pos_bias, gamma, beta,
           Wq, bq, Wk, bk, Wv, bv, Wp, Wo, bo) -> np.ndarray:
    in_maps = _prepare_in_maps(x, pos, content_bias, pos_bias, gamma, beta,
                               Wq, bq, Wk, bk, Wv, bv, Wp, Wo, bo)
    nc = _get_program()
    res = run_bass_kernel_spmd(nc, in_maps, core_ids=list(range(8)))
    return _combine(x, bo, res.results)


# revision 12
# speedup vs baseline: 165.6303x; 1.0025x over previous
"""Trainium2 Bass kernel v2 for Transformer-XL style MHSA (nn_MHSAModule).

Problem (hardcoded):
  B=4, T=1024, D=512, H=8, DK=64, L=2*T-1=2047, eps=1e-3
  out = x + (MHSA(LayerNorm(x), pos) @ Wo + bo)

Sharding: 8 cores = 4 batches x 2 head-groups (4 heads each). Each core
returns a partial output [T, D] f16 (its heads' contribution); the host
sums the two partials per batch and adds the residual x + bo.

v2 design notes (vs v1):
  - f16 activations/weights on the matmul path; f8e4m3 only for matmul
    B-operands no vector engine reads back: kT, pT, and the rel-shift
    band bounce (halves its DMA volume).
  - exp runs on Act straight from PSUM with accum_out giving the softmax
    denominator for free; E is normalized by one per-partition f16
    tensor_scalar instead of v1's psum-copy + scale chain.
  - PSUM extraction is the scarce resource (~4x an SBUF read): band
    extraction is split across DVE/Act/Pool, scores are extracted by the
    exp itself, ET extracted as f16 (2x cheaper than f32).
  - LayerNorm stats via f16 ones-matmuls; a/b rows reach all partitions
    via gpsimd.partition_broadcast instead of a DRAM round-trip.
  - attention@V stacks head pairs on 128 PSUM partitions; Wo is
    pair-stacked so the output projection contracts 128 rows per step.
  - engines execute in-order, so emission order IS the schedule: x/wq
    load first, the band of head h+1 and the attn@V of the previous pair
    are interleaved into head h's per-q-block score loop.
"""
import numpy as np
from contextlib import ExitStack

import concourse.bass as bass
import concourse.bacc as bacc
import concourse.tile as tile
from concourse import mybir
from concourse import masks
from concourse.bass_utils import run_bass_kernel_spmd

F32 = mybir.dt.float32
F16 = mybir.dt.float16
F8 = mybir.dt.float8e4
AF = mybir.ActivationFunctionType
OP = mybir.AluOpType

B, T, D, H, DK = 4, 1024, 512, 8, 64
L = 2 * T - 1
EPS = 1e-3
NH = 4          # heads per core
NP = 2          # head pairs per core
CH = D // 128   # 4 contraction chunks
QB = T // 128   # 8 q blocks
BAND = 1152    # positional band width per q block
PL = L + 2      # padded pT free size (2 zero pad cols)


def _build_program() -> bass.Bass:
    nc = bacc.Bacc("TRN2", target_bir_lowering=False, debug=False)

    # ---- DRAM I/O ----
    xT = nc.dram_tensor("xT", [D, T], F16, kind="ExternalInput")
    posT = nc.dram_tensor("posT", [D, L], F16, kind="ExternalInput")
    wq = nc.dram_tensor("wq", [D, NH * DK], F16, kind="ExternalInput")
    wk = nc.dram_tensor("wk", [D, NH * DK], F16, kind="ExternalInput")
    wv = nc.dram_tensor("wv", [D, NH * DK], F16, kind="ExternalInput")
    wp = nc.dram_tensor("wp", [D, NH * DK], F16, kind="ExternalInput")
    wo = nc.dram_tensor("wo", [128, NP * D], F16, kind="ExternalInput")
    qc_bias = nc.dram_tensor("qc_bias", [128, NP], F32, kind="ExternalInput")
    qp_bias = nc.dram_tensor("qp_bias", [128, NP], F32, kind="ExternalInput")
    k_bias = nc.dram_tensor("k_bias", [128, NP], F32, kind="ExternalInput")
    v_bias = nc.dram_tensor("v_bias", [NH * DK], F32, kind="ExternalInput")
    out_d = nc.dram_tensor("out_partial", [T, D], F16, kind="ExternalOutput")

    # internal scratch: rel-shift bounce, f8, double buffered
    bounce = nc.dram_tensor("bounce", [2, QB, 128, BAND], F8)

    with tile.TileContext(nc) as tc, ExitStack() as ctx:
        sb = ctx.enter_context(tc.tile_pool(name="sb", bufs=1))
        sb2 = ctx.enter_context(tc.tile_pool(name="sb2", bufs=2))
        sb3 = ctx.enter_context(tc.tile_pool(name="sb3", bufs=3))
        sbE = ctx.enter_context(tc.tile_pool(name="sbE", bufs=6))
        # PSUM: sc 2x[128,512]f32 (2 banks) + band 1x[128,1152]f32
        # (3 banks) + et 2x[128,1024]f16 (2 banks) + o 1x[128,512]f32
        # (1 bank) = 8 banks.
        ps_sc = ctx.enter_context(tc.tile_pool(name="ps_sc", bufs=2,
                                               space="PSUM"))
        # band psum: three independently-released tiles so the next band
        # matmul only waits on the one engine that extracts each slice
        ps_b = ctx.enter_context(tc.tile_pool(name="ps_b", bufs=1,
                                              space="PSUM"))
        ps_et = ctx.enter_context(tc.tile_pool(name="ps_et", bufs=2,
                                               space="PSUM"))
        ps_o = ctx.enter_context(tc.tile_pool(name="ps_o", bufs=1,
                                              space="PSUM"))

        # ---- persistent SBUF ----
        xT_sb = sb.tile([128, CH * T], F16)
        yT_sb = sb.tile([128, CH * T], F16)
        posT_sb = sb.tile([128, CH * L + 2], F16)
        pT_sb = sb.tile([128, NP * PL], F8)
        qcT_sb = sb.tile([128, NP * T], F8)
        qpT_sb = sb.tile([128, NP * T], F8)
        kT_sb = sb.tile([128, NP * T], F8)
        v_sb = sb.tile([128, QB * NH * DK], F16)
        oT_sb = sb.tile([128, NP * T], F16)
        wq_sb = sb.tile([128, CH * 256], F16)
        wk_sb = sb.tile([128, CH * 256], F16)
        wv_sb = sb.tile([128, CH * 256], F16)
        wp_sb = sb.tile([128, CH * 256], F16)
        wo_sb = sb.tile([128, NP * D], F16)
        qcb_sb = sb.tile([128, NP], F32)
        qpb_sb = sb.tile([128, NP], F32)
        kb_sb = sb.tile([128, NP], F32)
        vb_sb = sb.tile([128, 256], F32)
        arep = sb.tile([128, T], F32)
        brep = sb.tile([128, T], F32)
        ident16 = sb.tile([128, 128], F16)
        ident8 = sb.tile([128, 128], F8)
        ones_col = sb.tile([128, 1], F16)
        eps_col = sb.tile([1, 1], F32)

        nc.vector.memset(ones_col[:], 1.0)
        nc.vector.memset(eps_col[:], EPS)

        # ---- loads: x first (stats), then pos+wp (p proj), then q/k ----
        for tt in range(2):
            for c in range(CH):
                nc.sync.dma_start(
                    xT_sb[:, c * T + tt * 512: c * T + tt * 512 + 512],
                    xT[c * 128:(c + 1) * 128, tt * 512:(tt + 1) * 512])
        for c in range(CH):
            nc.sync.dma_start(wp_sb[:, c * 256:(c + 1) * 256],
                              wp[c * 128:(c + 1) * 128, :])
            nc.sync.dma_start(posT_sb[:, c * L:(c + 1) * L],
                              posT[c * 128:(c + 1) * 128, :])
        for c in range(CH):
            nc.sync.dma_start(wq_sb[:, c * 256:(c + 1) * 256],
                              wq[c * 128:(c + 1) * 128, :])
            nc.sync.dma_start(wk_sb[:, c * 256:(c + 1) * 256],
                              wk[c * 128:(c + 1) * 128, :])
        nc.sync.dma_start(qcb_sb[:], qc_bias[:])
        nc.sync.dma_start(qpb_sb[:], qp_bias[:])
        nc.sync.dma_start(kb_sb[:], k_bias[:])
        for c in range(CH):
            nc.sync.dma_start(wv_sb[:, c * 256:(c + 1) * 256],
                              wv[c * 128:(c + 1) * 128, :])
        nc.sync.dma_start(wo_sb[:], wo[:])
        nc.sync.dma_start(
            vb_sb[:], bass.AP(v_bias[:].tensor, 0, [[0, 128], [1, 256]]))

        # ---- LayerNorm stats; short [1,512]-row chain (row ops are
        # lane-serial, so every op counts): a = rsqrt(var+eps),
        # b = -mu*a, computed as
        #   t1 = s1*s1; v' = D*s2 - t1; a = Rsqrt(v'/D^2 + eps);
        #   b = (s1 * -1/D) * a
        a_row = sb.tile([1, T], F32)
        b_row = sb.tile([1, T], F32)
        for tt in range(2):
            s1 = ps_b.tile([1, 512], F32, tag="band0")
            for c in range(CH):
                xt = xT_sb[:, c * T + tt * 512: c * T + tt * 512 + 512]
                nc.tensor.matmul(s1[:], ones_col[:], xt,
                                 start=(c == 0), stop=(c == CH - 1))
            s2 = ps_b.tile([1, 512], F32, tag="band1")
            for ci, c in enumerate((0, 2, 1, 3)):
                xsq = sb3.tile([128, 512], F16, tag="xsq", name="xsq")
                xt = xT_sb[:, c * T + tt * 512: c * T + tt * 512 + 512]
                eng = nc.vector if c in (0, 2) else nc.gpsimd
                eng.tensor_tensor(xsq[:], xt, xt, op=OP.mult)
                nc.tensor.matmul(s2[:], ones_col[:], xsq[:],
                                 start=(ci == 0), stop=(ci == CH - 1))
            t1 = sb2.tile([1, 512], F32, tag="t1", name="t1")
            nc.scalar.activation(t1[:], s1[:], AF.Square)
            vv = sb2.tile([1, 512], F32, tag="vv", name="vv")
            nc.vector.scalar_tensor_tensor(vv[:], s2[:], float(D), t1[:],
                                           op0=OP.mult, op1=OP.subtract)
            stdh = sb2.tile([1, 512], F32, tag="stdh", name="stdh")
            nc.scalar.activation(stdh[:], vv[:], AF.Sqrt,
                                 scale=1.0 / (D * D), bias=eps_col[:])
            a_half = a_row[:, tt * 512:(tt + 1) * 512]
            nc.vector.reciprocal(a_half, stdh[:])
            nc.vector.scalar_tensor_tensor(
                b_row[:, tt * 512:(tt + 1) * 512], s1[:], -1.0 / D, a_half,
                op0=OP.mult, op1=OP.mult)
            nc.gpsimd.partition_broadcast(
                arep[:, tt * 512:(tt + 1) * 512], a_half)
            nc.gpsimd.partition_broadcast(
                brep[:, tt * 512:(tt + 1) * 512],
                b_row[:, tt * 512:(tt + 1) * 512])
        masks.make_identity(nc, ident16[:])
        masks.make_identity(nc, ident8[:])

        # ---- LayerNorm apply: yT = xT * a + b (f16), 512-col pieces so
        # the first projections start after the nt=0 halves ----
        for nt in range(2):
            for c in range(CH):
                eng = nc.gpsimd if (c == 1 and nt == 1) else nc.vector
                xs = xT_sb[:, c * T + nt * 512: c * T + nt * 512 + 512]
                ys = yT_sb[:, c * T + nt * 512: c * T + nt * 512 + 512]
                ar = arep[:, nt * 512:(nt + 1) * 512]
                br = brep[:, nt * 512:(nt + 1) * 512]
                eng.tensor_tensor(ys, xs, ar, op=OP.mult)
                eng.tensor_tensor(ys, ys, br, op=OP.add)

        # pad pT columns
        z8 = sb.tile([128, 4], F8)
        nc.vector.memset(z8[:], 0.0)
        zrow = sb.tile([128, 2], F16)
        nc.vector.memset(zrow[:], 0.0)
        nc.vector.tensor_copy(posT_sb[:, CH * L:], zrow[:])

        # ---- projection emitters ----
        # extraction engines are spread (qc->Act, qp->DVE, k/p->Pool,
        # v->DVE) so consecutive users of the psum ping-pong release in
        # parallel queues.
        def q_proj(p, nt, pool=None, ptag="sc"):
            prj = (pool or ps_sc).tile([128, 512], F32, tag=ptag,
                                       name="prj")
            for ci, c in enumerate((0, 2, 1, 3)):
                nc.tensor.matmul(
                    prj[:],
                    wq_sb[:, c * 256 + p * 128: c * 256 + p * 128 + 128],
                    yT_sb[:, c * T + nt * 512: c * T + nt * 512 + 512],
                    start=(c == 0), stop=(c == CH - 1))
            o = p * T + nt * 512
            nc.scalar.activation(qcT_sb[:, o:o + 512], prj[:],
                                 AF.Identity, bias=qcb_sb[:, p:p + 1])
            nc.scalar.activation(qpT_sb[:, o:o + 512], prj[:],
                                 AF.Identity, bias=qpb_sb[:, p:p + 1])

        def k_proj(p, nt, pool=None, ptag="sc"):
            prjk = (pool or ps_sc).tile([128, 512], F32, tag=ptag,
                                        name="prjk")
            for ci, c in enumerate((0, 2, 1, 3)):
                nc.tensor.matmul(
                    prjk[:],
                    wk_sb[:, c * 256 + p * 128: c * 256 + p * 128 + 128],
                    yT_sb[:, c * T + nt * 512: c * T + nt * 512 + 512],
                    start=(ci == 0), stop=(ci == CH - 1))
            nc.scalar.activation(kT_sb[:, p * T + nt * 512:
                                       p * T + nt * 512 + 512],
                                 prjk[:], AF.Identity,
                                 bias=kb_sb[:, p:p + 1])

        def p_proj(p, nt, pool=None, ptag="sc"):
            pps = (pool or ps_sc).tile([128, 512], F32, tag=ptag,
                                       name="pps")
            for ci, c in enumerate((0, 2, 1, 3)):
                nc.tensor.matmul(
                    pps[:],
                    wp_sb[:, c * 256 + p * 128: c * 256 + p * 128 + 128],
                    posT_sb[:, c * L + nt * 512: c * L + nt * 512 + 512],
                    start=(ci == 0), stop=(ci == CH - 1))
            nc.scalar.copy(
                pT_sb[:, p * PL + nt * 512: p * PL + nt * 512 + 512],
                pps[:])
            if nt == 3:
                nc.gpsimd.tensor_copy(pT_sb[:, p * PL + L: (p + 1) * PL],
                                      z8[:, :PL - L])

        def v_proj(t8, pool=None, ptag="sc"):
            vps = (pool or ps_sc).tile([128, 256], F32, tag=ptag,
                                       name="vps")
            for c in range(CH):
                nc.tensor.matmul(
                    vps[:],
                    yT_sb[:, c * T + t8 * 128: c * T + t8 * 128 + 128],
                    wv_sb[:, c * 256:(c + 1) * 256],
                    start=(c == 0), stop=(c == CH - 1))
            nc.vector.tensor_tensor(
                v_sb[:, t8 * 256:(t8 + 1) * 256], vps[:], vb_sb[:],
                op=OP.add)

        # ---- attention emitters ----
        def hsl(tile_, h, lo, w, stride=T):
            p, off = h // 2, (h % 2) * 64
            return tile_[off:off + 64, p * stride + lo: p * stride + lo + w]

        def band_qb(h, qb, shifted):
            """Band scores for (h, qb), bounce out, and the skewed read of
            this qb's block straight back into `shifted`. Three psum tiles,
            one extraction engine each, so the next band matmul waits on
            exactly one engine per tile."""
            ping = h % 2
            s0 = 897 - qb * 128
            qp_blk = hsl(qpT_sb, h, qb * 128, 128)
            bps0 = ps_b.tile([128, 832], F32, tag="band0")
            nc.tensor.matmul(bps0[:, :512], qp_blk,
                             hsl(pT_sb, h, s0, 512, stride=PL),
                             start=True, stop=True)
            nc.tensor.matmul(bps0[:, 512:], qp_blk,
                             hsl(pT_sb, h, s0 + 512, 320, stride=PL),
                             start=True, stop=True)
            bps1 = ps_b.tile([128, 320], F32, tag="band1")
            nc.tensor.matmul(bps1[:], qp_blk,
                             hsl(pT_sb, h, s0 + 832, 320, stride=PL),
                             start=True, stop=True)
            b8 = sbE.tile([128, BAND], F8, tag="band8", name="b8")
            nc.vector.tensor_copy(b8[:, :832], bps0[:])
            nc.scalar.copy(b8[:, 832:], bps1[:])
            nc.sync.dma_start(bounce[ping, qb], b8[:])
            if qb % 2 == 1:
                src = bass.AP(bounce[:].tensor,
                              (ping * QB + qb - 1) * (128 * BAND) + 127,
                              [[BAND - 1, 128], [128 * BAND, 2], [1, T]])
                nc.sync.dma_start(
                    shifted[:, (qb - 1) * T:(qb + 1) * T], src)

        def shifted_tile():
            shifted = sb2.tile([128, QB * T], F8, tag="shifted",
                               name="shifted")
            return shifted

        def scores_qb(h, qb, shifted):
            """Scores+exp+normalize for (h, qb); returns the E tile.
            Transposes are emitted one qb later (transpose_qb) so the PE
            queue never waits on the Act/DVE chain of the same qb."""
            E_sb = sbE.tile([128, T], F16, tag="E", name="E")
            den = sb3.tile([128, 2], F32, tag="den", name="den")
            for half in range(2):
                sps = ps_sc.tile([128, 512], F32, tag="sc")
                nc.tensor.matmul(
                    sps[:],
                    hsl(qcT_sb, h, qb * 128, 128),
                    hsl(kT_sb, h, half * 512, 512),
                    start=True, stop=False)
                if qb == 0 and half == 1:
                    # scores[0, 1023] += (q+pos_bias)[1] . p[0]
                    # (the reference rel_shift reshape wraps this element)
                    nc.tensor.matmul(
                        sps[0:1, 511:512],
                        hsl(qpT_sb, h, 1, 1),
                        hsl(pT_sb, h, 0, 1, stride=PL),
                        start=False, stop=False)
                nc.tensor.matmul(
                    sps[:],
                    ident8[:],
                    shifted[:, qb * T + half * 512:
                            qb * T + half * 512 + 512],
                    start=False, stop=True)
                nc.scalar.activation(
                    E_sb[:, half * 512:(half + 1) * 512], sps[:], AF.Exp,
                    scale=0.125, accum_out=den[:, half:half + 1])
            rec = sb3.tile([128, 1], F32, tag="rec", name="rec")
            nc.vector.tensor_tensor(rec[:], den[:, 0:1], den[:, 1:2],
                                    op=OP.add)
            nc.vector.reciprocal(rec[:], rec[:])
            nc.vector.tensor_scalar_mul(E_sb[:], E_sb[:], rec[:])
            return E_sb

        def transpose_qb(qb, E_sb, ET_sb):
            # transpose E (f16): 8 PE transposes -> one f16 psum bank
            etps = ps_et.tile([128, T], F16, tag="et")
            for kc in range(QB):
                nc.tensor.transpose(
                    etps[:, kc * 128: kc * 128 + 128],
                    E_sb[:, kc * 128: kc * 128 + 128],
                    ident16[:])
            # scatter-extract: ET_sb[:, kc*T + qb*128 ...] = etps block kc
            dst = bass.AP(ET_sb[:].tensor, qb * 128,
                          [[QB * T, 128], [T, QB], [1, 128]])
            nc.vector.tensor_copy(dst, etps[:])

        def attnv_chunk(p, ib, hh, kh, ET_pair, otps):
            hloc = 2 * p + hh
            for kc in range(4 * kh, 4 * kh + 4):
                nc.tensor.matmul(
                    otps[hh * 64:hh * 64 + 64, :],
                    v_sb[:, kc * 256 + hloc * 64: kc * 256 + hloc * 64 + 64],
                    ET_pair[hh][:, kc * T + ib * 512: kc * T + ib * 512 + 512],
                    start=(kc == 0), stop=(kc == QB - 1))

        def attnv_extract(p, ib, otps):
            dst = oT_sb[:, p * T + ib * 512: p * T + ib * 512 + 512]
            if (p + ib) % 2 == 0:
                nc.vector.tensor_copy(dst, otps[:])
            else:
                nc.scalar.copy(dst, otps[:])

        def out_proj(t8):
            ops_ = ps_sc.tile([128, 512], F32, tag="sc")
            for p in range(NP):
                nc.tensor.matmul(
                    ops_[:],
                    oT_sb[:, p * T + t8 * 128: p * T + t8 * 128 + 128],
                    wo_sb[:, p * D:(p + 1) * D],
                    start=(p == 0), stop=(p == NP - 1))
            osb = sbE.tile([128, 512], F16, tag="osb", name="osb")
            if t8 % 2 == 0:
                nc.vector.tensor_copy(osb[:], ops_[:])
            else:
                nc.scalar.copy(osb[:], ops_[:])
            nc.sync.dma_start(out_d[t8 * 128:(t8 + 1) * 128, :], osb[:])

        # ---- emission schedule ----
        # p proj first (needs no LayerNorm -> fills the stats-chain
        # latency). Only pair-0 projections + head-0 bands run before the
        # attention loop; pair-1 projections and v-proj are deferred as
        # per-q-block filler inside the head-0/1 loops so the in-order PE
        # queue reaches head-0 scores ~12us earlier.
        for nt in range(4):
            p_proj(0, nt)
        q_proj(0, 0)
        q_proj(0, 1)
        shifted = {0: shifted_tile()}
        for qb in range(QB):
            band_qb(0, qb, shifted[0])
        k_proj(0, 0)
        k_proj(0, 1)

        filler = {
            0: [lambda: q_proj(1, 0, ps_o, "o"),
                lambda: q_proj(1, 1, ps_o, "o"),
                lambda: k_proj(1, 0, ps_o, "o"),
                lambda: k_proj(1, 1, ps_o, "o"),
                lambda: p_proj(1, 0, ps_o, "o"),
                lambda: p_proj(1, 1, ps_o, "o"),
                lambda: p_proj(1, 2, ps_o, "o"),
                lambda: p_proj(1, 3, ps_o, "o")],
            1: [(lambda t8=t8: v_proj(t8, ps_o, "o")) for t8 in range(QB)],
        }

        ET_tiles = {}
        attnv_work = []   # deferred attn@V chunks for the previous pair
        for h in range(NH):
            # bufs=3: head h+2 must not wait on the deferred attn@V reads
            # of head h's ET (they interleave into head h+2's score loop)
            ET_sb = sb3.tile([128, QB * T], F16, tag="ET", name="ET")
            ET_tiles[h] = ET_sb
            if h + 1 < NH:
                shifted[h + 1] = shifted_tile()
            E_hist = []
            for qb in range(QB):
                if h + 1 < NH:
                    band_qb(h + 1, qb, shifted[h + 1])
                E_hist.append(scores_qb(h, qb, shifted[h]))
                if qb >= 2:
                    transpose_qb(qb - 2, E_hist[qb - 2], ET_sb)
                for fn_ in filler.get(h, [])[qb:qb + 1]:
                    fn_()
                # slot one deferred attn@V chunk of the previous pair
                # (head 2's first two slots skip so two chunks spill into
                # head 3's otherwise filler-free loop)
                if attnv_work and not (h == 2 and qb < 2):
                    attnv_work.pop(0)()
            transpose_qb(QB - 2, E_hist[QB - 2], ET_sb)
            transpose_qb(QB - 1, E_hist[QB - 1], ET_sb)
            if h % 2 == 1:
                p = h // 2
                ET_pair = (ET_tiles[2 * p], ET_tiles[2 * p + 1])
                chunks = []
                state = {}

                def make_chunk(p_, ib_, hh_, kh_, ET_pair_):
                    def go():
                        if ("ot", p_, ib_) not in state:
                            state[("ot", p_, ib_)] = ps_o.tile(
                                [128, 512], F32, tag="o", name="otps")
                        otps = state[("ot", p_, ib_)]
                        attnv_chunk(p_, ib_, hh_, kh_, ET_pair_, otps)
                        if hh_ == 1 and kh_ == 1:
                            attnv_extract(p_, ib_, otps)
                    return go

                for ib in range(2):
                    for hh in range(2):
                        for kh in range(2):
                            chunks.append(make_chunk(p, ib, hh, kh, ET_pair))
                if h == NH - 1:
                    for cfn in chunks[:4]:
                        cfn()
                    for i, cfn in enumerate(chunks[4:]):
                        out_proj(i)
                        cfn()
                    for t8 in range(4, QB):
                        out_proj(t8)
                else:
                    attnv_work.extend(chunks)

    nc.compile()
    return nc


_PROGRAM_CACHE: dict = {}


def _get_program() -> bass.Bass:
    if "nc" not in _PROGRAM_CACHE:
        _PROGRAM_CACHE["nc"] = _build_program()
    return _PROGRAM_CACHE["nc"]


def _prepare_in_maps(x, pos, content_bias, pos_bias, gamma, beta,
                     Wq, bq, Wk, bk, Wv, bv, Wp, Wo, bo):
    x = np.asarray(x, np.float32)
    pos = np.asarray(pos, np.float32)
    gamma = np.asarray(gamma, np.float32)
    beta = np.asarray(beta, np.float32)

    # gamma folding: y = yln*gamma + beta  =>  y@W = yln@(gamma*W) + beta@W
    def fold(W):
        W = np.asarray(W, np.float32)
        return W * gamma[:, None, None], np.einsum("d,dhk->hk", beta, W)

    Wq_f, bq_f = fold(Wq)
    Wk_f, bk_f = fold(Wk)
    Wv_f, bv_f = fold(Wv)
    Wp = np.asarray(Wp, np.float32)
    Wo = np.asarray(Wo, np.float32)

    in_maps = []
    for core in range(8):
        b = core // 2
        g = core % 2
        hs = slice(4 * g, 4 * g + 4)
        qcb = (np.asarray(bq) + np.asarray(content_bias) + bq_f)[hs]
        qpb = (np.asarray(bq) + np.asarray(pos_bias) + bq_f)[hs]
        kb = (np.asarray(bk) + bk_f)[hs]
        vb = (np.asarray(bv) + bv_f)[hs]
        # Wo pair-stacked: [128, NP*D]; pair p rows = Wo[2p] ++ Wo[2p+1]
        Wo_h = np.asarray(Wo)[hs]          # [4, DK, D]
        wo2 = np.concatenate(
            [np.concatenate([Wo_h[2 * p], Wo_h[2 * p + 1]], axis=0)
             for p in range(NP)], axis=1)  # [128, NP*D]
        in_maps.append({
            "xT": np.ascontiguousarray(x[b].T).astype(np.float16),
            "posT": np.ascontiguousarray(pos[b].T).astype(np.float16),
            "wq": np.ascontiguousarray(
                Wq_f[:, hs, :].reshape(D, NH * DK)).astype(np.float16),
            "wk": np.ascontiguousarray(
                Wk_f[:, hs, :].reshape(D, NH * DK)).astype(np.float16),
            "wv": np.ascontiguousarray(
                Wv_f[:, hs, :].reshape(D, NH * DK)).astype(np.float16),
            "wp": np.ascontiguousarray(
                Wp[:, hs, :].reshape(D, NH * DK)).astype(np.float16),
            "wo": np.ascontiguousarray(wo2).astype(np.float16),
            "qc_bias": np.ascontiguousarray(qcb.reshape(2, 128).T),
            "qp_bias": np.ascontiguousarray(qpb.reshape(2, 128).T),
            "k_bias": np.ascontiguousarray(kb.reshape(2, 128).T),
            "v_bias": np.ascontiguousarray(vb.reshape(NH * DK)),
        })

    return in_maps


def _combine(x, bo, results):
    parts = [r["out_partial"] for r in results]
    out = np.asarray(x, np.float32) + np.asarray(bo, np.float32)[None, None, :]
    for b in range(B):
        out[b] += parts[2 * b].astype(np.float32)
        out[b] += parts[2 * b + 1].astype(np.float32)
    return out.astype(np.float32)


def kernel(x, pos, content_bias, pos_bias, gamma, beta,
           Wq, bq, Wk, bk, Wv, bv, Wp, Wo, bo) -> np.ndarray:
    in_maps = _prepare_in_maps(x, pos, content_bias, pos_bias, gamma, beta,
                               Wq, bq, Wk, bk, Wv, bv, Wp, Wo, bo)
    nc = _get_program()
    res = run_bass_kernel_spmd(nc, in_maps, core_ids=list(range(8)))
    return _combine(x, bo, res.results)


# revision 13
# speedup vs baseline: 166.4072x; 1.0047x over previous
"""Trainium2 Bass kernel v2 for Transformer-XL style MHSA (nn_MHSAModule).

Problem (hardcoded):
  B=4, T=1024, D=512, H=8, DK=64, L=2*T-1=2047, eps=1e-3
  out = x + (MHSA(LayerNorm(x), pos) @ Wo + bo)

Sharding: 8 cores = 4 batches x 2 head-groups (4 heads each). Each core
returns a partial output [T, D] f16 (its heads' contribution); the host
sums the two partials per batch and adds the residual x + bo.

v2 design notes (vs v1):
  - f16 activations/weights on the matmul path; f8e4m3 only for matmul
    B-operands no vector engine reads back: kT, pT, and the rel-shift
    band bounce (halves its DMA volume).
  - exp runs on Act straight from PSUM with accum_out giving the softmax
    denominator for free; E is normalized by one per-partition f16
    tensor_scalar instead of v1's psum-copy + scale chain.
  - PSUM extraction is the scarce resource (~4x an SBUF read): band
    extraction is split across DVE/Act/Pool, scores are extracted by the
    exp itself, ET extracted as f16 (2x cheaper than f32).
  - LayerNorm stats via f16 ones-matmuls; a/b rows reach all partitions
    via gpsimd.partition_broadcast instead of a DRAM round-trip.
  - attention@V stacks head pairs on 128 PSUM partitions; Wo is
    pair-stacked so the output projection contracts 128 rows per step.
  - engines execute in-order, so emission order IS the schedule: x/wq
    load first, the band of head h+1 and the attn@V of the previous pair
    are interleaved into head h's per-q-block score loop.
"""
import numpy as np
from contextlib import ExitStack

import concourse.bass as bass
import concourse.bacc as bacc
import concourse.tile as tile
from concourse import mybir
from concourse import masks
from concourse.bass_utils import run_bass_kernel_spmd

F32 = mybir.dt.float32
F16 = mybir.dt.float16
F8 = mybir.dt.float8e4
AF = mybir.ActivationFunctionType
OP = mybir.AluOpType

B, T, D, H, DK = 4, 1024, 512, 8, 64
L = 2 * T - 1
EPS = 1e-3
NH = 4          # heads per core
NP = 2          # head pairs per core
CH = D // 128   # 4 contraction chunks
QB = T // 128   # 8 q blocks
BAND = 1152    # positional band width per q block
PL = L + 2      # padded pT free size (2 zero pad cols)


def _build_program() -> bass.Bass:
    nc = bacc.Bacc("TRN2", target_bir_lowering=False, debug=False)

    # ---- DRAM I/O ----
    xT = nc.dram_tensor("xT", [D, T], F16, kind="ExternalInput")
    posT = nc.dram_tensor("posT", [D, L], F16, kind="ExternalInput")
    wq = nc.dram_tensor("wq", [D, NH * DK], F16, kind="ExternalInput")
    wk = nc.dram_tensor("wk", [D, NH * DK], F16, kind="ExternalInput")
    wv = nc.dram_tensor("wv", [D, NH * DK], F16, kind="ExternalInput")
    wp = nc.dram_tensor("wp", [D, NH * DK], F16, kind="ExternalInput")
    wo = nc.dram_tensor("wo", [128, NP * D], F16, kind="ExternalInput")
    qc_bias = nc.dram_tensor("qc_bias", [128, NP], F32, kind="ExternalInput")
    qp_bias = nc.dram_tensor("qp_bias", [128, NP], F32, kind="ExternalInput")
    k_bias = nc.dram_tensor("k_bias", [128, NP], F32, kind="ExternalInput")
    v_bias = nc.dram_tensor("v_bias", [NH * DK], F32, kind="ExternalInput")
    out_d = nc.dram_tensor("out_partial", [T, D], F16, kind="ExternalOutput")

    # internal scratch: rel-shift bounce, f8, double buffered
    bounce = nc.dram_tensor("bounce", [2, QB, 128, BAND], F8)

    with tile.TileContext(nc) as tc, ExitStack() as ctx:
        sb = ctx.enter_context(tc.tile_pool(name="sb", bufs=1))
        sb2 = ctx.enter_context(tc.tile_pool(name="sb2", bufs=2))
        sb3 = ctx.enter_context(tc.tile_pool(name="sb3", bufs=3))
        sbE = ctx.enter_context(tc.tile_pool(name="sbE", bufs=6))
        # PSUM: sc 2x[128,512]f32 (2 banks) + band 1x[128,1152]f32
        # (3 banks) + et 2x[128,1024]f16 (2 banks) + o 1x[128,512]f32
        # (1 bank) = 8 banks.
        ps_sc = ctx.enter_context(tc.tile_pool(name="ps_sc", bufs=2,
                                               space="PSUM"))
        # band psum: three independently-released tiles so the next band
        # matmul only waits on the one engine that extracts each slice
        ps_b = ctx.enter_context(tc.tile_pool(name="ps_b", bufs=1,
                                              space="PSUM"))
        ps_et = ctx.enter_context(tc.tile_pool(name="ps_et", bufs=2,
                                               space="PSUM"))
        ps_o = ctx.enter_context(tc.tile_pool(name="ps_o", bufs=1,
                                              space="PSUM"))

        # ---- persistent SBUF ----
        xT_sb = sb.tile([128, CH * T], F16)
        yT_sb = sb.tile([128, CH * T], F16)
        posT_sb = sb.tile([128, CH * L + 2], F16)
        pT_sb = sb.tile([128, NP * PL], F8)
        qcT_sb = sb.tile([128, NP * T], F8)
        qpT_sb = sb.tile([128, NP * T], F8)
        kT_sb = sb.tile([128, NP * T], F8)
        v_sb = sb.tile([128, QB * NH * DK], F16)
        oT_sb = sb.tile([128, NP * T], F16)
        wq_sb = sb.tile([128, CH * 256], F16)
        wk_sb = sb.tile([128, CH * 256], F16)
        wv_sb = sb.tile([128, CH * 256], F16)
        wp_sb = sb.tile([128, CH * 256], F16)
        wo_sb = sb.tile([128, NP * D], F16)
        qcb_sb = sb.tile([128, NP], F32)
        qpb_sb = sb.tile([128, NP], F32)
        kb_sb = sb.tile([128, NP], F32)
        vb_sb = sb.tile([128, 256], F32)
        arep = sb.tile([128, T], F32)
        brep = sb.tile([128, T], F32)
        ident16 = sb.tile([128, 128], F16)
        ident8 = sb.tile([128, 128], F8)
        ones_col = sb.tile([128, 1], F16)
        eps_col = sb.tile([1, 1], F32)

        nc.vector.memset(ones_col[:], 1.0)
        nc.vector.memset(eps_col[:], EPS)

        # ---- loads: x first (stats), then pos+wp (p proj), then q/k ----
        for tt in range(2):
            for c in range(CH):
                nc.sync.dma_start(
                    xT_sb[:, c * T + tt * 512: c * T + tt * 512 + 512],
                    xT[c * 128:(c + 1) * 128, tt * 512:(tt + 1) * 512])
        for c in range(CH):
            nc.sync.dma_start(wp_sb[:, c * 256:(c + 1) * 256],
                              wp[c * 128:(c + 1) * 128, :])
            nc.sync.dma_start(posT_sb[:, c * L:(c + 1) * L],
                              posT[c * 128:(c + 1) * 128, :])
        for c in range(CH):
            nc.sync.dma_start(wq_sb[:, c * 256:(c + 1) * 256],
                              wq[c * 128:(c + 1) * 128, :])
            nc.sync.dma_start(wk_sb[:, c * 256:(c + 1) * 256],
                              wk[c * 128:(c + 1) * 128, :])
        nc.sync.dma_start(qcb_sb[:], qc_bias[:])
        nc.sync.dma_start(qpb_sb[:], qp_bias[:])
        nc.sync.dma_start(kb_sb[:], k_bias[:])
        for c in range(CH):
            nc.sync.dma_start(wv_sb[:, c * 256:(c + 1) * 256],
                              wv[c * 128:(c + 1) * 128, :])
        nc.sync.dma_start(wo_sb[:], wo[:])
        nc.sync.dma_start(
            vb_sb[:], bass.AP(v_bias[:].tensor, 0, [[0, 128], [1, 256]]))

        # ---- LayerNorm stats; short [1,512]-row chain (row ops are
        # lane-serial, so every op counts): a = rsqrt(var+eps),
        # b = -mu*a, computed as
        #   t1 = s1*s1; v' = D*s2 - t1; a = Rsqrt(v'/D^2 + eps);
        #   b = (s1 * -1/D) * a
        a_row = sb.tile([1, T], F32)
        b_row = sb.tile([1, T], F32)
        for tt in range(2):
            s1 = ps_b.tile([1, 512], F32, tag="band0")
            for c in range(CH):
                xt = xT_sb[:, c * T + tt * 512: c * T + tt * 512 + 512]
                nc.tensor.matmul(s1[:], ones_col[:], xt,
                                 start=(c == 0), stop=(c == CH - 1))
            s2 = ps_b.tile([1, 512], F32, tag="band1")
            for ci, c in enumerate((0, 2, 1, 3)):
                xsq = sb3.tile([128, 512], F16, tag="xsq", name="xsq")
                xt = xT_sb[:, c * T + tt * 512: c * T + tt * 512 + 512]
                eng = nc.vector if c in (0, 2) else nc.gpsimd
                eng.tensor_tensor(xsq[:], xt, xt, op=OP.mult)
                nc.tensor.matmul(s2[:], ones_col[:], xsq[:],
                                 start=(ci == 0), stop=(ci == CH - 1))
            t1 = sb2.tile([1, 512], F32, tag="t1", name="t1")
            nc.scalar.activation(t1[:], s1[:], AF.Square)
            vv = sb2.tile([1, 512], F32, tag="vv", name="vv")
            nc.vector.scalar_tensor_tensor(vv[:], s2[:], float(D), t1[:],
                                           op0=OP.mult, op1=OP.subtract)
            stdh = sb2.tile([1, 512], F32, tag="stdh", name="stdh")
            nc.scalar.activation(stdh[:], vv[:], AF.Sqrt,
                                 scale=1.0 / (D * D), bias=eps_col[:])
            a_half = a_row[:, tt * 512:(tt + 1) * 512]
            nc.vector.reciprocal(a_half, stdh[:])
            nc.vector.scalar_tensor_tensor(
                b_row[:, tt * 512:(tt + 1) * 512], s1[:], -1.0 / D, a_half,
                op0=OP.mult, op1=OP.mult)
            nc.gpsimd.partition_broadcast(
                arep[:, tt * 512:(tt + 1) * 512], a_half)
            nc.gpsimd.partition_broadcast(
                brep[:, tt * 512:(tt + 1) * 512],
                b_row[:, tt * 512:(tt + 1) * 512])
        masks.make_identity(nc, ident16[:])
        masks.make_identity(nc, ident8[:])

        # ---- LayerNorm apply: yT = xT * a + b (f16), 512-col pieces so
        # the first projections start after the nt=0 halves ----
        for nt in range(2):
            for c in range(CH):
                eng = nc.gpsimd if (c == 1 and nt == 1) else nc.vector
                xs = xT_sb[:, c * T + nt * 512: c * T + nt * 512 + 512]
                ys = yT_sb[:, c * T + nt * 512: c * T + nt * 512 + 512]
                ar = arep[:, nt * 512:(nt + 1) * 512]
                br = brep[:, nt * 512:(nt + 1) * 512]
                eng.tensor_tensor(ys, xs, ar, op=OP.mult)
                eng.tensor_tensor(ys, ys, br, op=OP.add)

        # pad pT columns
        z8 = sb.tile([128, 4], F8)
        nc.vector.memset(z8[:], 0.0)
        zrow = sb.tile([128, 2], F16)
        nc.vector.memset(zrow[:], 0.0)
        nc.vector.tensor_copy(posT_sb[:, CH * L:], zrow[:])

        # ---- projection emitters ----
        # extraction engines are spread (qc->Act, qp->DVE, k/p->Pool,
        # v->DVE) so consecutive users of the psum ping-pong release in
        # parallel queues.
        def q_proj(p, nt, pool=None, ptag="sc"):
            prj = (pool or ps_sc).tile([128, 512], F32, tag=ptag,
                                       name="prj")
            for ci, c in enumerate((0, 2, 1, 3)):
                nc.tensor.matmul(
                    prj[:],
                    wq_sb[:, c * 256 + p * 128: c * 256 + p * 128 + 128],
                    yT_sb[:, c * T + nt * 512: c * T + nt * 512 + 512],
                    start=(c == 0), stop=(c == CH - 1))
            o = p * T + nt * 512
            nc.scalar.activation(qcT_sb[:, o:o + 512], prj[:],
                                 AF.Identity, bias=qcb_sb[:, p:p + 1])
            nc.scalar.activation(qpT_sb[:, o:o + 512], prj[:],
                                 AF.Identity, bias=qpb_sb[:, p:p + 1])

        def k_proj(p, nt, pool=None, ptag="sc"):
            prjk = (pool or ps_sc).tile([128, 512], F32, tag=ptag,
                                        name="prjk")
            for ci, c in enumerate((0, 2, 1, 3)):
                nc.tensor.matmul(
                    prjk[:],
                    wk_sb[:, c * 256 + p * 128: c * 256 + p * 128 + 128],
                    yT_sb[:, c * T + nt * 512: c * T + nt * 512 + 512],
                    start=(ci == 0), stop=(ci == CH - 1))
            nc.scalar.activation(kT_sb[:, p * T + nt * 512:
                                       p * T + nt * 512 + 512],
                                 prjk[:], AF.Identity,
                                 bias=kb_sb[:, p:p + 1])

        def p_proj(p, nt, pool=None, ptag="sc"):
            pps = (pool or ps_sc).tile([128, 512], F32, tag=ptag,
                                       name="pps")
            for ci, c in enumerate((0, 2, 1, 3)):
                nc.tensor.matmul(
                    pps[:],
                    wp_sb[:, c * 256 + p * 128: c * 256 + p * 128 + 128],
                    posT_sb[:, c * L + nt * 512: c * L + nt * 512 + 512],
                    start=(ci == 0), stop=(ci == CH - 1))
            nc.scalar.copy(
                pT_sb[:, p * PL + nt * 512: p * PL + nt * 512 + 512],
                pps[:])
            if nt == 3:
                nc.gpsimd.tensor_copy(pT_sb[:, p * PL + L: (p + 1) * PL],
                                      z8[:, :PL - L])

        def v_proj(t8, pool=None, ptag="sc"):
            vps = (pool or ps_sc).tile([128, 256], F32, tag=ptag,
                                       name="vps")
            for c in range(CH):
                nc.tensor.matmul(
                    vps[:],
                    yT_sb[:, c * T + t8 * 128: c * T + t8 * 128 + 128],
                    wv_sb[:, c * 256:(c + 1) * 256],
                    start=(c == 0), stop=(c == CH - 1))
            nc.vector.tensor_tensor(
                v_sb[:, t8 * 256:(t8 + 1) * 256], vps[:], vb_sb[:],
                op=OP.add)

        # ---- attention emitters ----
        def hsl(tile_, h, lo, w, stride=T):
            p, off = h // 2, (h % 2) * 64
            return tile_[off:off + 64, p * stride + lo: p * stride + lo + w]

        def band_qb(h, qb, shifted):
            """Band scores for (h, qb), bounce out, and the skewed read of
            this qb's block straight back into `shifted`. Three psum tiles,
            one extraction engine each, so the next band matmul waits on
            exactly one engine per tile."""
            ping = h % 2
            s0 = 897 - qb * 128
            qp_blk = hsl(qpT_sb, h, qb * 128, 128)
            bps0 = ps_b.tile([128, 832], F32, tag="band0")
            nc.tensor.matmul(bps0[:, :512], qp_blk,
                             hsl(pT_sb, h, s0, 512, stride=PL),
                             start=True, stop=True)
            nc.tensor.matmul(bps0[:, 512:], qp_blk,
                             hsl(pT_sb, h, s0 + 512, 320, stride=PL),
                             start=True, stop=True)
            bps1 = ps_b.tile([128, 320], F32, tag="band1")
            nc.tensor.matmul(bps1[:], qp_blk,
                             hsl(pT_sb, h, s0 + 832, 320, stride=PL),
                             start=True, stop=True)
            b8 = sbE.tile([128, BAND], F8, tag="band8", name="b8")
            nc.vector.tensor_copy(b8[:, :832], bps0[:])
            nc.scalar.copy(b8[:, 832:], bps1[:])
            nc.sync.dma_start(bounce[ping, qb], b8[:])
            if h == 0:
                # head 0 is latency-critical: read each q-block's shifted
                # band as soon as its bounce lands
                src = bass.AP(bounce[:].tensor,
                              (ping * QB + qb) * (128 * BAND) + 127,
                              [[BAND - 1, 128], [1, T]])
                nc.sync.dma_start(shifted[:, qb * T:(qb + 1) * T], src)
            elif qb % 2 == 1:
                src = bass.AP(bounce[:].tensor,
                              (ping * QB + qb - 1) * (128 * BAND) + 127,
                              [[BAND - 1, 128], [128 * BAND, 2], [1, T]])
                nc.sync.dma_start(
                    shifted[:, (qb - 1) * T:(qb + 1) * T], src)

        def shifted_tile():
            shifted = sb2.tile([128, QB * T], F8, tag="shifted",
                               name="shifted")
            return shifted

        def scores_qb(h, qb, shifted):
            """Scores+exp+normalize for (h, qb); returns the E tile.
            Transposes are emitted one qb later (transpose_qb) so the PE
            queue never waits on the Act/DVE chain of the same qb."""
            E_sb = sbE.tile([128, T], F16, tag="E", name="E")
            den = sb3.tile([128, 2], F32, tag="den", name="den")
            for half in range(2):
                sps = ps_sc.tile([128, 512], F32, tag="sc")
                nc.tensor.matmul(
                    sps[:],
                    hsl(qcT_sb, h, qb * 128, 128),
                    hsl(kT_sb, h, half * 512, 512),
                    start=True, stop=False)
                if qb == 0 and half == 1:
                    # scores[0, 1023] += (q+pos_bias)[1] . p[0]
                    # (the reference rel_shift reshape wraps this element)
                    nc.tensor.matmul(
                        sps[0:1, 511:512],
                        hsl(qpT_sb, h, 1, 1),
                        hsl(pT_sb, h, 0, 1, stride=PL),
                        start=False, stop=False)
                nc.tensor.matmul(
                    sps[:],
                    ident8[:],
                    shifted[:, qb * T + half * 512:
                            qb * T + half * 512 + 512],
                    start=False, stop=True)
                nc.scalar.activation(
                    E_sb[:, half * 512:(half + 1) * 512], sps[:], AF.Exp,
                    scale=0.125, accum_out=den[:, half:half + 1])
            rec = sb3.tile([128, 1], F32, tag="rec", name="rec")
            nc.vector.tensor_tensor(rec[:], den[:, 0:1], den[:, 1:2],
                                    op=OP.add)
            nc.vector.reciprocal(rec[:], rec[:])
            nc.vector.tensor_scalar_mul(E_sb[:], E_sb[:], rec[:])
            return E_sb

        def transpose_qb(qb, E_sb, ET_sb):
            # transpose E (f16): 8 PE transposes -> one f16 psum bank
            etps = ps_et.tile([128, T], F16, tag="et")
            for kc in range(QB):
                nc.tensor.transpose(
                    etps[:, kc * 128: kc * 128 + 128],
                    E_sb[:, kc * 128: kc * 128 + 128],
                    ident16[:])
            # scatter-extract: ET_sb[:, kc*T + qb*128 ...] = etps block kc
            dst = bass.AP(ET_sb[:].tensor, qb * 128,
                          [[QB * T, 128], [T, QB], [1, 128]])
            nc.vector.tensor_copy(dst, etps[:])

        def attnv_chunk(p, ib, hh, kh, ET_pair, otps):
            hloc = 2 * p + hh
            for kc in range(4 * kh, 4 * kh + 4):
                nc.tensor.matmul(
                    otps[hh * 64:hh * 64 + 64, :],
                    v_sb[:, kc * 256 + hloc * 64: kc * 256 + hloc * 64 + 64],
                    ET_pair[hh][:, kc * T + ib * 512: kc * T + ib * 512 + 512],
                    start=(kc == 0), stop=(kc == QB - 1))

        def attnv_extract(p, ib, otps):
            dst = oT_sb[:, p * T + ib * 512: p * T + ib * 512 + 512]
            if (p + ib) % 2 == 0:
                nc.vector.tensor_copy(dst, otps[:])
            else:
                nc.scalar.copy(dst, otps[:])

        def out_proj(t8):
            ops_ = ps_sc.tile([128, 512], F32, tag="sc")
            for p in range(NP):
                nc.tensor.matmul(
                    ops_[:],
                    oT_sb[:, p * T + t8 * 128: p * T + t8 * 128 + 128],
                    wo_sb[:, p * D:(p + 1) * D],
                    start=(p == 0), stop=(p == NP - 1))
            osb = sbE.tile([128, 512], F16, tag="osb", name="osb")
            if t8 % 2 == 0:
                nc.vector.tensor_copy(osb[:], ops_[:])
            else:
                nc.scalar.copy(osb[:], ops_[:])
            nc.sync.dma_start(out_d[t8 * 128:(t8 + 1) * 128, :], osb[:])

        # ---- emission schedule ----
        # p proj first (needs no LayerNorm -> fills the stats-chain
        # latency). Only pair-0 projections + head-0 bands run before the
        # attention loop; pair-1 projections and v-proj are deferred as
        # per-q-block filler inside the head-0/1 loops so the in-order PE
        # queue reaches head-0 scores ~12us earlier.
        for nt in range(4):
            p_proj(0, nt)
        q_proj(0, 0)
        q_proj(0, 1)
        shifted = {0: shifted_tile()}
        for qb in range(QB):
            band_qb(0, qb, shifted[0])
        k_proj(0, 0)
        k_proj(0, 1)

        filler = {
            0: [lambda: q_proj(1, 0, ps_o, "o"),
                lambda: q_proj(1, 1, ps_o, "o"),
                lambda: k_proj(1, 0, ps_o, "o"),
                lambda: k_proj(1, 1, ps_o, "o"),
                lambda: p_proj(1, 0, ps_o, "o"),
                lambda: p_proj(1, 1, ps_o, "o"),
                lambda: p_proj(1, 2, ps_o, "o"),
                lambda: p_proj(1, 3, ps_o, "o")],
            1: [(lambda t8=t8: v_proj(t8, ps_o, "o")) for t8 in range(QB)],
        }

        ET_tiles = {}
        attnv_work = []   # deferred attn@V chunks for the previous pair
        for h in range(NH):
            # bufs=3: head h+2 must not wait on the deferred attn@V reads
            # of head h's ET (they interleave into head h+2's score loop)
            ET_sb = sb3.tile([128, QB * T], F16, tag="ET", name="ET")
            ET_tiles[h] = ET_sb
            if h + 1 < NH:
                shifted[h + 1] = shifted_tile()
            E_hist = []
            for qb in range(QB):
                if h + 1 < NH:
                    band_qb(h + 1, qb, shifted[h + 1])
                E_hist.append(scores_qb(h, qb, shifted[h]))
                if qb >= 2:
                    transpose_qb(qb - 2, E_hist[qb - 2], ET_sb)
                for fn_ in filler.get(h, [])[qb:qb + 1]:
                    fn_()
                # slot one deferred attn@V chunk of the previous pair
                # (head 2's first two slots skip so two chunks spill into
                # head 3's otherwise filler-free loop)
                if attnv_work and not (h == 2 and qb < 2):
                    attnv_work.pop(0)()
            transpose_qb(QB - 2, E_hist[QB - 2], ET_sb)
            transpose_qb(QB - 1, E_hist[QB - 1], ET_sb)
            if h % 2 == 1:
                p = h // 2
                ET_pair = (ET_tiles[2 * p], ET_tiles[2 * p + 1])
                chunks = []
                state = {}

                def make_chunk(p_, ib_, hh_, kh_, ET_pair_):
                    def go():
                        if ("ot", p_, ib_) not in state:
                            state[("ot", p_, ib_)] = ps_o.tile(
                                [128, 512], F32, tag="o", name="otps")
                        otps = state[("ot", p_, ib_)]
                        attnv_chunk(p_, ib_, hh_, kh_, ET_pair_, otps)
                        if hh_ == 1 and kh_ == 1:
                            attnv_extract(p_, ib_, otps)
                    return go

                for ib in range(2):
                    for hh in range(2):
                        for kh in range(2):
                            chunks.append(make_chunk(p, ib, hh, kh, ET_pair))
                if h == NH - 1:
                    for cfn in chunks[:4]:
                        cfn()
                    for i, cfn in enumerate(chunks[4:]):
                        out_proj(i)
                        cfn()
                    for t8 in range(4, QB):
                        out_proj(t8)
                else:
                    attnv_work.extend(chunks)

    nc.compile()
    return nc


_PROGRAM_CACHE: dict = {}


def _get_program() -> bass.Bass:
    if "nc" not in _PROGRAM_CACHE:
        _PROGRAM_CACHE["nc"] = _build_program()
    return _PROGRAM_CACHE["nc"]


def _prepare_in_maps(x, pos, content_bias, pos_bias, gamma, beta,
                     Wq, bq, Wk, bk, Wv, bv, Wp, Wo, bo):
    x = np.asarray(x, np.float32)
    pos = np.asarray(pos, np.float32)
    gamma = np.asarray(gamma, np.float32)
    beta = np.asarray(beta, np.float32)

    # gamma folding: y = yln*gamma + beta  =>  y@W = yln@(gamma*W) + beta@W
    def fold(W):
        W = np.asarray(W, np.float32)
        return W * gamma[:, None, None], np.einsum("d,dhk->hk", beta, W)

    Wq_f, bq_f = fold(Wq)
    Wk_f, bk_f = fold(Wk)
    Wv_f, bv_f = fold(Wv)
    Wp = np.asarray(Wp, np.float32)
    Wo = np.asarray(Wo, np.float32)

    in_maps = []
    for core in range(8):
        b = core // 2
        g = core % 2
        hs = slice(4 * g, 4 * g + 4)
        qcb = (np.asarray(bq) + np.asarray(content_bias) + bq_f)[hs]
        qpb = (np.asarray(bq) + np.asarray(pos_bias) + bq_f)[hs]
        kb = (np.asarray(bk) + bk_f)[hs]
        vb = (np.asarray(bv) + bv_f)[hs]
        # Wo pair-stacked: [128, NP*D]; pair p rows = Wo[2p] ++ Wo[2p+1]
        Wo_h = np.asarray(Wo)[hs]          # [4, DK, D]
        wo2 = np.concatenate(
            [np.concatenate([Wo_h[2 * p], Wo_h[2 * p + 1]], axis=0)
             for p in range(NP)], axis=1)  # [128, NP*D]
        in_maps.append({
            "xT": np.ascontiguousarray(x[b].T).astype(np.float16),
            "posT": np.ascontiguousarray(pos[b].T).astype(np.float16),
            "wq": np.ascontiguousarray(
                Wq_f[:, hs, :].reshape(D, NH * DK)).astype(np.float16),
            "wk": np.ascontiguousarray(
                Wk_f[:, hs, :].reshape(D, NH * DK)).astype(np.float16),
            "wv": np.ascontiguousarray(
                Wv_f[:, hs, :].reshape(D, NH * DK)).astype(np.float16),
            "wp": np.ascontiguousarray(
                Wp[:, hs, :].reshape(D, NH * DK)).astype(np.float16),
            "wo": np.ascontiguousarray(wo2).astype(np.float16),
            "qc_bias": np.ascontiguousarray(qcb.reshape(2, 128).T),
            "qp_bias": np.ascontiguousarray(qpb.reshape(2, 128).T),
            "k_bias": np.ascontiguousarray(kb.reshape(2, 128).T),
            "v_bias": np.ascontiguousarray(vb.reshape(NH * DK)),
        })

    return in_maps


def _combine(x, bo, results):
    parts = [r["out_partial"] for r in results]
    out = np.asarray(x, np.float32) + np.asarray(bo, np.float32)[None, None, :]
    for b in range(B):
        out[b] += parts[2 * b].astype(np.float32)
        out[b] += parts[2 * b + 1].astype(np.float32)
    return out.astype(np.float32)


def kernel(x, pos, content_bias, pos_bias, gamma, beta,
           Wq, bq, Wk, bk, Wv, bv, Wp, Wo, bo) -> np.ndarray:
    in_maps = _prepare_in_maps(x, pos, content_bias, pos_bias, gamma, beta,
                               Wq, bq, Wk, bk, Wv, bv, Wp, Wo, bo)
    nc = _get_program()
    res = run_bass_kernel_spmd(nc, in_maps, core_ids=list(range(8)))
    return _combine(x, bo, res.results)


# revision 14
# speedup vs baseline: 166.5488x; 1.0009x over previous
"""Trainium2 Bass kernel v2 for Transformer-XL style MHSA (nn_MHSAModule).

Problem (hardcoded):
  B=4, T=1024, D=512, H=8, DK=64, L=2*T-1=2047, eps=1e-3
  out = x + (MHSA(LayerNorm(x), pos) @ Wo + bo)

Sharding: 8 cores = 4 batches x 2 head-groups (4 heads each). Each core
returns a partial output [T, D] f16 (its heads' contribution); the host
sums the two partials per batch and adds the residual x + bo.

v2 design notes (vs v1):
  - f16 activations/weights on the matmul path; f8e4m3 only for matmul
    B-operands no vector engine reads back: kT, pT, and the rel-shift
    band bounce (halves its DMA volume).
  - exp runs on Act straight from PSUM with accum_out giving the softmax
    denominator for free; E is normalized by one per-partition f16
    tensor_scalar instead of v1's psum-copy + scale chain.
  - PSUM extraction is the scarce resource (~4x an SBUF read): band
    extraction is split across DVE/Act/Pool, scores are extracted by the
    exp itself, ET extracted as f16 (2x cheaper than f32).
  - LayerNorm stats via f16 ones-matmuls; a/b rows reach all partitions
    via gpsimd.partition_broadcast instead of a DRAM round-trip.
  - attention@V stacks head pairs on 128 PSUM partitions; Wo is
    pair-stacked so the output projection contracts 128 rows per step.
  - engines execute in-order, so emission order IS the schedule: x/wq
    load first, the band of head h+1 and the attn@V of the previous pair
    are interleaved into head h's per-q-block score loop.
"""
import numpy as np
from contextlib import ExitStack

import concourse.bass as bass
import concourse.bacc as bacc
import concourse.tile as tile
from concourse import mybir
from concourse import masks
from concourse.bass_utils import run_bass_kernel_spmd

F32 = mybir.dt.float32
F16 = mybir.dt.float16
F8 = mybir.dt.float8e4
AF = mybir.ActivationFunctionType
OP = mybir.AluOpType

B, T, D, H, DK = 4, 1024, 512, 8, 64
L = 2 * T - 1
EPS = 1e-3
NH = 4          # heads per core
NP = 2          # head pairs per core
CH = D // 128   # 4 contraction chunks
QB = T // 128   # 8 q blocks
BAND = 1152    # positional band width per q block
PL = L + 2      # padded pT free size (2 zero pad cols)


def _build_program() -> bass.Bass:
    nc = bacc.Bacc("TRN2", target_bir_lowering=False, debug=False)

    # ---- DRAM I/O ----
    xT = nc.dram_tensor("xT", [D, T], F16, kind="ExternalInput")
    posT = nc.dram_tensor("posT", [D, L], F16, kind="ExternalInput")
    wq = nc.dram_tensor("wq", [D, NH * DK], F16, kind="ExternalInput")
    wk = nc.dram_tensor("wk", [D, NH * DK], F16, kind="ExternalInput")
    wv = nc.dram_tensor("wv", [D, NH * DK], F16, kind="ExternalInput")
    wp = nc.dram_tensor("wp", [D, NH * DK], F16, kind="ExternalInput")
    wo = nc.dram_tensor("wo", [128, NP * D], F16, kind="ExternalInput")
    qc_bias = nc.dram_tensor("qc_bias", [128, NP], F32, kind="ExternalInput")
    qp_bias = nc.dram_tensor("qp_bias", [128, NP], F32, kind="ExternalInput")
    k_bias = nc.dram_tensor("k_bias", [128, NP], F32, kind="ExternalInput")
    v_bias = nc.dram_tensor("v_bias", [NH * DK], F32, kind="ExternalInput")
    out_d = nc.dram_tensor("out_partial", [T, D], F16, kind="ExternalOutput")

    # internal scratch: rel-shift bounce, f8, double buffered
    bounce = nc.dram_tensor("bounce", [2, QB, 128, BAND], F8)

    with tile.TileContext(nc) as tc, ExitStack() as ctx:
        sb = ctx.enter_context(tc.tile_pool(name="sb", bufs=1))
        sb2 = ctx.enter_context(tc.tile_pool(name="sb2", bufs=2))
        sb3 = ctx.enter_context(tc.tile_pool(name="sb3", bufs=3))
        sbE = ctx.enter_context(tc.tile_pool(name="sbE", bufs=8))
        # PSUM: sc 2x[128,512]f32 (2 banks) + band 1x[128,1152]f32
        # (3 banks) + et 2x[128,1024]f16 (2 banks) + o 1x[128,512]f32
        # (1 bank) = 8 banks.
        ps_sc = ctx.enter_context(tc.tile_pool(name="ps_sc", bufs=2,
                                               space="PSUM"))
        # band psum: three independently-released tiles so the next band
        # matmul only waits on the one engine that extracts each slice
        ps_b = ctx.enter_context(tc.tile_pool(name="ps_b", bufs=1,
                                              space="PSUM"))
        ps_et = ctx.enter_context(tc.tile_pool(name="ps_et", bufs=2,
                                               space="PSUM"))
        ps_o = ctx.enter_context(tc.tile_pool(name="ps_o", bufs=1,
                                              space="PSUM"))

        # ---- persistent SBUF ----
        xT_sb = sb.tile([128, CH * T], F16)
        yT_sb = sb.tile([128, CH * T], F16)
        posT_sb = sb.tile([128, CH * L + 2], F16)
        pT_sb = sb.tile([128, NP * PL], F8)
        qcT_sb = sb.tile([128, NP * T], F8)
        qpT_sb = sb.tile([128, NP * T], F8)
        kT_sb = sb.tile([128, NP * T], F8)
        v_sb = sb.tile([128, QB * NH * DK], F16)
        oT_sb = sb.tile([128, NP * T], F16)
        wq_sb = sb.tile([128, CH * 256], F16)
        wk_sb = sb.tile([128, CH * 256], F16)
        wv_sb = sb.tile([128, CH * 256], F16)
        wp_sb = sb.tile([128, CH * 256], F16)
        wo_sb = sb.tile([128, NP * D], F16)
        qcb_sb = sb.tile([128, NP], F32)
        qpb_sb = sb.tile([128, NP], F32)
        kb_sb = sb.tile([128, NP], F32)
        vb_sb = sb.tile([128, 256], F32)
        arep = sb.tile([128, T], F32)
        brep = sb.tile([128, T], F32)
        ident16 = sb.tile([128, 128], F16)
        ident8 = sb.tile([128, 128], F8)
        ones_col = sb.tile([128, 1], F16)
        eps_col = sb.tile([1, 1], F32)

        nc.vector.memset(ones_col[:], 1.0)
        nc.vector.memset(eps_col[:], EPS)

        # ---- loads: x first (stats), then pos+wp (p proj), then q/k ----
        for tt in range(2):
            for c in range(CH):
                nc.sync.dma_start(
                    xT_sb[:, c * T + tt * 512: c * T + tt * 512 + 512],
                    xT[c * 128:(c + 1) * 128, tt * 512:(tt + 1) * 512])
        for c in range(CH):
            nc.sync.dma_start(wp_sb[:, c * 256:(c + 1) * 256],
                              wp[c * 128:(c + 1) * 128, :])
            nc.sync.dma_start(posT_sb[:, c * L:(c + 1) * L],
                              posT[c * 128:(c + 1) * 128, :])
        for c in range(CH):
            nc.sync.dma_start(wq_sb[:, c * 256:(c + 1) * 256],
                              wq[c * 128:(c + 1) * 128, :])
            nc.sync.dma_start(wk_sb[:, c * 256:(c + 1) * 256],
                              wk[c * 128:(c + 1) * 128, :])
        nc.sync.dma_start(qcb_sb[:], qc_bias[:])
        nc.sync.dma_start(qpb_sb[:], qp_bias[:])
        nc.sync.dma_start(kb_sb[:], k_bias[:])
        for c in range(CH):
            nc.sync.dma_start(wv_sb[:, c * 256:(c + 1) * 256],
                              wv[c * 128:(c + 1) * 128, :])
        nc.sync.dma_start(wo_sb[:], wo[:])
        nc.sync.dma_start(
            vb_sb[:], bass.AP(v_bias[:].tensor, 0, [[0, 128], [1, 256]]))

        # ---- LayerNorm stats; short [1,512]-row chain (row ops are
        # lane-serial, so every op counts): a = rsqrt(var+eps),
        # b = -mu*a, computed as
        #   t1 = s1*s1; v' = D*s2 - t1; a = Rsqrt(v'/D^2 + eps);
        #   b = (s1 * -1/D) * a
        a_row = sb.tile([1, T], F32)
        b_row = sb.tile([1, T], F32)
        for tt in range(2):
            s1 = ps_b.tile([1, 512], F32, tag="band0")
            for c in range(CH):
                xt = xT_sb[:, c * T + tt * 512: c * T + tt * 512 + 512]
                nc.tensor.matmul(s1[:], ones_col[:], xt,
                                 start=(c == 0), stop=(c == CH - 1))
            s2 = ps_b.tile([1, 512], F32, tag="band1")
            for ci, c in enumerate((0, 2, 1, 3)):
                xsq = sb3.tile([128, 512], F16, tag="xsq", name="xsq")
                xt = xT_sb[:, c * T + tt * 512: c * T + tt * 512 + 512]
                eng = nc.vector if c in (0, 2) else nc.gpsimd
                eng.tensor_tensor(xsq[:], xt, xt, op=OP.mult)
                nc.tensor.matmul(s2[:], ones_col[:], xsq[:],
                                 start=(ci == 0), stop=(ci == CH - 1))
            t1 = sb2.tile([1, 512], F32, tag="t1", name="t1")
            nc.scalar.activation(t1[:], s1[:], AF.Square)
            vv = sb2.tile([1, 512], F32, tag="vv", name="vv")
            nc.vector.scalar_tensor_tensor(vv[:], s2[:], float(D), t1[:],
                                           op0=OP.mult, op1=OP.subtract)
            stdh = sb2.tile([1, 512], F32, tag="stdh", name="stdh")
            nc.scalar.activation(stdh[:], vv[:], AF.Sqrt,
                                 scale=1.0 / (D * D), bias=eps_col[:])
            a_half = a_row[:, tt * 512:(tt + 1) * 512]
            nc.vector.reciprocal(a_half, stdh[:])
            nc.vector.scalar_tensor_tensor(
                b_row[:, tt * 512:(tt + 1) * 512], s1[:], -1.0 / D, a_half,
                op0=OP.mult, op1=OP.mult)
            nc.gpsimd.partition_broadcast(
                arep[:, tt * 512:(tt + 1) * 512], a_half)
            nc.gpsimd.partition_broadcast(
                brep[:, tt * 512:(tt + 1) * 512],
                b_row[:, tt * 512:(tt + 1) * 512])
        masks.make_identity(nc, ident16[:])
        masks.make_identity(nc, ident8[:])

        # ---- LayerNorm apply: yT = xT * a + b (f16), 512-col pieces so
        # the first projections start after the nt=0 halves ----
        for nt in range(2):
            for c in range(CH):
                eng = nc.gpsimd if (c == 1 and nt == 1) else nc.vector
                xs = xT_sb[:, c * T + nt * 512: c * T + nt * 512 + 512]
                ys = yT_sb[:, c * T + nt * 512: c * T + nt * 512 + 512]
                ar = arep[:, nt * 512:(nt + 1) * 512]
                br = brep[:, nt * 512:(nt + 1) * 512]
                eng.tensor_tensor(ys, xs, ar, op=OP.mult)
                eng.tensor_tensor(ys, ys, br, op=OP.add)

        # pad pT columns
        z8 = sb.tile([128, 4], F8)
        nc.vector.memset(z8[:], 0.0)
        zrow = sb.tile([128, 2], F16)
        nc.vector.memset(zrow[:], 0.0)
        nc.vector.tensor_copy(posT_sb[:, CH * L:], zrow[:])

        # ---- projection emitters ----
        # extraction engines are spread (qc->Act, qp->DVE, k/p->Pool,
        # v->DVE) so consecutive users of the psum ping-pong release in
        # parallel queues.
        def q_proj(p, nt, pool=None, ptag="sc"):
            prj = (pool or ps_sc).tile([128, 512], F32, tag=ptag,
                                       name="prj")
            for ci, c in enumerate((0, 2, 1, 3)):
                nc.tensor.matmul(
                    prj[:],
                    wq_sb[:, c * 256 + p * 128: c * 256 + p * 128 + 128],
                    yT_sb[:, c * T + nt * 512: c * T + nt * 512 + 512],
                    start=(c == 0), stop=(c == CH - 1))
            o = p * T + nt * 512
            nc.scalar.activation(qcT_sb[:, o:o + 512], prj[:],
                                 AF.Identity, bias=qcb_sb[:, p:p + 1])
            nc.scalar.activation(qpT_sb[:, o:o + 512], prj[:],
                                 AF.Identity, bias=qpb_sb[:, p:p + 1])

        def k_proj(p, nt, pool=None, ptag="sc"):
            prjk = (pool or ps_sc).tile([128, 512], F32, tag=ptag,
                                        name="prjk")
            for ci, c in enumerate((0, 2, 1, 3)):
                nc.tensor.matmul(
                    prjk[:],
                    wk_sb[:, c * 256 + p * 128: c * 256 + p * 128 + 128],
                    yT_sb[:, c * T + nt * 512: c * T + nt * 512 + 512],
                    start=(ci == 0), stop=(ci == CH - 1))
            nc.scalar.activation(kT_sb[:, p * T + nt * 512:
                                       p * T + nt * 512 + 512],
                                 prjk[:], AF.Identity,
                                 bias=kb_sb[:, p:p + 1])

        def p_proj(p, nt, pool=None, ptag="sc"):
            pps = (pool or ps_sc).tile([128, 512], F32, tag=ptag,
                                       name="pps")
            for ci, c in enumerate((0, 2, 1, 3)):
                nc.tensor.matmul(
                    pps[:],
                    wp_sb[:, c * 256 + p * 128: c * 256 + p * 128 + 128],
                    posT_sb[:, c * L + nt * 512: c * L + nt * 512 + 512],
                    start=(ci == 0), stop=(ci == CH - 1))
            nc.scalar.copy(
                pT_sb[:, p * PL + nt * 512: p * PL + nt * 512 + 512],
                pps[:])
            if nt == 3:
                nc.gpsimd.tensor_copy(pT_sb[:, p * PL + L: (p + 1) * PL],
                                      z8[:, :PL - L])

        def v_proj(t8, pool=None, ptag="sc"):
            vps = (pool or ps_sc).tile([128, 256], F32, tag=ptag,
                                       name="vps")
            for c in range(CH):
                nc.tensor.matmul(
                    vps[:],
                    yT_sb[:, c * T + t8 * 128: c * T + t8 * 128 + 128],
                    wv_sb[:, c * 256:(c + 1) * 256],
                    start=(c == 0), stop=(c == CH - 1))
            nc.vector.tensor_tensor(
                v_sb[:, t8 * 256:(t8 + 1) * 256], vps[:], vb_sb[:],
                op=OP.add)

        # ---- attention emitters ----
        def hsl(tile_, h, lo, w, stride=T):
            p, off = h // 2, (h % 2) * 64
            return tile_[off:off + 64, p * stride + lo: p * stride + lo + w]

        def band_qb(h, qb, shifted):
            """Band scores for (h, qb), bounce out, and the skewed read of
            this qb's block straight back into `shifted`. Three psum tiles,
            one extraction engine each, so the next band matmul waits on
            exactly one engine per tile."""
            ping = h % 2
            s0 = 897 - qb * 128
            qp_blk = hsl(qpT_sb, h, qb * 128, 128)
            bps0 = ps_b.tile([128, 832], F32, tag="band0")
            nc.tensor.matmul(bps0[:, :512], qp_blk,
                             hsl(pT_sb, h, s0, 512, stride=PL),
                             start=True, stop=True)
            nc.tensor.matmul(bps0[:, 512:], qp_blk,
                             hsl(pT_sb, h, s0 + 512, 320, stride=PL),
                             start=True, stop=True)
            bps1 = ps_b.tile([128, 320], F32, tag="band1")
            nc.tensor.matmul(bps1[:], qp_blk,
                             hsl(pT_sb, h, s0 + 832, 320, stride=PL),
                             start=True, stop=True)
            b8 = sbE.tile([128, BAND], F8, tag="band8", name="b8")
            nc.vector.tensor_copy(b8[:, :832], bps0[:])
            nc.scalar.copy(b8[:, 832:], bps1[:])
            nc.sync.dma_start(bounce[ping, qb], b8[:])
            if h == 0:
                # head 0 is latency-critical: read each q-block's shifted
                # band as soon as its bounce lands
                src = bass.AP(bounce[:].tensor,
                              (ping * QB + qb) * (128 * BAND) + 127,
                              [[BAND - 1, 128], [1, T]])
                nc.sync.dma_start(shifted[:, qb * T:(qb + 1) * T], src)
            elif qb % 2 == 1:
                src = bass.AP(bounce[:].tensor,
                              (ping * QB + qb - 1) * (128 * BAND) + 127,
                              [[BAND - 1, 128], [128 * BAND, 2], [1, T]])
                nc.sync.dma_start(
                    shifted[:, (qb - 1) * T:(qb + 1) * T], src)

        def shifted_tile():
            shifted = sb2.tile([128, QB * T], F8, tag="shifted",
                               name="shifted")
            return shifted

        def scores_qb(h, qb, shifted):
            """Scores+exp+normalize for (h, qb); returns the E tile.
            Transposes are emitted one qb later (transpose_qb) so the PE
            queue never waits on the Act/DVE chain of the same qb."""
            E_sb = sbE.tile([128, T], F16, tag="E", name="E")
            den = sb3.tile([128, 2], F32, tag="den", name="den")
            for half in range(2):
                sps = ps_sc.tile([128, 512], F32, tag="sc")
                nc.tensor.matmul(
                    sps[:],
                    hsl(qcT_sb, h, qb * 128, 128),
                    hsl(kT_sb, h, half * 512, 512),
                    start=True, stop=False)
                if qb == 0 and half == 1:
                    # scores[0, 1023] += (q+pos_bias)[1] . p[0]
                    # (the reference rel_shift reshape wraps this element)
                    nc.tensor.matmul(
                        sps[0:1, 511:512],
                        hsl(qpT_sb, h, 1, 1),
                        hsl(pT_sb, h, 0, 1, stride=PL),
                        start=False, stop=False)
                nc.tensor.matmul(
                    sps[:],
                    ident8[:],
                    shifted[:, qb * T + half * 512:
                            qb * T + half * 512 + 512],
                    start=False, stop=True)
                nc.scalar.activation(
                    E_sb[:, half * 512:(half + 1) * 512], sps[:], AF.Exp,
                    scale=0.125, accum_out=den[:, half:half + 1])
            rec = sb3.tile([128, 1], F32, tag="rec", name="rec")
            nc.vector.tensor_tensor(rec[:], den[:, 0:1], den[:, 1:2],
                                    op=OP.add)
            nc.vector.reciprocal(rec[:], rec[:])
            nc.vector.tensor_scalar_mul(E_sb[:], E_sb[:], rec[:])
            return E_sb

        def transpose_qb(qb, E_sb, ET_sb):
            # transpose E (f16): 8 PE transposes -> one f16 psum bank
            etps = ps_et.tile([128, T], F16, tag="et")
            for kc in range(QB):
                nc.tensor.transpose(
                    etps[:, kc * 128: kc * 128 + 128],
                    E_sb[:, kc * 128: kc * 128 + 128],
                    ident16[:])
            # scatter-extract: ET_sb[:, kc*T + qb*128 ...] = etps block kc
            dst = bass.AP(ET_sb[:].tensor, qb * 128,
                          [[QB * T, 128], [T, QB], [1, 128]])
            nc.vector.tensor_copy(dst, etps[:])

        def attnv_chunk(p, ib, hh, kh, ET_pair, otps):
            hloc = 2 * p + hh
            for kc in range(4 * kh, 4 * kh + 4):
                nc.tensor.matmul(
                    otps[hh * 64:hh * 64 + 64, :],
                    v_sb[:, kc * 256 + hloc * 64: kc * 256 + hloc * 64 + 64],
                    ET_pair[hh][:, kc * T + ib * 512: kc * T + ib * 512 + 512],
                    start=(kc == 0), stop=(kc == QB - 1))

        def attnv_extract(p, ib, otps):
            dst = oT_sb[:, p * T + ib * 512: p * T + ib * 512 + 512]
            if (p + ib) % 2 == 0:
                nc.vector.tensor_copy(dst, otps[:])
            else:
                nc.scalar.copy(dst, otps[:])

        def out_proj(t8):
            ops_ = ps_sc.tile([128, 512], F32, tag="sc")
            for p in range(NP):
                nc.tensor.matmul(
                    ops_[:],
                    oT_sb[:, p * T + t8 * 128: p * T + t8 * 128 + 128],
                    wo_sb[:, p * D:(p + 1) * D],
                    start=(p == 0), stop=(p == NP - 1))
            osb = sbE.tile([128, 512], F16, tag="osb", name="osb")
            if t8 % 2 == 0:
                nc.vector.tensor_copy(osb[:], ops_[:])
            else:
                nc.scalar.copy(osb[:], ops_[:])
            nc.sync.dma_start(out_d[t8 * 128:(t8 + 1) * 128, :], osb[:])

        # ---- emission schedule ----
        # p proj first (needs no LayerNorm -> fills the stats-chain
        # latency). Only pair-0 projections + head-0 bands run before the
        # attention loop; pair-1 projections and v-proj are deferred as
        # per-q-block filler inside the head-0/1 loops so the in-order PE
        # queue reaches head-0 scores ~12us earlier.
        for nt in range(4):
            p_proj(0, nt)
        q_proj(0, 0)
        q_proj(0, 1)
        shifted = {0: shifted_tile()}
        for qb in range(QB):
            band_qb(0, qb, shifted[0])
        k_proj(0, 0)
        k_proj(0, 1)

        filler = {
            0: [lambda: q_proj(1, 0, ps_o, "o"),
                lambda: q_proj(1, 1, ps_o, "o"),
                lambda: k_proj(1, 0, ps_o, "o"),
                lambda: k_proj(1, 1, ps_o, "o"),
                lambda: p_proj(1, 0, ps_o, "o"),
                lambda: p_proj(1, 1, ps_o, "o"),
                lambda: p_proj(1, 2, ps_o, "o"),
                lambda: p_proj(1, 3, ps_o, "o")],
            1: [(lambda t8=t8: v_proj(t8, ps_o, "o")) for t8 in range(QB)],
        }

        ET_tiles = {}
        attnv_work = []   # deferred attn@V chunks for the previous pair
        for h in range(NH):
            # bufs=3: head h+2 must not wait on the deferred attn@V reads
            # of head h's ET (they interleave into head h+2's score loop)
            ET_sb = sb3.tile([128, QB * T], F16, tag="ET", name="ET")
            ET_tiles[h] = ET_sb
            if h + 1 < NH:
                shifted[h + 1] = shifted_tile()
            E_hist = []
            for qb in range(QB):
                if h + 1 < NH:
                    band_qb(h + 1, qb, shifted[h + 1])
                E_hist.append(scores_qb(h, qb, shifted[h]))
                if qb >= 2:
                    transpose_qb(qb - 2, E_hist[qb - 2], ET_sb)
                for fn_ in filler.get(h, [])[qb:qb + 1]:
                    fn_()
                # slot one deferred attn@V chunk of the previous pair
                # (head 2's first two slots skip so two chunks spill into
                # head 3's otherwise filler-free loop)
                if attnv_work and not (h == 2 and qb < 2):
                    attnv_work.pop(0)()
            transpose_qb(QB - 2, E_hist[QB - 2], ET_sb)
            transpose_qb(QB - 1, E_hist[QB - 1], ET_sb)
            if h % 2 == 1:
                p = h // 2
                ET_pair = (ET_tiles[2 * p], ET_tiles[2 * p + 1])
                chunks = []
                state = {}

                def make_chunk(p_, ib_, hh_, kh_, ET_pair_):
                    def go():
                        if ("ot", p_, ib_) not in state:
                            state[("ot", p_, ib_)] = ps_o.tile(
                                [128, 512], F32, tag="o", name="otps")
                        otps = state[("ot", p_, ib_)]
                        attnv_chunk(p_, ib_, hh_, kh_, ET_pair_, otps)
                        if hh_ == 1 and kh_ == 1:
                            attnv_extract(p_, ib_, otps)
                    return go

                for ib in range(2):
                    for hh in range(2):
                        for kh in range(2):
                            chunks.append(make_chunk(p, ib, hh, kh, ET_pair))
                if h == NH - 1:
                    for cfn in chunks[:4]:
                        cfn()
                    for i, cfn in enumerate(chunks[4:]):
                        out_proj(i)
                        cfn()
                    for t8 in range(4, QB):
                        out_proj(t8)
                else:
                    attnv_work.extend(chunks)

    nc.compile()
    return nc


_PROGRAM_CACHE: dict = {}


def _get_program() -> bass.Bass:
    if "nc" not in _PROGRAM_CACHE:
        _PROGRAM_CACHE["nc"] = _build_program()
    return _PROGRAM_CACHE["nc"]


def _prepare_in_maps(x, pos, content_bias, pos_bias, gamma, beta,
                     Wq, bq, Wk, bk, Wv, bv, Wp, Wo, bo):
    x = np.asarray(x, np.float32)
    pos = np.asarray(pos, np.float32)
    gamma = np.asarray(gamma, np.float32)
    beta = np.asarray(beta, np.float32)

    # gamma folding: y = yln*gamma + beta  =>  y@W = yln@(gamma*W) + beta@W
    def fold(W):
        W = np.asarray(W, np.float32)
        return W * gamma[:, None, None], np.einsum("d,dhk->hk", beta, W)

    Wq_f, bq_f = fold(Wq)
    Wk_f, bk_f = fold(Wk)
    Wv_f, bv_f = fold(Wv)
    Wp = np.asarray(Wp, np.float32)
    Wo = np.asarray(Wo, np.float32)

    in_maps = []
    for core in range(8):
        b = core // 2
        g = core % 2
        hs = slice(4 * g, 4 * g + 4)
        qcb = (np.asarray(bq) + np.asarray(content_bias) + bq_f)[hs]
        qpb = (np.asarray(bq) + np.asarray(pos_bias) + bq_f)[hs]
        kb = (np.asarray(bk) + bk_f)[hs]
        vb = (np.asarray(bv) + bv_f)[hs]
        # Wo pair-stacked: [128, NP*D]; pair p rows = Wo[2p] ++ Wo[2p+1]
        Wo_h = np.asarray(Wo)[hs]          # [4, DK, D]
        wo2 = np.concatenate(
            [np.concatenate([Wo_h[2 * p], Wo_h[2 * p + 1]], axis=0)
             for p in range(NP)], axis=1)  # [128, NP*D]
        in_maps.append({
            "xT": np.ascontiguousarray(x[b].T).astype(np.float16),
            "posT": np.ascontiguousarray(pos[b].T).astype(np.float16),
            "wq": np.ascontiguousarray(
                Wq_f[:, hs, :].reshape(D, NH * DK)).astype(np.float16),
            "wk": np.ascontiguousarray(
                Wk_f[:, hs, :].reshape(D, NH * DK)).astype(np.float16),
            "wv": np.ascontiguousarray(
                Wv_f[:, hs, :].reshape(D, NH * DK)).astype(np.float16),
            "wp": np.ascontiguousarray(
                Wp[:, hs, :].reshape(D, NH * DK)).astype(np.float16),
            "wo": np.ascontiguousarray(wo2).astype(np.float16),
            "qc_bias": np.ascontiguousarray(qcb.reshape(2, 128).T),
            "qp_bias": np.ascontiguousarray(qpb.reshape(2, 128).T),
            "k_bias": np.ascontiguousarray(kb.reshape(2, 128).T),
            "v_bias": np.ascontiguousarray(vb.reshape(NH * DK)),
        })

    return in_maps


def _combine(x, bo, results):
    parts = [r["out_partial"] for r in results]
    out = np.asarray(x, np.float32) + np.asarray(bo, np.float32)[None, None, :]
    for b in range(B):
        out[b] += parts[2 * b].astype(np.float32)
        out[b] += parts[2 * b + 1].astype(np.float32)
    return out.astype(np.float32)


def kernel(x, pos, content_bias, pos_bias, gamma, beta,
           Wq, bq, Wk, bk, Wv, bv, Wp, Wo, bo) -> np.ndarray:
    in_maps = _prepare_in_maps(x, pos, content_bias, pos_bias, gamma, beta,
                               Wq, bq, Wk, bk, Wv, bv, Wp, Wo, bo)
    nc = _get_program()
    res = run_bass_kernel_spmd(nc, in_maps, core_ids=list(range(8)))
    return _combine(x, bo, res.results)
